# revision 27
# baseline (speedup 1.0000x reference)
"""Trainium2 Bass kernel for the AttentionQFunction problem.

Contract: kernel(**inputs) takes FULL inputs (B=256) and returns the FULL
[256] float32 output. Internally the batch is sharded 32-per-core across 8
NeuronCores (pure data parallel); the small MLP weights are replicated.

Math (per batch element b, N=512 obstacles, H=256):
  x      = [obs broadcast (12) | obstacle_data (4)]            [N, 16]
  q,k    = 3-layer MLP(x) (relu between), v = same w/ relu out [N, H]
  S^T    = (k h-chunks)^T-matmuls -> scores^T [keys, queries] (q prescaled 1/16)
  E^T    = exp(S^T + negbias[key])   (key mask; no max-subtract -- scores
           are O(0.2) for this model family, verified on host)
  U      = E^T-chunks^T @ [v | 1 | 1] -> [queries, H+2]; col H is sum_keys E
           (two ones columns: fp32r needs an even moving-free size)
  out    = U[:, :H] * (mask[q] / U[:, H])        per-partition scale
  pooled = max over queries (pairwise max + PE transpose + free-dim max)
  qval   = head MLP([pooled | obs | act])        (head W1 rows reordered)

All MLP layers run in transposed-activation layout [H, N] so every bias is
per-partition. All matmuls use float32r (1 cycle/row for N>=256 vs 4 for
fp32; ~1e-4 rel err). The K=4 L1 matmuls for q/k/v run concurrently in the
PE array via tile_position row groups 0/32/64 (weights+rhs replicated into
those partition bands). Elementwise ops are emitted as nc.any so the Tile
scheduler load-balances them across ScalarE/VectorE; exp stays on ScalarE.
The pool-stage PE transposes of each batch pair are emitted one pair late so
they never stall the next pair's L1/L2 matmuls, and the softmax-denominator
ones-columns are written by the v bias-row matmul itself (no per-batch
constant-write ops). Cost-model (TimelineSim) predicts ~358us/core; on real
silicon the packed L1 (which the model charges serially, ~65us) should land
meaningfully below that.
"""

import numpy as np

import concourse.bass as bass
import concourse.mybir as mybir
import concourse.tile as tile
from concourse import bacc
from concourse.bass_utils import run_bass_kernel_spmd

F32 = mybir.dt.float32
F32R = mybir.dt.float32r
AF = mybir.ActivationFunctionType
OP = mybir.AluOpType

N_CORES = 8
B = 256
BPC = B // N_CORES  # 32 batch elements per core
N = 512             # obstacles
H = 256             # hidden
D_OBS = 12
OBD = 4
ACT_D = 2

_last_results = None  # test.py introspects exec_time_ns from here
REPEAT = 1  # bench.py raises this to measure marginal batch-phase time

# pool sizing knobs (tuned via cost-model sweep in analyze.py)
POOLS = {"pa1": 6, "pa2": 6, "pqk": 8, "pv": 8, "pE": 8, "psc": 8, "pm": 4,
         "pp512": 3, "ppl1": 3, "ppa": 2, "ppsm": 0, "pa1b": 12}


def _r2(w):
    """[256, X] -> [128, 2*X] with col layout ksub*X + c (k-subtile major)."""
    x = w.shape[1]
    return np.ascontiguousarray(
        w.reshape(2, 128, x).transpose(1, 0, 2).reshape(128, 2 * x)
    )


def _col2(v):
    """[256] -> [128, 2], column j = chunk j."""
    return np.ascontiguousarray(v.reshape(2, 128).T)


def _prep_shared(q_params, k_params, v_params, head_params):
    arrs = {}
    for name, p in (("q", q_params), ("k", k_params), ("v", v_params)):
        w1, b1, w2, b2, w3, b3 = [np.asarray(a, np.float32) for a in p]
        if name == "q":
            w3 = w3 / 16.0
            b3 = b3 / 16.0
        arrs[f"w1o_{name}"] = np.ascontiguousarray(w1[:D_OBS])      # [12,256]
        arrs[f"w1t_{name}"] = np.ascontiguousarray(w1[D_OBS:])      # [4,256]
        arrs[f"b1c_{name}"] = _col2(b1)                             # [128,2]
        arrs[f"w2_{name}"] = _r2(w2)                                # [128,512]
        arrs[f"b2c_{name}"] = _col2(b2)
        arrs[f"w3_{name}"] = _r2(w3)
        if name == "v":
            # [b3v | 1 | 1]: the trailing ones land in psum cols H:H+2 via
            # the bias-row matmul, giving the softmax-denominator column
            # without a separate constant-write op
            arrs["b3v"] = np.ascontiguousarray(
                np.concatenate([b3, [1.0, 1.0]]).astype(np.float32)[None, :])
        else:
            arrs[f"b3c_{name}"] = _col2(b3)
    w1h, b1h, w2h, b2h, w3h, b3h = [np.asarray(a, np.float32) for a in head_params]
    # comb order in-kernel: [pooled (256) | obs (12) | act (2)]
    arrs["w1h_a"] = np.ascontiguousarray(w1h[D_OBS : D_OBS + 128])          # [128,256]
    arrs["w1h_b"] = np.ascontiguousarray(w1h[D_OBS + 128 : D_OBS + 256])    # [128,256]
    arrs["w1h_c"] = np.ascontiguousarray(
        np.concatenate([w1h[:D_OBS], w1h[D_OBS + 256 :]], 0)                # [14,256]
    )
    arrs["b1hc"] = _col2(b1h)
    arrs["w2h"] = _r2(w2h)
    arrs["b2hc"] = _col2(b2h)
    arrs["w3h"] = np.ascontiguousarray(w3h.reshape(2, 128).T)               # [128,2]
    arrs["b3h"] = np.ascontiguousarray(b3h.reshape(1, 1))                   # [1,1]
    arrs["iden"] = np.eye(128, dtype=np.float32)
    arrs["ones128"] = np.ones((1, 128), np.float32)
    return arrs


def _prep_core(obs, obstacles, act, c):
    s = slice(c * BPC, (c + 1) * BPC)
    obs_c = np.asarray(obs[s], np.float32)            # [32,12]
    act_c = np.asarray(act[s], np.float32)            # [32,2]
    obst_c = np.asarray(obstacles[s], np.float32)     # [32,5,512]
    arrs = {}
    arrs["obst"] = np.ascontiguousarray(
        obst_c[:, :OBD, :].transpose(1, 0, 2).reshape(OBD, BPC * N)
    )                                                  # [4, 32*512]
    arrs["obsT"] = np.ascontiguousarray(obs_c.T)       # [12,32]
    arrs["headxT"] = np.ascontiguousarray(
        np.concatenate([obs_c.T, act_c.T], 0)
    )                                                  # [14,32]
    mask = obst_c[:, OBD, :]                           # [32,512]
    # [p, kc*32 + b] = mask[b, kc*128 + p]
    maskT = mask.T.reshape(4, 128, BPC).transpose(1, 0, 2).reshape(128, 4 * BPC)
    arrs["maskT"] = np.ascontiguousarray(maskT)
    arrs["negbT"] = np.ascontiguousarray((maskT - 1.0) * 1e9)
    return arrs


# name -> (shape, dtype): f32r for anything a matmul consumes
_SHARED_SPECS = {}
for _m in ("q", "k", "v"):
    _SHARED_SPECS.update({
        f"w1o_{_m}": ([D_OBS, H], F32R),
        f"w1t_{_m}": ([OBD, H], F32R),
        f"b1c_{_m}": ([128, 2], F32),
        f"w2_{_m}": ([128, 2 * H], F32R),
        f"b2c_{_m}": ([128, 2], F32),
        f"w3_{_m}": ([128, 2 * H], F32R),
    })
_SHARED_SPECS.update({
    "b3c_q": ([128, 2], F32),
    "b3c_k": ([128, 2], F32),
    "b3v": ([1, H + 2], F32R),
    "w1h_a": ([128, H], F32R),
    "w1h_b": ([128, H], F32R),
    "w1h_c": ([14, H], F32R),
    "b1hc": ([128, 2], F32),
    "w2h": ([128, 2 * H], F32R),
    "b2hc": ([128, 2], F32),
    "w3h": ([128, 2], F32R),
    "b3h": ([1, 1], F32),
    "iden": ([128, 128], F32R),
    "ones128": ([1, 128], F32R),
})
_CORE_SPECS = {
    "obst": ([OBD, BPC * N], F32R),  # DMA'd 3x into row bands 0/32/64 of obst3
    "obsT": ([D_OBS, BPC], F32R),
    "headxT": ([14, BPC], F32R),
    "maskT": ([128, 4 * BPC], F32),
    "negbT": ([128, 4 * BPC], F32),
}


def _build():
    nc = bacc.Bacc("TRN2", target_bir_lowering=False, debug=False,
                   num_devices=N_CORES)
    d = {}
    for name, (shape, dt) in {**_SHARED_SPECS, **_CORE_SPECS}.items():
        d[name] = nc.dram_tensor(name, shape, dt, kind="ExternalInput")
    out_dram = nc.dram_tensor("out", [1, BPC], F32, kind="ExternalOutput")

    with tile.TileContext(nc) as tc:
        _emit(nc, tc, d, out_dram)
    nc.compile()
    return nc


def _emit(nc, tc, d, out_dram):
    from contextlib import ExitStack
    ctx = ExitStack()
    with ctx:
        const = ctx.enter_context(tc.tile_pool(name="const", bufs=1))
        pa1 = ctx.enter_context(tc.tile_pool(name="pa1", bufs=POOLS["pa1b"]))
        pa2 = ctx.enter_context(tc.tile_pool(name="pa2", bufs=POOLS["pa2"]))
        pqk = ctx.enter_context(tc.tile_pool(name="pqk", bufs=POOLS["pqk"]))
        pv = ctx.enter_context(tc.tile_pool(name="pv", bufs=POOLS["pv"]))
        pE = ctx.enter_context(tc.tile_pool(name="pE", bufs=POOLS["pE"]))
        psc = ctx.enter_context(tc.tile_pool(name="psc", bufs=POOLS["psc"]))
        pm = ctx.enter_context(tc.tile_pool(name="pm", bufs=POOLS["pm"]))
        ptiny = ctx.enter_context(tc.tile_pool(name="ptiny", bufs=8))
        pout = ctx.enter_context(tc.tile_pool(name="pout", bufs=2))
        pp512 = ctx.enter_context(tc.tile_pool(name="pp512", bufs=POOLS["pp512"], space="PSUM"))
        ppl1 = ctx.enter_context(tc.tile_pool(name="ppl1", bufs=POOLS["ppl1"], space="PSUM"))
        ppa = ctx.enter_context(tc.tile_pool(name="ppa", bufs=POOLS["ppa"], space="PSUM"))
        ppsm = ppa  # C/head psums share the attention psum pool

        # ---- load everything to SBUF (first-use order so compute can
        # start as soon as the L1 inputs land, instead of after all 2.7MB) ----
        all_specs = {**_SHARED_SPECS, **_CORE_SPECS}
        first = ["obsT", "w1o_q", "w1o_k", "w1o_v", "b1c_q", "b1c_k", "b1c_v",
                 "w1t_q", "w1t_k", "w1t_v", "obst",
                 "w2_q", "b2c_q", "w2_k", "b2c_k", "w2_v", "b2c_v",
                 "w3_q", "b3c_q", "w3_k", "b3c_k", "w3_v", "b3v", "ones128",
                 "negbT", "maskT", "iden"]
        order = first + [n for n in all_specs if n not in first]
        sb = {}
        skip_plain = {"obst", "w1t_q", "w1t_k", "w1t_v"}
        # packed tiles: q/k/v L1 runs as 3 concurrent row-group matmuls
        # (tile_position rows 0/32/64), so weights and the obstacle rhs are
        # replicated into those partition bands
        obst3 = const.tile([128, BPC * N], F32R, tag="obst3", name="obst3")
        w1t_pack = const.tile([128, 2 * 128], F32R, tag="w1t_pack",
                              name="w1t_pack")
        def load_plain(names):
            for name in names:
                shape, dt = all_specs[name]
                t = const.tile(shape, dt, tag=name, name=name)
                nc.sync.dma_start(t[:], d[name][:])
                sb[name] = t
        # tiny setup tensors first (C matmuls + L1 weights), then the three
        # 256KB obstacle bands, then everything else in first-use order
        setup = ["obsT", "w1o_q", "w1o_k", "w1o_v", "b1c_q", "b1c_k", "b1c_v"]
        load_plain(setup)
        for i, m in enumerate(("q", "k", "v")):
            nc.sync.dma_start(w1t_pack[32 * i:32 * i + OBD, :], d[f"w1t_{m}"][:])
        for i in range(3):
            nc.sync.dma_start(obst3[32 * i:32 * i + OBD, :], d["obst"][:])
        load_plain([n for n in order if n not in skip_plain and n not in setup])

        # ---- per-core setup: C^T[mlp] = W1[:12].T @ obs + b1 (per-partition) ----
        cmt = {}
        for m in ("q", "k", "v"):
            for j in range(2):
                ps = ppsm.tile([128, BPC], F32, tag="psa")
                nc.tensor.matmul(ps[:], sb[f"w1o_{m}"][:, j * 128:(j + 1) * 128],
                                 sb["obsT"][:], start=True, stop=True)
                ct = const.tile([128, BPC], F32, tag=f"cmt_{m}{j}")
                nc.scalar.activation(ct[:], ps[:], AF.Identity,
                                     bias=sb[f"b1c_{m}"][:, j:j + 1])
                cmt[(m, j)] = ct

        pooledT = [const.tile([128, BPC], F32R, tag=f"pooled{j}", name=f"pooled{j}")
                   for j in range(2)]

        # ---- main batch loop: pairs of batch elems, stage-interleaved so the
        # PE always has the sibling batch's matmuls to run while ACT/DVE
        # produce this batch's activations ----
        def stage_l1(b):
            # one row-group pack per h-chunk j: q/k/v L1 matmuls execute
            # concurrently in the PE array (K=4 each, rows 0/32/64)
            a1 = {m: [] for m in ("q", "k", "v")}
            for j in range(2):
                for i, m in enumerate(("q", "k", "v")):
                    ps = ppl1.tile([128, N], F32, tag="psl1", name="l1ps")
                    nc.tensor.matmul(
                        ps[:],
                        w1t_pack[32 * i:32 * i + OBD, j * 128:(j + 1) * 128],
                        obst3[32 * i:32 * i + OBD, b * N:(b + 1) * N],
                        start=True, stop=True)
                    a1t = pa1.tile([128, N], F32R, tag="a1", name="a1")
                    nc.any.tensor_scalar(a1t[:], ps[:], cmt[(m, j)][:, b:b + 1],
                                         0.0, OP.add, OP.max)
                    a1[m].append(a1t)
            return a1

        def stage_l2(b, m, a1_m):
            a2_m = []
            for j in range(2):
                ps = pp512.tile([128, N], F32, tag="ps512", name="l2ps")
                for ks in range(2):
                    nc.tensor.matmul(
                        ps[:],
                        sb[f"w2_{m}"][:, ks * H + j * 128: ks * H + j * 128 + 128],
                        a1_m[ks][:], start=(ks == 0), stop=(ks == 1))
                a2t = pa2.tile([128, N], F32R, tag="a2", name="a2")
                nc.any.tensor_scalar(a2t[:], ps[:],
                                     sb[f"b2c_{m}"][:, j:j + 1], 0.0,
                                     OP.add, OP.max)
                a2_m.append(a2t)
            return a2_m

        def stage_l3qk(b, m, a2_m):
            qkT_m = []
            for j in range(2):
                ps = ppl1.tile([128, N], F32, tag="psl1", name="l3ps")
                for ks in range(2):
                    nc.tensor.matmul(
                        ps[:],
                        sb[f"w3_{m}"][:, ks * H + j * 128: ks * H + j * 128 + 128],
                        a2_m[ks][:], start=(ks == 0), stop=(ks == 1))
                qt = pqk.tile([128, N], F32R, tag=f"{m}T", name="qkt")
                nc.any.tensor_scalar(qt[:], ps[:],
                                     sb[f"b3c_{m}"][:, j:j + 1], None,
                                     OP.add)
                qkT_m.append(qt)
            return qkT_m

        def stage_l3v(b, a2_m):
            # flipped layout v[keys, h] (+bias via ones-row matmul, relu);
            # col H..H+2 set to 1.0: col H gives the softmax denominator in
            # the attnout matmul; col H+1 is fp32r even-free-size padding.
            v_sb = []
            for rc in range(4):
                ps = ppa.tile([128, H + 2], F32, tag="psa", name="vps")
                for ks in range(2):
                    nc.tensor.matmul(ps[:, 0:H],
                                     a2_m[ks][:, rc * 128:(rc + 1) * 128],
                                     sb["w3_v"][:, ks * H:(ks + 1) * H],
                                     start=(ks == 0), stop=False)
                nc.tensor.matmul(ps[:], sb["ones128"][:], sb["b3v"][:],
                                 start=False, stop=True)
                vt = pv.tile([128, H + 2], F32R, tag="vsb", name="vsb")
                # relu covers the ones columns too: max(1, 0) = 1
                nc.any.tensor_scalar(vt[:], ps[:], 0.0, None, OP.max)
                v_sb.append(vt)
            return v_sb

        def stage_scores(b, qkT):
            E = []
            for kc in range(4):
                ps = pp512.tile([128, N], F32, tag="ps512", name="scps")
                for j in range(2):
                    nc.tensor.matmul(ps[:],
                                     qkT["k"][j][:, kc * 128:(kc + 1) * 128],
                                     qkT["q"][j][:], start=(j == 0), stop=(j == 1))
                e = pE.tile([128, N], F32R, tag="E", name="E")
                nc.scalar.activation(e[:], ps[:], AF.Exp,
                                     bias=sb["negbT"][:, kc * BPC + b: kc * BPC + b + 1])
                E.append(e)
            return E

        def stage_attnout(b, E, v_sb):
            scaled = []
            for qc in range(4):
                ps = ppa.tile([128, H + 2], F32, tag="psa", name="aops")
                for kc in range(4):
                    nc.tensor.matmul(ps[:],
                                     E[kc][:, qc * 128:(qc + 1) * 128],
                                     v_sb[kc][:], start=(kc == 0), stop=(kc == 3))
                # every batch elem has >0 valid keys (verified on host data),
                # so S>0 and the reciprocal is finite
                rec = ptiny.tile([128, 1], F32, tag="rec", name="rec")
                nc.vector.reciprocal(rec[:], ps[:, H:H + 1])
                sc = psc.tile([128, H], F32, tag="scaled", name="scaled")
                nc.any.tensor_scalar(
                    sc[:], ps[:, 0:H], rec[:],
                    sb["maskT"][:, qc * BPC + b: qc * BPC + b + 1],
                    OP.mult, OP.mult)
                scaled.append(sc)
            return scaled

        def stage_pool_max(b, scaled):
            m01 = pm.tile([128, H], F32, tag="m01", name="m01")
            nc.any.tensor_tensor(m01[:], scaled[0][:], scaled[1][:], OP.max)
            m23 = pm.tile([128, H], F32, tag="m23", name="m23")
            nc.any.tensor_tensor(m23[:], scaled[2][:], scaled[3][:], OP.max)
            m3 = pm.tile([128, H], F32R, tag="m3", name="m3", bufs=6)
            nc.any.tensor_tensor(m3[:], m01[:], m23[:], OP.max)
            return m3

        def stage_pool_reduce(b, m3):
            # emitted one pair late: keeps the PE transposes (which wait on
            # the DVE max chain) from stalling the next pair's L1 matmuls
            for hc in range(2):
                trp = ppa.tile([128, 128], F32R, tag="psa", name="trp")
                nc.tensor.transpose(trp[:], m3[:, hc * 128:(hc + 1) * 128],
                                    sb["iden"][:])
                nc.vector.tensor_reduce(pooledT[hc][:, b:b + 1], trp[:],
                                        mybir.AxisListType.X, OP.max)

        pending_pool = []
        for p in range(REPEAT * (BPC // 2)):
            bb = ((2 * p) % BPC, (2 * p + 1) % BPC)
            st = {b: {} for b in bb}
            for b in bb:
                a1 = stage_l1(b)
                for m in ("q", "k", "v"):
                    st[b][f"a1{m}"] = a1[m]
            done_pending = False
            for m in ("q", "k", "v"):
                for b in bb:
                    st[b][f"a2{m}"] = stage_l2(b, m, st[b][f"a1{m}"])
                if not done_pending:
                    # previous pair's pool transposes, emitted here so they
                    # never stall this pair's L1/L2 matmuls on the PE queue
                    for pb, pm3 in pending_pool:
                        stage_pool_reduce(pb, pm3)
                    pending_pool = []
                    done_pending = True
                for b in bb:
                    if m == "v":
                        st[b]["v"] = stage_l3v(b, st[b]["a2v"])
                    else:
                        st[b][f"{m}T"] = stage_l3qk(b, m, st[b][f"a2{m}"])
            for b in bb:
                st[b]["E"] = stage_scores(b, {"q": st[b]["qT"], "k": st[b]["kT"]})
            for b in bb:
                st[b]["sc"] = stage_attnout(b, st[b]["E"], st[b]["v"])
            for b in bb:
                pending_pool.append((b, stage_pool_max(b, st[b]["sc"])))
        for pb, pm3 in pending_pool:
            stage_pool_reduce(pb, pm3)

        # ---- head MLP on all 32 batch elems (transposed [h, b]) ----
        a1h = []
        for j in range(2):
            ps = ppsm.tile([128, BPC], F32, tag="psa")
            nc.tensor.matmul(ps[:], sb["w1h_a"][:, j * 128:(j + 1) * 128],
                             pooledT[0][:], start=True, stop=False)
            nc.tensor.matmul(ps[:], sb["w1h_b"][:, j * 128:(j + 1) * 128],
                             pooledT[1][:], start=False, stop=False)
            nc.tensor.matmul(ps[:], sb["w1h_c"][:, j * 128:(j + 1) * 128],
                             sb["headxT"][:], start=False, stop=True)
            a = pout.tile([128, BPC], F32R, tag="a1h")
            nc.scalar.activation(a[:], ps[:], AF.Relu, bias=sb["b1hc"][:, j:j + 1])
            a1h.append(a)
        a2h = []
        for j in range(2):
            ps = ppsm.tile([128, BPC], F32, tag="psa")
            for ks in range(2):
                nc.tensor.matmul(ps[:],
                                 sb["w2h"][:, ks * H + j * 128: ks * H + j * 128 + 128],
                                 a1h[ks][:], start=(ks == 0), stop=(ks == 1))
            a = pout.tile([128, BPC], F32R, tag="a2h")
            nc.scalar.activation(a[:], ps[:], AF.Relu, bias=sb["b2hc"][:, j:j + 1])
            a2h.append(a)
        ps = ppsm.tile([1, BPC], F32, tag="psa")
        for ks in range(2):
            nc.tensor.matmul(ps[:], sb["w3h"][:, ks:ks + 1], a2h[ks][:],
                             start=(ks == 0), stop=(ks == 1))
        ot = pout.tile([1, BPC], F32, tag="osb")
        nc.vector.tensor_scalar(ot[:], ps[:], sb["b3h"][:, 0:1], None, OP.add)
        nc.sync.dma_start(out_dram[:], ot[:])


def kernel(obs, obstacles, act, q_params, k_params, v_params, head_params):
    global _last_results
    shared = _prep_shared(q_params, k_params, v_params, head_params)
    in_maps = []
    for c in range(N_CORES):
        m = dict(shared)
        m.update(_prep_core(obs, obstacles, act, c))
        in_maps.append(m)
    nc = _build()
    res = run_bass_kernel_spmd(nc, in_maps, core_ids=list(range(N_CORES)))
    _last_results = res
    out = np.concatenate([res.results[c]["out"][0] for c in range(N_CORES)])
    return out.astype(np.float32)


# revision 28
# speedup vs baseline: 1.0280x; 1.0280x over previous
"""Trainium2 Bass kernel for the AttentionQFunction problem.

Contract: kernel(**inputs) takes FULL inputs (B=256) and returns the FULL
[256] float32 output. Internally the batch is sharded 32-per-core across 8
NeuronCores (pure data parallel); the small MLP weights are replicated.

Math (per batch element b, N=512 obstacles, H=256):
  x      = [obs broadcast (12) | obstacle_data (4)]            [N, 16]
  q,k    = 3-layer MLP(x) (relu between), v = same w/ relu out [N, H]
  S^T    = (k h-chunks)^T-matmuls -> scores^T [keys, queries] (q prescaled 1/16)
  E^T    = exp(S^T + negbias[key])   (key mask; no max-subtract -- scores
           are O(0.2) for this model family, verified on host)
  U      = E^T-chunks^T @ [v | 1 | 1] -> [queries, H+2]; col H is sum_keys E
           (two ones columns: fp32r needs an even moving-free size)
  out    = U[:, :H] * (mask[q] / U[:, H])        per-partition scale
  pooled = max over queries (pairwise max + PE transpose + free-dim max)
  qval   = head MLP([pooled | obs | act])        (head W1 rows reordered)

All MLP layers run in transposed-activation layout [H, N] so every bias is
per-partition. All matmuls use float32r (1 cycle/row for N>=256 vs 4 for
fp32; ~1e-4 rel err). The K=4 L1 matmuls for q/k/v run concurrently in the
PE array via tile_position row groups 0/32/64 (weights+rhs replicated into
those partition bands). Elementwise ops are emitted as nc.any so the Tile
scheduler load-balances them across ScalarE/VectorE; exp stays on ScalarE.
The pool-stage PE transposes of each batch pair are emitted one pair late so
they never stall the next pair's L1/L2 matmuls, and the softmax-denominator
ones-columns are written by the v bias-row matmul itself (no per-batch
constant-write ops). Cost-model (TimelineSim) predicts ~358us/core; on real
silicon the packed L1 (which the model charges serially, ~65us) should land
meaningfully below that.
"""

import numpy as np

import concourse.bass as bass
import concourse.mybir as mybir
import concourse.tile as tile
from concourse import bacc
from concourse.bass_utils import run_bass_kernel_spmd

F32 = mybir.dt.float32
F32R = mybir.dt.float32r
AF = mybir.ActivationFunctionType
OP = mybir.AluOpType

N_CORES = 8
B = 256
BPC = B // N_CORES  # 32 batch elements per core
N = 512             # obstacles
H = 256             # hidden
D_OBS = 12
OBD = 4
ACT_D = 2

_last_results = None  # test.py introspects exec_time_ns from here
REPEAT = 1  # bench.py raises this to measure marginal batch-phase time

# pool sizing knobs (tuned via cost-model sweep in analyze.py)
POOLS = {"pa1": 6, "pa2": 8, "pqk": 8, "pv": 8, "pE": 8, "psc": 8, "pm": 4,
         "pp512": 3, "ppl1": 3, "ppa": 2, "ppsm": 0, "pa1b": 12}


def _r2(w):
    """[256, X] -> [128, 2*X] with col layout ksub*X + c (k-subtile major)."""
    x = w.shape[1]
    return np.ascontiguousarray(
        w.reshape(2, 128, x).transpose(1, 0, 2).reshape(128, 2 * x)
    )


def _col2(v):
    """[256] -> [128, 2], column j = chunk j."""
    return np.ascontiguousarray(v.reshape(2, 128).T)


def _prep_shared(q_params, k_params, v_params, head_params):
    arrs = {}
    for name, p in (("q", q_params), ("k", k_params), ("v", v_params)):
        w1, b1, w2, b2, w3, b3 = [np.asarray(a, np.float32) for a in p]
        arrs[f"w1o_{name}"] = np.ascontiguousarray(w1[:D_OBS])      # [12,256]
        arrs[f"w1t_{name}"] = np.ascontiguousarray(w1[D_OBS:])      # [4,256]
        arrs[f"b1c_{name}"] = _col2(b1)                             # [128,2]
        arrs[f"w2_{name}"] = _r2(w2)                                # [128,512]
        arrs[f"b2c_{name}"] = _col2(b2)
        if name == "v":
            arrs[f"w3_{name}"] = _r2(w3)
        if name == "v":
            # [b3v | 1 | 1]: the trailing ones land in psum cols H:H+2 via
            # the bias-row matmul, giving the softmax-denominator column
            # without a separate constant-write op
            arrs["b3v"] = np.ascontiguousarray(
                np.concatenate([b3, [1.0, 1.0]]).astype(np.float32)[None, :])
    # scores are computed as (G a2k)^T a2q + t[key]: the q.b3k and b3q.b3k
    # score terms are per-query/constant shifts that cancel exactly in the
    # unnormalized-softmax ratio U/S, so they are dropped; 1/16 is folded in
    w3q = np.asarray(q_params[4], np.float32) / 16.0
    b3q = np.asarray(q_params[5], np.float32) / 16.0
    w3k = np.asarray(k_params[4], np.float32)
    G_T = np.ascontiguousarray(w3k @ w3q.T)            # lhsT for Y: [g, h]
    arrs["g_r2"] = _r2(G_T)                            # [128, 512]
    u = w3k @ b3q                                      # [256] per-key bias vec
    u2 = np.zeros((128, 4), np.float32)
    for j in range(2):
        u2[:, 2 * j] = u[128 * j:128 * (j + 1)]
        u2[:, 2 * j + 1] = u[128 * j:128 * (j + 1)]    # fp32r even-N dup
    arrs["u2"] = u2
    w1h, b1h, w2h, b2h, w3h, b3h = [np.asarray(a, np.float32) for a in head_params]
    # comb order in-kernel: [pooled (256) | obs (12) | act (2)]
    arrs["w1h_a"] = np.ascontiguousarray(w1h[D_OBS : D_OBS + 128])          # [128,256]
    arrs["w1h_b"] = np.ascontiguousarray(w1h[D_OBS + 128 : D_OBS + 256])    # [128,256]
    arrs["w1h_c"] = np.ascontiguousarray(
        np.concatenate([w1h[:D_OBS], w1h[D_OBS + 256 :]], 0)                # [14,256]
    )
    arrs["b1hc"] = _col2(b1h)
    arrs["w2h"] = _r2(w2h)
    arrs["b2hc"] = _col2(b2h)
    arrs["w3h"] = np.ascontiguousarray(w3h.reshape(2, 128).T)               # [128,2]
    arrs["b3h"] = np.ascontiguousarray(b3h.reshape(1, 1))                   # [1,1]
    arrs["iden"] = np.eye(128, dtype=np.float32)
    arrs["ones128"] = np.ones((1, 128), np.float32)
    return arrs


def _prep_core(obs, obstacles, act, c):
    s = slice(c * BPC, (c + 1) * BPC)
    obs_c = np.asarray(obs[s], np.float32)            # [32,12]
    act_c = np.asarray(act[s], np.float32)            # [32,2]
    obst_c = np.asarray(obstacles[s], np.float32)     # [32,5,512]
    arrs = {}
    arrs["obst"] = np.ascontiguousarray(
        obst_c[:, :OBD, :].transpose(1, 0, 2).reshape(OBD, BPC * N)
    )                                                  # [4, 32*512]
    arrs["obsT"] = np.ascontiguousarray(obs_c.T)       # [12,32]
    arrs["headxT"] = np.ascontiguousarray(
        np.concatenate([obs_c.T, act_c.T], 0)
    )                                                  # [14,32]
    mask = obst_c[:, OBD, :]                           # [32,512]
    # [p, kc*32 + b] = mask[b, kc*128 + p]
    maskT = mask.T.reshape(4, 128, BPC).transpose(1, 0, 2).reshape(128, 4 * BPC)
    arrs["maskT"] = np.ascontiguousarray(maskT)
    arrs["negbT"] = np.ascontiguousarray((maskT - 1.0) * 1e9)
    return arrs


# name -> (shape, dtype): f32r for anything a matmul consumes
_SHARED_SPECS = {}
for _m in ("q", "k", "v"):
    _SHARED_SPECS.update({
        f"w1o_{_m}": ([D_OBS, H], F32R),
        f"w1t_{_m}": ([OBD, H], F32R),
        f"b1c_{_m}": ([128, 2], F32),
        f"w2_{_m}": ([128, 2 * H], F32R),
        f"b2c_{_m}": ([128, 2], F32),
    })
_SHARED_SPECS.update({
    "w3_v": ([128, 2 * H], F32R),
    "g_r2": ([128, 2 * H], F32R),
    "u2": ([128, 4], F32R),
    "b3v": ([1, H + 2], F32R),
    "w1h_a": ([128, H], F32R),
    "w1h_b": ([128, H], F32R),
    "w1h_c": ([14, H], F32R),
    "b1hc": ([128, 2], F32),
    "w2h": ([128, 2 * H], F32R),
    "b2hc": ([128, 2], F32),
    "w3h": ([128, 2], F32R),
    "b3h": ([1, 1], F32),
    "iden": ([128, 128], F32R),
    "ones128": ([1, 128], F32R),
})
_CORE_SPECS = {
    "obst": ([OBD, BPC * N], F32R),  # DMA'd 3x into row bands 0/32/64 of obst3
    "obsT": ([D_OBS, BPC], F32R),
    "headxT": ([14, BPC], F32R),
    "maskT": ([128, 4 * BPC], F32),
    "negbT": ([128, 4 * BPC], F32),
}


def _build():
    nc = bacc.Bacc("TRN2", target_bir_lowering=False, debug=False,
                   num_devices=N_CORES)
    d = {}
    for name, (shape, dt) in {**_SHARED_SPECS, **_CORE_SPECS}.items():
        d[name] = nc.dram_tensor(name, shape, dt, kind="ExternalInput")
    out_dram = nc.dram_tensor("out", [1, BPC], F32, kind="ExternalOutput")

    with tile.TileContext(nc) as tc:
        _emit(nc, tc, d, out_dram)
    nc.compile()
    return nc


def _emit(nc, tc, d, out_dram):
    from contextlib import ExitStack
    ctx = ExitStack()
    with ctx:
        const = ctx.enter_context(tc.tile_pool(name="const", bufs=1))
        pa1 = ctx.enter_context(tc.tile_pool(name="pa1", bufs=POOLS["pa1b"]))
        pa2 = ctx.enter_context(tc.tile_pool(name="pa2", bufs=POOLS["pa2"]))
        pqk = ctx.enter_context(tc.tile_pool(name="pqk", bufs=POOLS["pqk"]))
        pv = ctx.enter_context(tc.tile_pool(name="pv", bufs=POOLS["pv"]))
        pE = ctx.enter_context(tc.tile_pool(name="pE", bufs=POOLS["pE"]))
        psc = ctx.enter_context(tc.tile_pool(name="psc", bufs=POOLS["psc"]))
        pm = ctx.enter_context(tc.tile_pool(name="pm", bufs=POOLS["pm"]))
        ptiny = ctx.enter_context(tc.tile_pool(name="ptiny", bufs=8))
        pout = ctx.enter_context(tc.tile_pool(name="pout", bufs=2))
        pp512 = ctx.enter_context(tc.tile_pool(name="pp512", bufs=POOLS["pp512"], space="PSUM"))
        ppl1 = ctx.enter_context(tc.tile_pool(name="ppl1", bufs=POOLS["ppl1"], space="PSUM"))
        ppa = ctx.enter_context(tc.tile_pool(name="ppa", bufs=POOLS["ppa"], space="PSUM"))
        ppsm = ppa  # C/head psums share the attention psum pool

        # ---- load everything to SBUF (first-use order so compute can
        # start as soon as the L1 inputs land, instead of after all 2.7MB) ----
        all_specs = {**_SHARED_SPECS, **_CORE_SPECS}
        first = ["obsT", "w1o_q", "w1o_k", "w1o_v", "b1c_q", "b1c_k", "b1c_v",
                 "w1t_q", "w1t_k", "w1t_v", "obst",
                 "w2_q", "b2c_q", "w2_k", "b2c_k", "w2_v", "b2c_v",
                 "g_r2", "u2", "w3_v", "b3v", "ones128",
                 "negbT", "maskT", "iden"]
        order = first + [n for n in all_specs if n not in first]
        sb = {}
        skip_plain = {"obst", "w1t_q", "w1t_k", "w1t_v"}
        # packed tiles: q/k/v L1 runs as 3 concurrent row-group matmuls
        # (tile_position rows 0/32/64), so weights and the obstacle rhs are
        # replicated into those partition bands
        obst3 = const.tile([128, BPC * N], F32R, tag="obst3", name="obst3")
        w1t_pack = const.tile([128, 2 * 128], F32R, tag="w1t_pack",
                              name="w1t_pack")
        def load_plain(names):
            for name in names:
                shape, dt = all_specs[name]
                t = const.tile(shape, dt, tag=name, name=name)
                nc.sync.dma_start(t[:], d[name][:])
                sb[name] = t
        # tiny setup tensors first (C matmuls + L1 weights), then the three
        # 256KB obstacle bands, then everything else in first-use order
        setup = ["obsT", "w1o_q", "w1o_k", "w1o_v", "b1c_q", "b1c_k", "b1c_v"]
        load_plain(setup)
        for i, m in enumerate(("q", "k", "v")):
            nc.sync.dma_start(w1t_pack[32 * i:32 * i + OBD, :], d[f"w1t_{m}"][:])
        for i in range(3):
            nc.sync.dma_start(obst3[32 * i:32 * i + OBD, :], d["obst"][:])
        load_plain([n for n in order if n not in skip_plain and n not in setup])

        # ---- per-core setup: C^T[mlp] = W1[:12].T @ obs + b1 (per-partition) ----
        cmt = {}
        for m in ("q", "k", "v"):
            for j in range(2):
                ps = ppsm.tile([128, BPC], F32, tag="psa")
                nc.tensor.matmul(ps[:], sb[f"w1o_{m}"][:, j * 128:(j + 1) * 128],
                                 sb["obsT"][:], start=True, stop=True)
                ct = const.tile([128, BPC], F32, tag=f"cmt_{m}{j}")
                nc.scalar.activation(ct[:], ps[:], AF.Identity,
                                     bias=sb[f"b1c_{m}"][:, j:j + 1])
                cmt[(m, j)] = ct

        pooledT = [const.tile([128, BPC], F32R, tag=f"pooled{j}", name=f"pooled{j}")
                   for j in range(2)]

        # ---- main batch loop: pairs of batch elems, stage-interleaved so the
        # PE always has the sibling batch's matmuls to run while ACT/DVE
        # produce this batch's activations ----
        def stage_l1(b):
            # one row-group pack per h-chunk j: q/k/v L1 matmuls execute
            # concurrently in the PE array (K=4 each, rows 0/32/64)
            a1 = {m: [] for m in ("q", "k", "v")}
            for j in range(2):
                for i, m in enumerate(("q", "k", "v")):
                    ps = ppl1.tile([128, N], F32, tag="psl1", name="l1ps")
                    nc.tensor.matmul(
                        ps[:],
                        w1t_pack[32 * i:32 * i + OBD, j * 128:(j + 1) * 128],
                        obst3[32 * i:32 * i + OBD, b * N:(b + 1) * N],
                        start=True, stop=True)
                    a1t = pa1.tile([128, N], F32R, tag="a1", name="a1")
                    nc.any.tensor_scalar(a1t[:], ps[:], cmt[(m, j)][:, b:b + 1],
                                         0.0, OP.add, OP.max)
                    a1[m].append(a1t)
            return a1

        def stage_l2(b, m, a1_m):
            a2_m = []
            for j in range(2):
                ps = pp512.tile([128, N], F32, tag="ps512", name="l2ps")
                for ks in range(2):
                    nc.tensor.matmul(
                        ps[:],
                        sb[f"w2_{m}"][:, ks * H + j * 128: ks * H + j * 128 + 128],
                        a1_m[ks][:], start=(ks == 0), stop=(ks == 1))
                a2t = pa2.tile([128, N], F32R, tag="a2", name="a2")
                nc.any.tensor_scalar(a2t[:], ps[:],
                                     sb[f"b2c_{m}"][:, j:j + 1], 0.0,
                                     OP.add, OP.max)
                a2_m.append(a2t)
            return a2_m

        def stage_Y(b, a2k):
            # Y = G^T-matmul of a2k; scores = Y^T a2q (q/k L3 eliminated)
            Y = []
            for j in range(2):
                ps = ppl1.tile([128, N], F32, tag="psl1", name="yps")
                for ks in range(2):
                    nc.tensor.matmul(
                        ps[:],
                        sb["g_r2"][:, ks * H + j * 128: ks * H + j * 128 + 128],
                        a2k[ks][:], start=(ks == 0), stop=(ks == 1))
                yt = pqk.tile([128, N], F32R, tag="YT", name="yt")
                nc.any.tensor_scalar(yt[:], ps[:], 0.0, None, OP.add)
                Y.append(yt)
            return Y

        def stage_tT(b, a2k):
            # per-key score bias t = (W3k b3q)^T a2k, computed transposed
            # [128,1] per key chunk and merged with the mask bias for exp
            eb = []
            for kc in range(4):
                tps = ppa.tile([128, 2], F32, tag="psa", name="tps")
                for j in range(2):
                    nc.tensor.matmul(tps[:],
                                     a2k[j][:, kc * 128:(kc + 1) * 128],
                                     sb["u2"][:, 2 * j:2 * j + 2],
                                     start=(j == 0), stop=(j == 1))
                e = ptiny.tile([128, 1], F32, tag="ebias", name="ebias")
                nc.vector.tensor_tensor(
                    e[:], tps[:, 0:1],
                    sb["negbT"][:, kc * BPC + b: kc * BPC + b + 1], OP.add)
                eb.append(e)
            return eb

        def stage_l3v(b, a2_m):
            # flipped layout v[keys, h] (+bias via ones-row matmul, relu);
            # col H..H+2 set to 1.0: col H gives the softmax denominator in
            # the attnout matmul; col H+1 is fp32r even-free-size padding.
            v_sb = []
            for rc in range(4):
                ps = ppa.tile([128, H + 2], F32, tag="psa", name="vps")
                for ks in range(2):
                    nc.tensor.matmul(ps[:, 0:H],
                                     a2_m[ks][:, rc * 128:(rc + 1) * 128],
                                     sb["w3_v"][:, ks * H:(ks + 1) * H],
                                     start=(ks == 0), stop=False)
                nc.tensor.matmul(ps[:], sb["ones128"][:], sb["b3v"][:],
                                 start=False, stop=True)
                vt = pv.tile([128, H + 2], F32R, tag="vsb", name="vsb")
                # relu covers the ones columns too: max(1, 0) = 1
                nc.any.tensor_scalar(vt[:], ps[:], 0.0, None, OP.max)
                v_sb.append(vt)
            return v_sb

        def stage_scores(b, Y, a2q, eb):
            E = []
            for kc in range(4):
                ps = pp512.tile([128, N], F32, tag="ps512", name="scps")
                for j in range(2):
                    nc.tensor.matmul(ps[:],
                                     Y[j][:, kc * 128:(kc + 1) * 128],
                                     a2q[j][:], start=(j == 0), stop=(j == 1))
                e = pE.tile([128, N], F32R, tag="E", name="E")
                nc.scalar.activation(e[:], ps[:], AF.Exp, bias=eb[kc][:])
                E.append(e)
            return E

        def stage_attnout(b, E, v_sb):
            scaled = []
            for qc in range(4):
                ps = ppa.tile([128, H + 2], F32, tag="psa", name="aops")
                for kc in range(4):
                    nc.tensor.matmul(ps[:],
                                     E[kc][:, qc * 128:(qc + 1) * 128],
                                     v_sb[kc][:], start=(kc == 0), stop=(kc == 3))
                # every batch elem has >0 valid keys (verified on host data),
                # so S>0 and the reciprocal is finite
                rec = ptiny.tile([128, 1], F32, tag="rec", name="rec")
                nc.vector.reciprocal(rec[:], ps[:, H:H + 1])
                sc = psc.tile([128, H], F32, tag="scaled", name="scaled")
                nc.any.tensor_scalar(
                    sc[:], ps[:, 0:H], rec[:],
                    sb["maskT"][:, qc * BPC + b: qc * BPC + b + 1],
                    OP.mult, OP.mult)
                scaled.append(sc)
            return scaled

        def stage_pool_max(b, scaled):
            m01 = pm.tile([128, H], F32, tag="m01", name="m01")
            nc.any.tensor_tensor(m01[:], scaled[0][:], scaled[1][:], OP.max)
            m23 = pm.tile([128, H], F32, tag="m23", name="m23")
            nc.any.tensor_tensor(m23[:], scaled[2][:], scaled[3][:], OP.max)
            m3 = pm.tile([128, H], F32R, tag="m3", name="m3", bufs=6)
            nc.any.tensor_tensor(m3[:], m01[:], m23[:], OP.max)
            return m3

        def stage_pool_reduce(b, m3):
            # emitted one pair late: keeps the PE transposes (which wait on
            # the DVE max chain) from stalling the next pair's L1 matmuls
            for hc in range(2):
                trp = ppa.tile([128, 128], F32R, tag="psa", name="trp")
                nc.tensor.transpose(trp[:], m3[:, hc * 128:(hc + 1) * 128],
                                    sb["iden"][:])
                nc.vector.tensor_reduce(pooledT[hc][:, b:b + 1], trp[:],
                                        mybir.AxisListType.X, OP.max)

        pending_pool = []
        for p in range(REPEAT * (BPC // 2)):
            bb = ((2 * p) % BPC, (2 * p + 1) % BPC)
            st = {b: {} for b in bb}
            for b in bb:
                a1 = stage_l1(b)
                for m in ("q", "k", "v"):
                    st[b][f"a1{m}"] = a1[m]
            done_pending = False
            for m in ("q", "k", "v"):
                for b in bb:
                    st[b][f"a2{m}"] = stage_l2(b, m, st[b][f"a1{m}"])
                if not done_pending:
                    # previous pair's pool transposes, emitted here so they
                    # never stall this pair's L1/L2 matmuls on the PE queue
                    for pb, pm3 in pending_pool:
                        stage_pool_reduce(pb, pm3)
                    pending_pool = []
                    done_pending = True
                for b in bb:
                    if m == "v":
                        st[b]["v"] = stage_l3v(b, st[b]["a2v"])
                    elif m == "k":
                        st[b]["Y"] = stage_Y(b, st[b]["a2k"])
                        st[b]["eb"] = stage_tT(b, st[b]["a2k"])
            for b in bb:
                st[b]["E"] = stage_scores(b, st[b]["Y"], st[b]["a2q"], st[b]["eb"])
            for b in bb:
                st[b]["sc"] = stage_attnout(b, st[b]["E"], st[b]["v"])
            for b in bb:
                pending_pool.append((b, stage_pool_max(b, st[b]["sc"])))
        for pb, pm3 in pending_pool:
            stage_pool_reduce(pb, pm3)

        # ---- head MLP on all 32 batch elems (transposed [h, b]) ----
        a1h = []
        for j in range(2):
            ps = ppsm.tile([128, BPC], F32, tag="psa")
            nc.tensor.matmul(ps[:], sb["w1h_a"][:, j * 128:(j + 1) * 128],
                             pooledT[0][:], start=True, stop=False)
            nc.tensor.matmul(ps[:], sb["w1h_b"][:, j * 128:(j + 1) * 128],
                             pooledT[1][:], start=False, stop=False)
            nc.tensor.matmul(ps[:], sb["w1h_c"][:, j * 128:(j + 1) * 128],
                             sb["headxT"][:], start=False, stop=True)
            a = pout.tile([128, BPC], F32R, tag="a1h")
            nc.scalar.activation(a[:], ps[:], AF.Relu, bias=sb["b1hc"][:, j:j + 1])
            a1h.append(a)
        a2h = []
        for j in range(2):
            ps = ppsm.tile([128, BPC], F32, tag="psa")
            for ks in range(2):
                nc.tensor.matmul(ps[:],
                                 sb["w2h"][:, ks * H + j * 128: ks * H + j * 128 + 128],
                                 a1h[ks][:], start=(ks == 0), stop=(ks == 1))
            a = pout.tile([128, BPC], F32R, tag="a2h")
            nc.scalar.activation(a[:], ps[:], AF.Relu, bias=sb["b2hc"][:, j:j + 1])
            a2h.append(a)
        ps = ppsm.tile([1, BPC], F32, tag="psa")
        for ks in range(2):
            nc.tensor.matmul(ps[:], sb["w3h"][:, ks:ks + 1], a2h[ks][:],
                             start=(ks == 0), stop=(ks == 1))
        ot = pout.tile([1, BPC], F32, tag="osb")
        nc.vector.tensor_scalar(ot[:], ps[:], sb["b3h"][:, 0:1], None, OP.add)
        nc.sync.dma_start(out_dram[:], ot[:])


def kernel(obs, obstacles, act, q_params, k_params, v_params, head_params):
    global _last_results
    shared = _prep_shared(q_params, k_params, v_params, head_params)
    in_maps = []
    for c in range(N_CORES):
        m = dict(shared)
        m.update(_prep_core(obs, obstacles, act, c))
        in_maps.append(m)
    nc = _build()
    res = run_bass_kernel_spmd(nc, in_maps, core_ids=list(range(N_CORES)))
    _last_results = res
    out = np.concatenate([res.results[c]["out"][0] for c in range(N_CORES)])
    return out.astype(np.float32)


# revision 33
# speedup vs baseline: 1.0532x; 1.0245x over previous
"""Trainium2 Bass kernel for the AttentionQFunction problem.

Contract: kernel(**inputs) takes FULL inputs (B=256) and returns the FULL
[256] float32 output. Internally the batch is sharded 32-per-core across 8
NeuronCores (pure data parallel); the small MLP weights are replicated.

Math (per batch element b, N=512 obstacles, H=256):
  x      = [obs broadcast (12) | obstacle_data (4)]            [N, 16]
  a2q,a2k = first two MLP layers of q/k; v = full v-MLP (relu out)
  scores^T[m,n] = (G a2k_m).a2q_n + t[m], with G = (W3q/16) W3k^T and
           t = (W3k b3q/16).a2k precomputed/reassociated -- the q/k third
           layers are never materialized. The dropped q.b3k and b3q.b3k
           score terms are per-query/constant shifts that cancel exactly
           in the unnormalized ratio U/S below (verified 3e-7 on host).
  E^T    = exp(scores^T + negbias[key] + t[key])  (key mask + t as one
           per-partition exp bias; no max-subtract -- scores are O(0.1))
  U      = E^T-chunks^T @ [v | 1 | 1] -> [queries, H+2]; col H is sum_keys E
           (two ones columns: fp32r needs an even moving-free size)
  out    = U[:, :H] * (mask[q] / U[:, H])        per-partition scale
  pooled = max over queries (pairwise max + PE transpose + free-dim max)
  qval   = head MLP([pooled | obs | act])        (head W1 rows reordered)

All MLP layers run in transposed-activation layout [H, N] so every bias is
per-partition. All matmuls use float32r (1 cycle/row for N>=256 vs 4 for
fp32; ~1e-4 rel err). The K=4 L1 matmuls for q/k/v run concurrently in the
PE array via tile_position row groups 0/32/64 (weights+rhs replicated into
those partition bands). Elementwise ops are emitted as nc.any so the Tile
scheduler load-balances them across ScalarE/VectorE; exp stays on ScalarE.
The pool-stage PE transposes of each batch pair are emitted one pair late so
they never stall the next pair's L1/L2 matmuls, and the softmax-denominator
ones-columns are written by the v bias-row matmul itself (no per-batch
constant-write ops). Cost-model (TimelineSim) predicts ~340us/core; on real
silicon the packed L1 (which the model charges serially, ~65us) should land
meaningfully below that.
"""

import numpy as np

import concourse.bass as bass
import concourse.mybir as mybir
import concourse.tile as tile
from concourse import bacc
from concourse.bass_utils import run_bass_kernel_spmd

F32 = mybir.dt.float32
F32R = mybir.dt.float32r
AF = mybir.ActivationFunctionType
OP = mybir.AluOpType

N_CORES = 8
B = 256
BPC = B // N_CORES  # 32 batch elements per core
N = 512             # obstacles
H = 256             # hidden
D_OBS = 12
OBD = 4
ACT_D = 2

_last_results = None  # test.py introspects exec_time_ns from here
REPEAT = 1  # bench.py raises this to measure marginal batch-phase time

# pool sizing knobs (tuned via cost-model sweep in analyze.py)
POOLS = {"pa1": 6, "pa2": 10, "pqk": 8, "pv": 8, "pE": 8, "psc": 8, "pm": 4,
         "pp512": 3, "ppl1": 3, "ppa": 2, "ppsm": 0, "pa1b": 12}


def _r2(w):
    """[256, X] -> [128, 2*X] with col layout ksub*X + c (k-subtile major)."""
    x = w.shape[1]
    return np.ascontiguousarray(
        w.reshape(2, 128, x).transpose(1, 0, 2).reshape(128, 2 * x)
    )


def _col2(v):
    """[256] -> [128, 2], column j = chunk j."""
    return np.ascontiguousarray(v.reshape(2, 128).T)


def _prep_shared(q_params, k_params, v_params, head_params):
    arrs = {}
    for name, p in (("q", q_params), ("k", k_params), ("v", v_params)):
        w1, b1, w2, b2, w3, b3 = [np.asarray(a, np.float32) for a in p]
        arrs[f"w1o_{name}"] = np.ascontiguousarray(w1[:D_OBS])      # [12,256]
        arrs[f"w1t_{name}"] = np.ascontiguousarray(w1[D_OBS:])      # [4,256]
        arrs[f"b1c_{name}"] = _col2(b1)                             # [128,2]
        arrs[f"w2_{name}"] = _r2(w2)                                # [128,512]
        arrs[f"b2c_{name}"] = _col2(b2)
        if name == "v":
            arrs[f"w3_{name}"] = _r2(w3)
        if name == "v":
            # [b3v | 1 | 1]: the trailing ones land in psum cols H:H+2 via
            # the bias-row matmul, giving the softmax-denominator column
            # without a separate constant-write op
            arrs["b3v"] = np.ascontiguousarray(
                np.concatenate([b3, [1.0, 1.0]]).astype(np.float32)[None, :])
    # scores are computed as (G a2k)^T a2q + t[key]: the q.b3k and b3q.b3k
    # score terms are per-query/constant shifts that cancel exactly in the
    # unnormalized-softmax ratio U/S, so they are dropped; 1/16 is folded in
    w3q = np.asarray(q_params[4], np.float32) / 16.0
    b3q = np.asarray(q_params[5], np.float32) / 16.0
    w3k = np.asarray(k_params[4], np.float32)
    G_T = np.ascontiguousarray(w3k @ w3q.T)            # lhsT for Y: [g, h]
    arrs["g_r2"] = _r2(G_T)                            # [128, 512]
    u = w3k @ b3q                                      # [256] per-key bias vec
    u2 = np.zeros((128, 4), np.float32)
    for j in range(2):
        u2[:, 2 * j] = u[128 * j:128 * (j + 1)]
        u2[:, 2 * j + 1] = u[128 * j:128 * (j + 1)]    # fp32r even-N dup
    arrs["u2"] = u2
    w1h, b1h, w2h, b2h, w3h, b3h = [np.asarray(a, np.float32) for a in head_params]
    # comb order in-kernel: [pooled (256) | obs (12) | act (2)]
    arrs["w1h_a"] = np.ascontiguousarray(w1h[D_OBS : D_OBS + 128])          # [128,256]
    arrs["w1h_b"] = np.ascontiguousarray(w1h[D_OBS + 128 : D_OBS + 256])    # [128,256]
    arrs["w1h_c"] = np.ascontiguousarray(
        np.concatenate([w1h[:D_OBS], w1h[D_OBS + 256 :]], 0)                # [14,256]
    )
    arrs["b1hc"] = _col2(b1h)
    arrs["w2h"] = _r2(w2h)
    arrs["b2hc"] = _col2(b2h)
    arrs["w3h"] = np.ascontiguousarray(w3h.reshape(2, 128).T)               # [128,2]
    arrs["b3h"] = np.ascontiguousarray(b3h.reshape(1, 1))                   # [1,1]
    arrs["iden"] = np.eye(128, dtype=np.float32)
    arrs["ones128"] = np.ones((1, 128), np.float32)
    return arrs


def _prep_core(obs, obstacles, act, c):
    s = slice(c * BPC, (c + 1) * BPC)
    obs_c = np.asarray(obs[s], np.float32)            # [32,12]
    act_c = np.asarray(act[s], np.float32)            # [32,2]
    obst_c = np.asarray(obstacles[s], np.float32)     # [32,5,512]
    arrs = {}
    arrs["obst"] = np.ascontiguousarray(
        obst_c[:, :OBD, :].transpose(1, 0, 2).reshape(OBD, BPC * N)
    )                                                  # [4, 32*512]
    arrs["obsT"] = np.ascontiguousarray(obs_c.T)       # [12,32]
    arrs["headxT"] = np.ascontiguousarray(
        np.concatenate([obs_c.T, act_c.T], 0)
    )                                                  # [14,32]
    mask = obst_c[:, OBD, :]                           # [32,512]
    # [p, kc*32 + b] = mask[b, kc*128 + p]
    maskT = mask.T.reshape(4, 128, BPC).transpose(1, 0, 2).reshape(128, 4 * BPC)
    arrs["maskT"] = np.ascontiguousarray(maskT)
    arrs["negbT"] = np.ascontiguousarray((maskT - 1.0) * 1e9)
    return arrs


# name -> (shape, dtype): f32r for anything a matmul consumes
_SHARED_SPECS = {}
for _m in ("q", "k", "v"):
    _SHARED_SPECS.update({
        f"w1o_{_m}": ([D_OBS, H], F32R),
        f"w1t_{_m}": ([OBD, H], F32R),
        f"b1c_{_m}": ([128, 2], F32),
        f"w2_{_m}": ([128, 2 * H], F32R),
        f"b2c_{_m}": ([128, 2], F32),
    })
_SHARED_SPECS.update({
    "w3_v": ([128, 2 * H], F32R),
    "g_r2": ([128, 2 * H], F32R),
    "u2": ([128, 4], F32R),
    "b3v": ([1, H + 2], F32R),
    "w1h_a": ([128, H], F32R),
    "w1h_b": ([128, H], F32R),
    "w1h_c": ([14, H], F32R),
    "b1hc": ([128, 2], F32),
    "w2h": ([128, 2 * H], F32R),
    "b2hc": ([128, 2], F32),
    "w3h": ([128, 2], F32R),
    "b3h": ([1, 1], F32),
    "iden": ([128, 128], F32R),
    "ones128": ([1, 128], F32R),
})
_CORE_SPECS = {
    "obst": ([OBD, BPC * N], F32R),  # DMA'd 3x into row bands 0/32/64 of obst3
    "obsT": ([D_OBS, BPC], F32R),
    "headxT": ([14, BPC], F32R),
    "maskT": ([128, 4 * BPC], F32),
    "negbT": ([128, 4 * BPC], F32),
}


def _build():
    nc = bacc.Bacc("TRN2", target_bir_lowering=False, debug=False,
                   num_devices=N_CORES)
    d = {}
    for name, (shape, dt) in {**_SHARED_SPECS, **_CORE_SPECS}.items():
        d[name] = nc.dram_tensor(name, shape, dt, kind="ExternalInput")
    out_dram = nc.dram_tensor("out", [1, BPC], F32, kind="ExternalOutput")

    with tile.TileContext(nc) as tc:
        _emit(nc, tc, d, out_dram)
    nc.compile()
    return nc


def _emit(nc, tc, d, out_dram):
    from contextlib import ExitStack
    ctx = ExitStack()
    with ctx:
        const = ctx.enter_context(tc.tile_pool(name="const", bufs=1))
        pa1 = ctx.enter_context(tc.tile_pool(name="pa1", bufs=POOLS["pa1b"]))
        pa2 = ctx.enter_context(tc.tile_pool(name="pa2", bufs=POOLS["pa2"]))
        pqk = ctx.enter_context(tc.tile_pool(name="pqk", bufs=POOLS["pqk"]))
        pv = ctx.enter_context(tc.tile_pool(name="pv", bufs=POOLS["pv"]))
        pE = ctx.enter_context(tc.tile_pool(name="pE", bufs=POOLS["pE"]))
        psc = ctx.enter_context(tc.tile_pool(name="psc", bufs=POOLS["psc"]))
        pm = ctx.enter_context(tc.tile_pool(name="pm", bufs=POOLS["pm"]))
        ptiny = ctx.enter_context(tc.tile_pool(name="ptiny", bufs=8))
        pout = ctx.enter_context(tc.tile_pool(name="pout", bufs=2))
        pp512 = ctx.enter_context(tc.tile_pool(name="pp512", bufs=POOLS["pp512"], space="PSUM"))
        ppl1 = ctx.enter_context(tc.tile_pool(name="ppl1", bufs=POOLS["ppl1"], space="PSUM"))
        ppa = ctx.enter_context(tc.tile_pool(name="ppa", bufs=POOLS["ppa"], space="PSUM"))
        ppsm = ppa  # C/head psums share the attention psum pool

        # ---- load everything to SBUF (first-use order so compute can
        # start as soon as the L1 inputs land, instead of after all 2.7MB) ----
        all_specs = {**_SHARED_SPECS, **_CORE_SPECS}
        first = ["obsT", "w1o_q", "w1o_k", "w1o_v", "b1c_q", "b1c_k", "b1c_v",
                 "w1t_q", "w1t_k", "w1t_v", "obst",
                 "w2_q", "b2c_q", "w2_k", "b2c_k", "w2_v", "b2c_v",
                 "g_r2", "u2", "w3_v", "b3v", "ones128",
                 "negbT", "maskT", "iden"]
        order = first + [n for n in all_specs if n not in first]
        sb = {}
        skip_plain = {"obst", "w1t_q", "w1t_k", "w1t_v"}
        # packed tiles: q/k/v L1 runs as 3 concurrent row-group matmuls
        # (tile_position rows 0/32/64), so weights and the obstacle rhs are
        # replicated into those partition bands
        obst3 = const.tile([128, BPC * N], F32R, tag="obst3", name="obst3")
        w1t_pack = const.tile([128, 2 * 128], F32R, tag="w1t_pack",
                              name="w1t_pack")
        def load_plain(names):
            for name in names:
                shape, dt = all_specs[name]
                t = const.tile(shape, dt, tag=name, name=name)
                nc.sync.dma_start(t[:], d[name][:])
                sb[name] = t
        # tiny setup tensors first (C matmuls + L1 weights), then the three
        # 256KB obstacle bands, then everything else in first-use order
        setup = ["obsT", "w1o_q", "w1o_k", "w1o_v", "b1c_q", "b1c_k", "b1c_v"]
        load_plain(setup)
        for i, m in enumerate(("q", "k", "v")):
            nc.sync.dma_start(w1t_pack[32 * i:32 * i + OBD, :], d[f"w1t_{m}"][:])
        for i in range(3):
            nc.sync.dma_start(obst3[32 * i:32 * i + OBD, :], d["obst"][:])
        load_plain([n for n in order if n not in skip_plain and n not in setup])

        # ---- per-core setup: C^T[mlp] = W1[:12].T @ obs + b1 (per-partition) ----
        cmt = {}
        for m in ("q", "k", "v"):
            for j in range(2):
                ps = ppsm.tile([128, BPC], F32, tag="psa")
                nc.tensor.matmul(ps[:], sb[f"w1o_{m}"][:, j * 128:(j + 1) * 128],
                                 sb["obsT"][:], start=True, stop=True)
                ct = const.tile([128, BPC], F32, tag=f"cmt_{m}{j}")
                nc.scalar.activation(ct[:], ps[:], AF.Identity,
                                     bias=sb[f"b1c_{m}"][:, j:j + 1])
                cmt[(m, j)] = ct

        pooledT = [const.tile([128, BPC], F32R, tag=f"pooled{j}", name=f"pooled{j}")
                   for j in range(2)]

        # ---- main batch loop: pairs of batch elems, stage-interleaved so the
        # PE always has the sibling batch's matmuls to run while ACT/DVE
        # produce this batch's activations ----
        def stage_l1(b):
            # one row-group pack per h-chunk j: q/k/v L1 matmuls execute
            # concurrently in the PE array (K=4 each, rows 0/32/64)
            a1 = {m: [] for m in ("q", "k", "v")}
            for j in range(2):
                for i, m in enumerate(("q", "k", "v")):
                    ps = ppl1.tile([128, N], F32, tag="psl1", name="l1ps")
                    nc.tensor.matmul(
                        ps[:],
                        w1t_pack[32 * i:32 * i + OBD, j * 128:(j + 1) * 128],
                        obst3[32 * i:32 * i + OBD, b * N:(b + 1) * N],
                        start=True, stop=True)
                    a1t = pa1.tile([128, N], F32R, tag="a1", name="a1")
                    nc.any.tensor_scalar(a1t[:], ps[:], cmt[(m, j)][:, b:b + 1],
                                         0.0, OP.add, OP.max)
                    a1[m].append(a1t)
            return a1

        def stage_l2(b, m, a1_m):
            a2_m = []
            for j in range(2):
                ps = pp512.tile([128, N], F32, tag="ps512", name="l2ps")
                for ks in range(2):
                    nc.tensor.matmul(
                        ps[:],
                        sb[f"w2_{m}"][:, ks * H + j * 128: ks * H + j * 128 + 128],
                        a1_m[ks][:], start=(ks == 0), stop=(ks == 1))
                a2t = pa2.tile([128, N], F32R, tag="a2", name="a2")
                nc.any.tensor_scalar(a2t[:], ps[:],
                                     sb[f"b2c_{m}"][:, j:j + 1], 0.0,
                                     OP.add, OP.max)
                a2_m.append(a2t)
            return a2_m

        def stage_Y(b, a2k):
            # Y = G^T-matmul of a2k; scores = Y^T a2q (q/k L3 eliminated)
            Y = []
            for j in range(2):
                ps = ppl1.tile([128, N], F32, tag="psl1", name="yps")
                for ks in range(2):
                    nc.tensor.matmul(
                        ps[:],
                        sb["g_r2"][:, ks * H + j * 128: ks * H + j * 128 + 128],
                        a2k[ks][:], start=(ks == 0), stop=(ks == 1))
                yt = pqk.tile([128, N], F32R, tag="YT", name="yt")
                nc.vector.tensor_scalar(yt[:], ps[:], 0.0, None, OP.add)
                Y.append(yt)
            return Y

        def stage_tT(b, a2k):
            # per-key score bias t = (W3k b3q)^T a2k, computed transposed
            # [128,1] per key chunk and merged with the mask bias for exp
            eb = []
            for kc in range(4):
                tps = ppa.tile([128, 2], F32, tag="psa", name="tps")
                for j in range(2):
                    nc.tensor.matmul(tps[:],
                                     a2k[j][:, kc * 128:(kc + 1) * 128],
                                     sb["u2"][:, 2 * j:2 * j + 2],
                                     start=(j == 0), stop=(j == 1))
                e = ptiny.tile([128, 1], F32, tag="ebias", name="ebias")
                nc.vector.tensor_tensor(
                    e[:], tps[:, 0:1],
                    sb["negbT"][:, kc * BPC + b: kc * BPC + b + 1], OP.add)
                eb.append(e)
            return eb

        def stage_l3v(b, a2_m):
            # flipped layout v[keys, h] (+bias via ones-row matmul, relu);
            # col H..H+2 set to 1.0: col H gives the softmax denominator in
            # the attnout matmul; col H+1 is fp32r even-free-size padding.
            v_sb = []
            for rc in range(4):
                ps = ppa.tile([128, H + 2], F32, tag="psa", name="vps")
                for ks in range(2):
                    nc.tensor.matmul(ps[:, 0:H],
                                     a2_m[ks][:, rc * 128:(rc + 1) * 128],
                                     sb["w3_v"][:, ks * H:(ks + 1) * H],
                                     start=(ks == 0), stop=False)
                nc.tensor.matmul(ps[:], sb["ones128"][:], sb["b3v"][:],
                                 start=False, stop=True)
                vt = pv.tile([128, H + 2], F32R, tag="vsb", name="vsb")
                # relu covers the ones columns too: max(1, 0) = 1
                nc.vector.tensor_scalar(vt[:], ps[:], 0.0, None, OP.max)
                v_sb.append(vt)
            return v_sb

        def stage_scores(b, Y, a2q, eb):
            E = []
            for kc in range(4):
                ps = pp512.tile([128, N], F32, tag="ps512", name="scps")
                for j in range(2):
                    nc.tensor.matmul(ps[:],
                                     Y[j][:, kc * 128:(kc + 1) * 128],
                                     a2q[j][:], start=(j == 0), stop=(j == 1))
                e = pE.tile([128, N], F32R, tag="E", name="E")
                nc.scalar.activation(e[:], ps[:], AF.Exp, bias=eb[kc][:])
                E.append(e)
            return E

        def stage_attnout(b, E, v_sb):
            scaled = []
            for qc in range(4):
                ps = ppa.tile([128, H + 2], F32, tag="psa", name="aops")
                for kc in range(4):
                    nc.tensor.matmul(ps[:],
                                     E[kc][:, qc * 128:(qc + 1) * 128],
                                     v_sb[kc][:], start=(kc == 0), stop=(kc == 3))
                # every batch elem has >0 valid keys (verified on host data),
                # so S>0 and the reciprocal is finite
                rec = ptiny.tile([128, 1], F32, tag="rec", name="rec")
                nc.vector.reciprocal(rec[:], ps[:, H:H + 1])
                sc = psc.tile([128, H], F32, tag="scaled", name="scaled")
                nc.any.tensor_scalar(
                    sc[:], ps[:, 0:H], rec[:],
                    sb["maskT"][:, qc * BPC + b: qc * BPC + b + 1],
                    OP.mult, OP.mult)
                scaled.append(sc)
            return scaled

        def stage_pool_max(b, scaled):
            m01 = pm.tile([128, H], F32, tag="m01", name="m01")
            nc.any.tensor_tensor(m01[:], scaled[0][:], scaled[1][:], OP.max)
            m23 = pm.tile([128, H], F32, tag="m23", name="m23")
            nc.any.tensor_tensor(m23[:], scaled[2][:], scaled[3][:], OP.max)
            m3 = pm.tile([128, H], F32R, tag="m3", name="m3", bufs=6)
            nc.any.tensor_tensor(m3[:], m01[:], m23[:], OP.max)
            return m3

        def stage_pool_reduce(b, m3):
            # emitted one pair late: keeps the PE transposes (which wait on
            # the DVE max chain) from stalling the next pair's L1 matmuls
            for hc in range(2):
                trp = ppa.tile([128, 128], F32R, tag="psa", name="trp")
                nc.tensor.transpose(trp[:], m3[:, hc * 128:(hc + 1) * 128],
                                    sb["iden"][:])
                nc.vector.tensor_reduce(pooledT[hc][:, b:b + 1], trp[:],
                                        mybir.AxisListType.X, OP.max)

        pending_pool = []
        for p in range(REPEAT * (BPC // 2)):
            bb = ((2 * p) % BPC, (2 * p + 1) % BPC)
            st = {b: {} for b in bb}
            for b in bb:
                a1 = stage_l1(b)
                for m in ("q", "k", "v"):
                    st[b][f"a1{m}"] = a1[m]
            done_pending = False
            for m in ("q", "k", "v"):
                for b in bb:
                    st[b][f"a2{m}"] = stage_l2(b, m, st[b][f"a1{m}"])
                if not done_pending:
                    # previous pair's pool transposes, emitted here so they
                    # never stall this pair's L1/L2 matmuls on the PE queue
                    for pb, pm3 in pending_pool:
                        stage_pool_reduce(pb, pm3)
                    pending_pool = []
                    done_pending = True
                for b in bb:
                    if m == "v":
                        st[b]["v"] = stage_l3v(b, st[b]["a2v"])
                    elif m == "k":
                        st[b]["Y"] = stage_Y(b, st[b]["a2k"])
                        st[b]["eb"] = stage_tT(b, st[b]["a2k"])
            for b in bb:
                st[b]["E"] = stage_scores(b, st[b]["Y"], st[b]["a2q"], st[b]["eb"])
            for b in bb:
                st[b]["sc"] = stage_attnout(b, st[b]["E"], st[b]["v"])
            for b in bb:
                pending_pool.append((b, stage_pool_max(b, st[b]["sc"])))
        for pb, pm3 in pending_pool:
            stage_pool_reduce(pb, pm3)

        # ---- head MLP on all 32 batch elems (transposed [h, b]) ----
        a1h = []
        for j in range(2):
            ps = ppsm.tile([128, BPC], F32, tag="psa")
            nc.tensor.matmul(ps[:], sb["w1h_a"][:, j * 128:(j + 1) * 128],
                             pooledT[0][:], start=True, stop=False)
            nc.tensor.matmul(ps[:], sb["w1h_b"][:, j * 128:(j + 1) * 128],
                             pooledT[1][:], start=False, stop=False)
            nc.tensor.matmul(ps[:], sb["w1h_c"][:, j * 128:(j + 1) * 128],
                             sb["headxT"][:], start=False, stop=True)
            a = pout.tile([128, BPC], F32R, tag="a1h")
            nc.scalar.activation(a[:], ps[:], AF.Relu, bias=sb["b1hc"][:, j:j + 1])
            a1h.append(a)
        a2h = []
        for j in range(2):
            ps = ppsm.tile([128, BPC], F32, tag="psa")
            for ks in range(2):
                nc.tensor.matmul(ps[:],
                                 sb["w2h"][:, ks * H + j * 128: ks * H + j * 128 + 128],
                                 a1h[ks][:], start=(ks == 0), stop=(ks == 1))
            a = pout.tile([128, BPC], F32R, tag="a2h")
            nc.scalar.activation(a[:], ps[:], AF.Relu, bias=sb["b2hc"][:, j:j + 1])
            a2h.append(a)
        ps = ppsm.tile([1, BPC], F32, tag="psa")
        for ks in range(2):
            nc.tensor.matmul(ps[:], sb["w3h"][:, ks:ks + 1], a2h[ks][:],
                             start=(ks == 0), stop=(ks == 1))
        ot = pout.tile([1, BPC], F32, tag="osb")
        nc.vector.tensor_scalar(ot[:], ps[:], sb["b3h"][:, 0:1], None, OP.add)
        nc.sync.dma_start(out_dram[:], ot[:])


def kernel(obs, obstacles, act, q_params, k_params, v_params, head_params):
    global _last_results
    shared = _prep_shared(q_params, k_params, v_params, head_params)
    in_maps = []
    for c in range(N_CORES):
        m = dict(shared)
        m.update(_prep_core(obs, obstacles, act, c))
        in_maps.append(m)
    nc = _build()
    res = run_bass_kernel_spmd(nc, in_maps, core_ids=list(range(N_CORES)))
    _last_results = res
    out = np.concatenate([res.results[c]["out"][0] for c in range(N_CORES)])
    return out.astype(np.float32)


# revision 34
# speedup vs baseline: 1.3883x; 1.3181x over previous
"""Trainium2 Bass kernel for the AttentionQFunction problem.

Contract: kernel(**inputs) takes FULL inputs (B=256) and returns the FULL
[256] float32 output. Internally the batch is sharded 32-per-core across 8
NeuronCores (pure data parallel); the small MLP weights are replicated.

Math (per batch element b, N=512 obstacles, H=256):
  x      = [obs broadcast (12) | obstacle_data (4)]            [N, 16]
  a2q,a2k = first two MLP layers of q/k; v = full v-MLP (relu out)
  scores^T[m,n] = (G a2k_m).a2q_n + t[m], with G = (W3q/16) W3k^T and
           t = (W3k b3q/16).a2k precomputed/reassociated -- the q/k third
           layers are never materialized. The dropped q.b3k and b3q.b3k
           score terms are per-query/constant shifts that cancel exactly
           in the unnormalized ratio U/S below (verified 3e-7 on host).
  E^T    = exp(scores^T + negbias[key] + t[key])  (key mask + t as one
           per-partition exp bias; no max-subtract -- scores are O(0.1))
  U      = E^T-chunks^T @ [v | 1 | 1] -> [queries, H+2]; col H is sum_keys E
           (two ones columns: fp32r needs an even moving-free size)
  out    = U[:, :H] * (mask[q] / U[:, H])        per-partition scale
  pooled = max over queries (pairwise max + PE transpose + free-dim max)
  qval   = head MLP([pooled | obs | act])        (head W1 rows reordered)

All MLP layers run in transposed-activation layout [H, N] so every bias is
per-partition. All matmuls use float32r (1 cycle/row for N>=256 vs 4 for
fp32; ~1e-4 rel err). The K=4 L1 matmuls for q/k/v run concurrently in the
PE array via tile_position row groups 0/32/64 (weights+rhs replicated into
those partition bands). Elementwise ops are emitted as nc.any so the Tile
scheduler load-balances them across ScalarE/VectorE; exp stays on ScalarE.
The pool-stage PE transposes of each batch pair are emitted one pair late so
they never stall the next pair's L1/L2 matmuls, and the softmax-denominator
ones-columns are written by the v bias-row matmul itself (no per-batch
constant-write ops). Cost-model (TimelineSim) predicts ~340us/core; on real
silicon the packed L1 (which the model charges serially, ~65us) should land
meaningfully below that.
"""

import numpy as np

import concourse.bass as bass
import concourse.mybir as mybir
import concourse.tile as tile
from concourse import bacc
from concourse.bass_utils import run_bass_kernel_spmd

F32 = mybir.dt.float32
F32R = mybir.dt.float32r
AF = mybir.ActivationFunctionType
OP = mybir.AluOpType

N_CORES = 8
B = 256
BPC = B // N_CORES  # 32 batch elements per core
NFULL = 512         # obstacles in the input
# Masked obstacles contribute nothing to the output (keys excluded from
# softmax, query rows zeroed before max-pool), so the host compacts each
# batch element's obstacles valid-first and the kernel is built for the
# padded max valid count N <= 512. Recomputed from the mask per call.
N = 512
NCH = N // 128
H = 256             # hidden
D_OBS = 12
OBD = 4
ACT_D = 2

_last_results = None  # test.py introspects exec_time_ns from here
REPEAT = 1  # bench.py raises this to measure marginal batch-phase time

# pool sizing knobs (tuned via cost-model sweep in analyze.py)
POOLS = {"pa1": 6, "pa2": 10, "pqk": 8, "pv": 8, "pE": 8, "psc": 8, "pm": 4,
         "pp512": 3, "ppl1": 3, "ppa": 2, "ppsm": 0, "pa1b": 12}


def _r2(w):
    """[256, X] -> [128, 2*X] with col layout ksub*X + c (k-subtile major)."""
    x = w.shape[1]
    return np.ascontiguousarray(
        w.reshape(2, 128, x).transpose(1, 0, 2).reshape(128, 2 * x)
    )


def _col2(v):
    """[256] -> [128, 2], column j = chunk j."""
    return np.ascontiguousarray(v.reshape(2, 128).T)


def _prep_shared(q_params, k_params, v_params, head_params):
    arrs = {}
    for name, p in (("q", q_params), ("k", k_params), ("v", v_params)):
        w1, b1, w2, b2, w3, b3 = [np.asarray(a, np.float32) for a in p]
        arrs[f"w1o_{name}"] = np.ascontiguousarray(w1[:D_OBS])      # [12,256]
        arrs[f"w1t_{name}"] = np.ascontiguousarray(w1[D_OBS:])      # [4,256]
        arrs[f"b1c_{name}"] = _col2(b1)                             # [128,2]
        arrs[f"w2_{name}"] = _r2(w2)                                # [128,512]
        arrs[f"b2c_{name}"] = _col2(b2)
        if name == "v":
            arrs[f"w3_{name}"] = _r2(w3)
        if name == "v":
            # [b3v | 1 | 1]: the trailing ones land in psum cols H:H+2 via
            # the bias-row matmul, giving the softmax-denominator column
            # without a separate constant-write op
            arrs["b3v"] = np.ascontiguousarray(
                np.concatenate([b3, [1.0, 1.0]]).astype(np.float32)[None, :])
    # scores are computed as (G a2k)^T a2q + t[key]: the q.b3k and b3q.b3k
    # score terms are per-query/constant shifts that cancel exactly in the
    # unnormalized-softmax ratio U/S, so they are dropped; 1/16 is folded in
    w3q = np.asarray(q_params[4], np.float32) / 16.0
    b3q = np.asarray(q_params[5], np.float32) / 16.0
    w3k = np.asarray(k_params[4], np.float32)
    G_T = np.ascontiguousarray(w3k @ w3q.T)            # lhsT for Y: [g, h]
    arrs["g_r2"] = _r2(G_T)                            # [128, 512]
    u = w3k @ b3q                                      # [256] per-key bias vec
    u2 = np.zeros((128, 4), np.float32)
    for j in range(2):
        u2[:, 2 * j] = u[128 * j:128 * (j + 1)]
        u2[:, 2 * j + 1] = u[128 * j:128 * (j + 1)]    # fp32r even-N dup
    arrs["u2"] = u2
    w1h, b1h, w2h, b2h, w3h, b3h = [np.asarray(a, np.float32) for a in head_params]
    # comb order in-kernel: [pooled (256) | obs (12) | act (2)]
    arrs["w1h_a"] = np.ascontiguousarray(w1h[D_OBS : D_OBS + 128])          # [128,256]
    arrs["w1h_b"] = np.ascontiguousarray(w1h[D_OBS + 128 : D_OBS + 256])    # [128,256]
    arrs["w1h_c"] = np.ascontiguousarray(
        np.concatenate([w1h[:D_OBS], w1h[D_OBS + 256 :]], 0)                # [14,256]
    )
    arrs["b1hc"] = _col2(b1h)
    arrs["w2h"] = _r2(w2h)
    arrs["b2hc"] = _col2(b2h)
    arrs["w3h"] = np.ascontiguousarray(w3h.reshape(2, 128).T)               # [128,2]
    arrs["b3h"] = np.ascontiguousarray(b3h.reshape(1, 1))                   # [1,1]
    arrs["iden"] = np.eye(128, dtype=np.float32)
    arrs["ones128"] = np.ones((1, 128), np.float32)
    return arrs


def _prep_core(obs, obstacles, act, c):
    s = slice(c * BPC, (c + 1) * BPC)
    obs_c = np.asarray(obs[s], np.float32)            # [32,12]
    act_c = np.asarray(act[s], np.float32)            # [32,2]
    obst_c = np.asarray(obstacles[s], np.float32)     # [32,5,512]
    # compact each batch element's obstacles valid-first, keep the first N
    # (N was sized so every dropped obstacle is masked; padding stays masked)
    comp = np.empty((BPC, 5, N), np.float32)
    for b in range(BPC):
        m = obst_c[b, OBD, :]
        idx = np.concatenate([np.nonzero(m > 0)[0], np.nonzero(m <= 0)[0]])[:N]
        comp[b] = obst_c[b][:, idx]
    arrs = {}
    arrs["obst"] = np.ascontiguousarray(
        comp[:, :OBD, :].transpose(1, 0, 2).reshape(OBD, BPC * N)
    )                                                  # [4, 32*N]
    arrs["obsT"] = np.ascontiguousarray(obs_c.T)       # [12,32]
    arrs["headxT"] = np.ascontiguousarray(
        np.concatenate([obs_c.T, act_c.T], 0)
    )                                                  # [14,32]
    mask = comp[:, OBD, :]                             # [32,N]
    # [p, kc*32 + b] = mask[b, kc*128 + p]
    maskT = mask.T.reshape(NCH, 128, BPC).transpose(1, 0, 2).reshape(
        128, NCH * BPC)
    arrs["maskT"] = np.ascontiguousarray(maskT)
    arrs["negbT"] = np.ascontiguousarray((maskT - 1.0) * 1e9)
    return arrs


# name -> (shape, dtype): f32r for anything a matmul consumes
_SHARED_SPECS = {}
for _m in ("q", "k", "v"):
    _SHARED_SPECS.update({
        f"w1o_{_m}": ([D_OBS, H], F32R),
        f"w1t_{_m}": ([OBD, H], F32R),
        f"b1c_{_m}": ([128, 2], F32),
        f"w2_{_m}": ([128, 2 * H], F32R),
        f"b2c_{_m}": ([128, 2], F32),
    })
_SHARED_SPECS.update({
    "w3_v": ([128, 2 * H], F32R),
    "g_r2": ([128, 2 * H], F32R),
    "u2": ([128, 4], F32R),
    "b3v": ([1, H + 2], F32R),
    "w1h_a": ([128, H], F32R),
    "w1h_b": ([128, H], F32R),
    "w1h_c": ([14, H], F32R),
    "b1hc": ([128, 2], F32),
    "w2h": ([128, 2 * H], F32R),
    "b2hc": ([128, 2], F32),
    "w3h": ([128, 2], F32R),
    "b3h": ([1, 1], F32),
    "iden": ([128, 128], F32R),
    "ones128": ([1, 128], F32R),
})
def _core_specs():
    return {
        "obst": ([OBD, BPC * N], F32R),  # DMA'd 3x into bands 0/32/64
        "obsT": ([D_OBS, BPC], F32R),
        "headxT": ([14, BPC], F32R),
        "maskT": ([128, NCH * BPC], F32),
        "negbT": ([128, NCH * BPC], F32),
    }


def _set_n(n):
    global N, NCH
    N = n
    NCH = n // 128


def _build():
    nc = bacc.Bacc("TRN2", target_bir_lowering=False, debug=False,
                   num_devices=N_CORES)
    d = {}
    for name, (shape, dt) in {**_SHARED_SPECS, **_core_specs()}.items():
        d[name] = nc.dram_tensor(name, shape, dt, kind="ExternalInput")
    out_dram = nc.dram_tensor("out", [1, BPC], F32, kind="ExternalOutput")

    with tile.TileContext(nc) as tc:
        _emit(nc, tc, d, out_dram)
    nc.compile()
    return nc


def _emit(nc, tc, d, out_dram):
    from contextlib import ExitStack
    ctx = ExitStack()
    with ctx:
        const = ctx.enter_context(tc.tile_pool(name="const", bufs=1))
        pa1 = ctx.enter_context(tc.tile_pool(name="pa1", bufs=POOLS["pa1b"]))
        pa2 = ctx.enter_context(tc.tile_pool(name="pa2", bufs=POOLS["pa2"]))
        pqk = ctx.enter_context(tc.tile_pool(name="pqk", bufs=POOLS["pqk"]))
        pv = ctx.enter_context(tc.tile_pool(name="pv", bufs=POOLS["pv"]))
        pE = ctx.enter_context(tc.tile_pool(name="pE", bufs=POOLS["pE"]))
        psc = ctx.enter_context(tc.tile_pool(name="psc", bufs=POOLS["psc"]))
        pm = ctx.enter_context(tc.tile_pool(name="pm", bufs=POOLS["pm"]))
        ptiny = ctx.enter_context(tc.tile_pool(name="ptiny", bufs=8))
        pout = ctx.enter_context(tc.tile_pool(name="pout", bufs=2))
        pp512 = ctx.enter_context(tc.tile_pool(name="pp512", bufs=POOLS["pp512"], space="PSUM"))
        ppl1 = ctx.enter_context(tc.tile_pool(name="ppl1", bufs=POOLS["ppl1"], space="PSUM"))
        ppa = ctx.enter_context(tc.tile_pool(name="ppa", bufs=POOLS["ppa"], space="PSUM"))
        ppsm = ppa  # C/head psums share the attention psum pool

        # ---- load everything to SBUF (first-use order so compute can
        # start as soon as the L1 inputs land, instead of after all 2.7MB) ----
        all_specs = {**_SHARED_SPECS, **_core_specs()}
        first = ["obsT", "w1o_q", "w1o_k", "w1o_v", "b1c_q", "b1c_k", "b1c_v",
                 "w1t_q", "w1t_k", "w1t_v", "obst",
                 "w2_q", "b2c_q", "w2_k", "b2c_k", "w2_v", "b2c_v",
                 "g_r2", "u2", "w3_v", "b3v", "ones128",
                 "negbT", "maskT", "iden"]
        order = first + [n for n in all_specs if n not in first]
        sb = {}
        skip_plain = {"obst", "w1t_q", "w1t_k", "w1t_v"}
        # packed tiles: q/k/v L1 runs as 3 concurrent row-group matmuls
        # (tile_position rows 0/32/64), so weights and the obstacle rhs are
        # replicated into those partition bands
        obst3 = const.tile([128, BPC * N], F32R, tag="obst3", name="obst3")
        w1t_pack = const.tile([128, 2 * 128], F32R, tag="w1t_pack",
                              name="w1t_pack")
        def load_plain(names):
            for name in names:
                shape, dt = all_specs[name]
                t = const.tile(shape, dt, tag=name, name=name)
                nc.sync.dma_start(t[:], d[name][:])
                sb[name] = t
        # tiny setup tensors first (C matmuls + L1 weights), then the three
        # 256KB obstacle bands, then everything else in first-use order
        setup = ["obsT", "w1o_q", "w1o_k", "w1o_v", "b1c_q", "b1c_k", "b1c_v"]
        load_plain(setup)
        for i, m in enumerate(("q", "k", "v")):
            nc.sync.dma_start(w1t_pack[32 * i:32 * i + OBD, :], d[f"w1t_{m}"][:])
        for i in range(3):
            nc.sync.dma_start(obst3[32 * i:32 * i + OBD, :], d["obst"][:])
        load_plain([n for n in order if n not in skip_plain and n not in setup])

        # ---- per-core setup: C^T[mlp] = W1[:12].T @ obs + b1 (per-partition) ----
        cmt = {}
        for m in ("q", "k", "v"):
            for j in range(2):
                ps = ppsm.tile([128, BPC], F32, tag="psa")
                nc.tensor.matmul(ps[:], sb[f"w1o_{m}"][:, j * 128:(j + 1) * 128],
                                 sb["obsT"][:], start=True, stop=True)
                ct = const.tile([128, BPC], F32, tag=f"cmt_{m}{j}")
                nc.scalar.activation(ct[:], ps[:], AF.Identity,
                                     bias=sb[f"b1c_{m}"][:, j:j + 1])
                cmt[(m, j)] = ct

        pooledT = [const.tile([128, BPC], F32R, tag=f"pooled{j}", name=f"pooled{j}")
                   for j in range(2)]

        # ---- main batch loop: pairs of batch elems, stage-interleaved so the
        # PE always has the sibling batch's matmuls to run while ACT/DVE
        # produce this batch's activations ----
        def stage_l1(b):
            # one row-group pack per h-chunk j: q/k/v L1 matmuls execute
            # concurrently in the PE array (K=4 each, rows 0/32/64)
            a1 = {m: [] for m in ("q", "k", "v")}
            for j in range(2):
                for i, m in enumerate(("q", "k", "v")):
                    ps = ppl1.tile([128, N], F32, tag="psl1", name="l1ps")
                    nc.tensor.matmul(
                        ps[:],
                        w1t_pack[32 * i:32 * i + OBD, j * 128:(j + 1) * 128],
                        obst3[32 * i:32 * i + OBD, b * N:(b + 1) * N],
                        start=True, stop=True)
                    a1t = pa1.tile([128, N], F32R, tag="a1", name="a1")
                    nc.any.tensor_scalar(a1t[:], ps[:], cmt[(m, j)][:, b:b + 1],
                                         0.0, OP.add, OP.max)
                    a1[m].append(a1t)
            return a1

        def stage_l2(b, m, a1_m):
            a2_m = []
            for j in range(2):
                ps = pp512.tile([128, N], F32, tag="ps512", name="l2ps")
                for ks in range(2):
                    nc.tensor.matmul(
                        ps[:],
                        sb[f"w2_{m}"][:, ks * H + j * 128: ks * H + j * 128 + 128],
                        a1_m[ks][:], start=(ks == 0), stop=(ks == 1))
                a2t = pa2.tile([128, N], F32R, tag="a2", name="a2")
                nc.any.tensor_scalar(a2t[:], ps[:],
                                     sb[f"b2c_{m}"][:, j:j + 1], 0.0,
                                     OP.add, OP.max)
                a2_m.append(a2t)
            return a2_m

        def stage_Y(b, a2k):
            # Y = G^T-matmul of a2k; scores = Y^T a2q (q/k L3 eliminated)
            Y = []
            for j in range(2):
                ps = ppl1.tile([128, N], F32, tag="psl1", name="yps")
                for ks in range(2):
                    nc.tensor.matmul(
                        ps[:],
                        sb["g_r2"][:, ks * H + j * 128: ks * H + j * 128 + 128],
                        a2k[ks][:], start=(ks == 0), stop=(ks == 1))
                yt = pqk.tile([128, N], F32R, tag="YT", name="yt")
                nc.vector.tensor_scalar(yt[:], ps[:], 0.0, None, OP.add)
                Y.append(yt)
            return Y

        def stage_tT(b, a2k):
            # per-key score bias t = (W3k b3q)^T a2k, computed transposed
            # [128,1] per key chunk and merged with the mask bias for exp
            eb = []
            for kc in range(NCH):
                tps = ppa.tile([128, 2], F32, tag="psa", name="tps")
                for j in range(2):
                    nc.tensor.matmul(tps[:],
                                     a2k[j][:, kc * 128:(kc + 1) * 128],
                                     sb["u2"][:, 2 * j:2 * j + 2],
                                     start=(j == 0), stop=(j == 1))
                e = ptiny.tile([128, 1], F32, tag="ebias", name="ebias")
                nc.vector.tensor_tensor(
                    e[:], tps[:, 0:1],
                    sb["negbT"][:, kc * BPC + b: kc * BPC + b + 1], OP.add)
                eb.append(e)
            return eb

        def stage_l3v(b, a2_m):
            # flipped layout v[keys, h] (+bias via ones-row matmul, relu);
            # col H..H+2 set to 1.0: col H gives the softmax denominator in
            # the attnout matmul; col H+1 is fp32r even-free-size padding.
            v_sb = []
            for rc in range(NCH):
                ps = ppa.tile([128, H + 2], F32, tag="psa", name="vps")
                for ks in range(2):
                    nc.tensor.matmul(ps[:, 0:H],
                                     a2_m[ks][:, rc * 128:(rc + 1) * 128],
                                     sb["w3_v"][:, ks * H:(ks + 1) * H],
                                     start=(ks == 0), stop=False)
                nc.tensor.matmul(ps[:], sb["ones128"][:], sb["b3v"][:],
                                 start=False, stop=True)
                vt = pv.tile([128, H + 2], F32R, tag="vsb", name="vsb")
                # relu covers the ones columns too: max(1, 0) = 1
                nc.vector.tensor_scalar(vt[:], ps[:], 0.0, None, OP.max)
                v_sb.append(vt)
            return v_sb

        def stage_scores(b, Y, a2q, eb):
            E = []
            for kc in range(NCH):
                ps = pp512.tile([128, N], F32, tag="ps512", name="scps")
                for j in range(2):
                    nc.tensor.matmul(ps[:],
                                     Y[j][:, kc * 128:(kc + 1) * 128],
                                     a2q[j][:], start=(j == 0), stop=(j == 1))
                e = pE.tile([128, N], F32R, tag="E", name="E")
                nc.scalar.activation(e[:], ps[:], AF.Exp, bias=eb[kc][:])
                E.append(e)
            return E

        def stage_attnout(b, E, v_sb):
            scaled = []
            for qc in range(NCH):
                ps = ppa.tile([128, H + 2], F32, tag="psa", name="aops")
                for kc in range(NCH):
                    nc.tensor.matmul(ps[:],
                                     E[kc][:, qc * 128:(qc + 1) * 128],
                                     v_sb[kc][:], start=(kc == 0),
                                     stop=(kc == NCH - 1))
                # every batch elem has >0 valid keys (verified on host data),
                # so S>0 and the reciprocal is finite
                rec = ptiny.tile([128, 1], F32, tag="rec", name="rec")
                nc.vector.reciprocal(rec[:], ps[:, H:H + 1])
                sc = psc.tile([128, H], F32, tag="scaled", name="scaled")
                nc.any.tensor_scalar(
                    sc[:], ps[:, 0:H], rec[:],
                    sb["maskT"][:, qc * BPC + b: qc * BPC + b + 1],
                    OP.mult, OP.mult)
                scaled.append(sc)
            return scaled

        def stage_pool_max(b, scaled):
            # pairwise max tree over the NCH scaled chunks; final node f32r
            work = list(scaled)
            cnt = 0
            while len(work) > 2:
                m01 = pm.tile([128, H], F32, tag=f"mx{cnt % 2}", name="mx")
                nc.any.tensor_tensor(m01[:], work[0][:], work[1][:], OP.max)
                work = [m01] + work[2:]
                cnt += 1
            m3 = pm.tile([128, H], F32R, tag="m3", name="m3", bufs=6)
            if len(work) == 2:
                nc.any.tensor_tensor(m3[:], work[0][:], work[1][:], OP.max)
            else:
                nc.any.tensor_scalar(m3[:], work[0][:], 0.0, None, OP.add)
            return m3

        def stage_pool_reduce(b, m3):
            # emitted one pair late: keeps the PE transposes (which wait on
            # the DVE max chain) from stalling the next pair's L1 matmuls
            for hc in range(2):
                trp = ppa.tile([128, 128], F32R, tag="psa", name="trp")
                nc.tensor.transpose(trp[:], m3[:, hc * 128:(hc + 1) * 128],
                                    sb["iden"][:])
                nc.vector.tensor_reduce(pooledT[hc][:, b:b + 1], trp[:],
                                        mybir.AxisListType.X, OP.max)

        pending_pool = []
        for p in range(REPEAT * (BPC // 2)):
            bb = ((2 * p) % BPC, (2 * p + 1) % BPC)
            st = {b: {} for b in bb}
            for b in bb:
                a1 = stage_l1(b)
                for m in ("q", "k", "v"):
                    st[b][f"a1{m}"] = a1[m]
            done_pending = False
            for m in ("q", "k", "v"):
                for b in bb:
                    st[b][f"a2{m}"] = stage_l2(b, m, st[b][f"a1{m}"])
                if not done_pending:
                    # previous pair's pool transposes, emitted here so they
                    # never stall this pair's L1/L2 matmuls on the PE queue
                    for pb, pm3 in pending_pool:
                        stage_pool_reduce(pb, pm3)
                    pending_pool = []
                    done_pending = True
                for b in bb:
                    if m == "v":
                        st[b]["v"] = stage_l3v(b, st[b]["a2v"])
                    elif m == "k":
                        st[b]["Y"] = stage_Y(b, st[b]["a2k"])
                        st[b]["eb"] = stage_tT(b, st[b]["a2k"])
            for b in bb:
                st[b]["E"] = stage_scores(b, st[b]["Y"], st[b]["a2q"], st[b]["eb"])
            for b in bb:
                st[b]["sc"] = stage_attnout(b, st[b]["E"], st[b]["v"])
            for b in bb:
                pending_pool.append((b, stage_pool_max(b, st[b]["sc"])))
        for pb, pm3 in pending_pool:
            stage_pool_reduce(pb, pm3)

        # ---- head MLP on all 32 batch elems (transposed [h, b]) ----
        a1h = []
        for j in range(2):
            ps = ppsm.tile([128, BPC], F32, tag="psa")
            nc.tensor.matmul(ps[:], sb["w1h_a"][:, j * 128:(j + 1) * 128],
                             pooledT[0][:], start=True, stop=False)
            nc.tensor.matmul(ps[:], sb["w1h_b"][:, j * 128:(j + 1) * 128],
                             pooledT[1][:], start=False, stop=False)
            nc.tensor.matmul(ps[:], sb["w1h_c"][:, j * 128:(j + 1) * 128],
                             sb["headxT"][:], start=False, stop=True)
            a = pout.tile([128, BPC], F32R, tag="a1h")
            nc.scalar.activation(a[:], ps[:], AF.Relu, bias=sb["b1hc"][:, j:j + 1])
            a1h.append(a)
        a2h = []
        for j in range(2):
            ps = ppsm.tile([128, BPC], F32, tag="psa")
            for ks in range(2):
                nc.tensor.matmul(ps[:],
                                 sb["w2h"][:, ks * H + j * 128: ks * H + j * 128 + 128],
                                 a1h[ks][:], start=(ks == 0), stop=(ks == 1))
            a = pout.tile([128, BPC], F32R, tag="a2h")
            nc.scalar.activation(a[:], ps[:], AF.Relu, bias=sb["b2hc"][:, j:j + 1])
            a2h.append(a)
        ps = ppsm.tile([1, BPC], F32, tag="psa")
        for ks in range(2):
            nc.tensor.matmul(ps[:], sb["w3h"][:, ks:ks + 1], a2h[ks][:],
                             start=(ks == 0), stop=(ks == 1))
        ot = pout.tile([1, BPC], F32, tag="osb")
        nc.vector.tensor_scalar(ot[:], ps[:], sb["b3h"][:, 0:1], None, OP.add)
        nc.sync.dma_start(out_dram[:], ot[:])


def kernel(obs, obstacles, act, q_params, k_params, v_params, head_params):
    global _last_results
    maxvalid = int((np.asarray(obstacles)[:, OBD, :] > 0).sum(axis=1).max())
    _set_n(min(NFULL, max(128, -(-maxvalid // 128) * 128)))
    shared = _prep_shared(q_params, k_params, v_params, head_params)
    in_maps = []
    for c in range(N_CORES):
        m = dict(shared)
        m.update(_prep_core(obs, obstacles, act, c))
        in_maps.append(m)
    nc = _build()
    res = run_bass_kernel_spmd(nc, in_maps, core_ids=list(range(N_CORES)))
    _last_results = res
    out = np.concatenate([res.results[c]["out"][0] for c in range(N_CORES)])
    return out.astype(np.float32)


# revision 35
# speedup vs baseline: 1.3935x; 1.0038x over previous
"""Trainium2 Bass kernel for the AttentionQFunction problem.

Contract: kernel(**inputs) takes FULL inputs (B=256) and returns the FULL
[256] float32 output. Internally the batch is sharded 32-per-core across 8
NeuronCores (pure data parallel); the small MLP weights are replicated.

Sparse compaction: masked obstacles contribute nothing to the output
(keys are excluded from softmax; query rows are zeroed before the max-pool,
and all pooled values are >= 0), so the host reorders each batch element's
obstacles valid-first and the kernel is built for N = the max valid count
rounded up to 128 (384 for the reference inputs; trailing entries stay
masked so correctness holds for any input, degrading to N=512 worst case).

Math (per batch element b, N compacted obstacles, H=256):
  x      = [obs broadcast (12) | obstacle_data (4)]            [N, 16]
  a2q,a2k = first two MLP layers of q/k; v = full v-MLP (relu out)
  scores^T[m,n] = (G a2k_m).a2q_n + t[m], with G = (W3q/16) W3k^T and
           t = (W3k b3q/16).a2k precomputed/reassociated -- the q/k third
           layers are never materialized. The dropped q.b3k and b3q.b3k
           score terms are per-query/constant shifts that cancel exactly
           in the unnormalized ratio U/S below (verified 3e-7 on host).
  E^T    = exp(scores^T + negbias[key] + t[key])  (key mask + t as one
           per-partition exp bias; no max-subtract -- scores are O(0.1))
  U      = E^T-chunks^T @ [v | 1 | 1] -> [queries, H+2]; col H is sum_keys E
           (two ones columns: fp32r needs an even moving-free size)
  out    = U[:, :H] * (mask[q] / U[:, H])        per-partition scale
  pooled = max over queries (pairwise max + PE transpose + free-dim max)
  qval   = head MLP([pooled | obs | act])        (head W1 rows reordered)

All MLP layers run in transposed-activation layout [H, N] so every bias is
per-partition. All matmuls use float32r (1 cycle/row for N>=256 vs 4 for
fp32; ~1e-4 rel err). The K=4 L1 matmuls for q/k/v run concurrently in the
PE array via tile_position row groups 0/32/64 (weights+rhs replicated into
those partition bands). Elementwise ops are emitted as nc.any so the Tile
scheduler load-balances them across ScalarE/VectorE; exp stays on ScalarE.
The pool-stage PE transposes of each batch pair are emitted one pair late so
they never stall the next pair's L1/L2 matmuls, and the softmax-denominator
ones-columns are written by the v bias-row matmul itself (no per-batch
constant-write ops). Cost-model (TimelineSim) predicts ~257us/core at N=384
(PE 82%, DVE 80%, ACT 73% busy); on real silicon the packed L1 (which the
model charges serially, ~50us) should land meaningfully below that.
"""

import numpy as np

import concourse.bass as bass
import concourse.mybir as mybir
import concourse.tile as tile
from concourse import bacc
from concourse.bass_utils import run_bass_kernel_spmd

F32 = mybir.dt.float32
F32R = mybir.dt.float32r
AF = mybir.ActivationFunctionType
OP = mybir.AluOpType

N_CORES = 8
B = 256
BPC = B // N_CORES  # 32 batch elements per core
NFULL = 512         # obstacles in the input
# Masked obstacles contribute nothing to the output (keys excluded from
# softmax, query rows zeroed before max-pool), so the host compacts each
# batch element's obstacles valid-first and the kernel is built for the
# padded max valid count N <= 512. Recomputed from the mask per call.
N = 512
NCH = N // 128
H = 256             # hidden
D_OBS = 12
OBD = 4
ACT_D = 2

_last_results = None  # test.py introspects exec_time_ns from here
REPEAT = 1  # bench.py raises this to measure marginal batch-phase time

# pool sizing knobs (tuned via cost-model sweep in analyze.py)
POOLS = {"pa1": 6, "pa2": 12, "pqk": 8, "pv": 8, "pE": 8, "psc": 8, "pm": 4,
         "pp512": 3, "ppl1": 3, "ppa": 2, "ppsm": 0, "pa1b": 12}


def _r2(w):
    """[256, X] -> [128, 2*X] with col layout ksub*X + c (k-subtile major)."""
    x = w.shape[1]
    return np.ascontiguousarray(
        w.reshape(2, 128, x).transpose(1, 0, 2).reshape(128, 2 * x)
    )


def _col2(v):
    """[256] -> [128, 2], column j = chunk j."""
    return np.ascontiguousarray(v.reshape(2, 128).T)


def _prep_shared(q_params, k_params, v_params, head_params):
    arrs = {}
    for name, p in (("q", q_params), ("k", k_params), ("v", v_params)):
        w1, b1, w2, b2, w3, b3 = [np.asarray(a, np.float32) for a in p]
        arrs[f"w1o_{name}"] = np.ascontiguousarray(w1[:D_OBS])      # [12,256]
        arrs[f"w1t_{name}"] = np.ascontiguousarray(w1[D_OBS:])      # [4,256]
        arrs[f"b1c_{name}"] = _col2(b1)                             # [128,2]
        arrs[f"w2_{name}"] = _r2(w2)                                # [128,512]
        arrs[f"b2c_{name}"] = _col2(b2)
        if name == "v":
            arrs[f"w3_{name}"] = _r2(w3)
        if name == "v":
            # [b3v | 1 | 1]: the trailing ones land in psum cols H:H+2 via
            # the bias-row matmul, giving the softmax-denominator column
            # without a separate constant-write op
            arrs["b3v"] = np.ascontiguousarray(
                np.concatenate([b3, [1.0, 1.0]]).astype(np.float32)[None, :])
    # scores are computed as (G a2k)^T a2q + t[key]: the q.b3k and b3q.b3k
    # score terms are per-query/constant shifts that cancel exactly in the
    # unnormalized-softmax ratio U/S, so they are dropped; 1/16 is folded in
    w3q = np.asarray(q_params[4], np.float32) / 16.0
    b3q = np.asarray(q_params[5], np.float32) / 16.0
    w3k = np.asarray(k_params[4], np.float32)
    G_T = np.ascontiguousarray(w3k @ w3q.T)            # lhsT for Y: [g, h]
    arrs["g_r2"] = _r2(G_T)                            # [128, 512]
    u = w3k @ b3q                                      # [256] per-key bias vec
    u2 = np.zeros((128, 4), np.float32)
    for j in range(2):
        u2[:, 2 * j] = u[128 * j:128 * (j + 1)]
        u2[:, 2 * j + 1] = u[128 * j:128 * (j + 1)]    # fp32r even-N dup
    arrs["u2"] = u2
    w1h, b1h, w2h, b2h, w3h, b3h = [np.asarray(a, np.float32) for a in head_params]
    # comb order in-kernel: [pooled (256) | obs (12) | act (2)]
    arrs["w1h_a"] = np.ascontiguousarray(w1h[D_OBS : D_OBS + 128])          # [128,256]
    arrs["w1h_b"] = np.ascontiguousarray(w1h[D_OBS + 128 : D_OBS + 256])    # [128,256]
    arrs["w1h_c"] = np.ascontiguousarray(
        np.concatenate([w1h[:D_OBS], w1h[D_OBS + 256 :]], 0)                # [14,256]
    )
    arrs["b1hc"] = _col2(b1h)
    arrs["w2h"] = _r2(w2h)
    arrs["b2hc"] = _col2(b2h)
    arrs["w3h"] = np.ascontiguousarray(w3h.reshape(2, 128).T)               # [128,2]
    arrs["b3h"] = np.ascontiguousarray(b3h.reshape(1, 1))                   # [1,1]
    arrs["iden"] = np.eye(128, dtype=np.float32)
    arrs["ones128"] = np.ones((1, 128), np.float32)
    return arrs


def _prep_core(obs, obstacles, act, c):
    s = slice(c * BPC, (c + 1) * BPC)
    obs_c = np.asarray(obs[s], np.float32)            # [32,12]
    act_c = np.asarray(act[s], np.float32)            # [32,2]
    obst_c = np.asarray(obstacles[s], np.float32)     # [32,5,512]
    # compact each batch element's obstacles valid-first, keep the first N
    # (N was sized so every dropped obstacle is masked; padding stays masked)
    comp = np.empty((BPC, 5, N), np.float32)
    for b in range(BPC):
        m = obst_c[b, OBD, :]
        idx = np.concatenate([np.nonzero(m > 0)[0], np.nonzero(m <= 0)[0]])[:N]
        comp[b] = obst_c[b][:, idx]
    arrs = {}
    arrs["obst"] = np.ascontiguousarray(
        comp[:, :OBD, :].transpose(1, 0, 2).reshape(OBD, BPC * N)
    )                                                  # [4, 32*N]
    arrs["obsT"] = np.ascontiguousarray(obs_c.T)       # [12,32]
    arrs["headxT"] = np.ascontiguousarray(
        np.concatenate([obs_c.T, act_c.T], 0)
    )                                                  # [14,32]
    mask = comp[:, OBD, :]                             # [32,N]
    # [p, kc*32 + b] = mask[b, kc*128 + p]
    maskT = mask.T.reshape(NCH, 128, BPC).transpose(1, 0, 2).reshape(
        128, NCH * BPC)
    arrs["maskT"] = np.ascontiguousarray(maskT)
    arrs["negbT"] = np.ascontiguousarray((maskT - 1.0) * 1e9)
    return arrs


# name -> (shape, dtype): f32r for anything a matmul consumes
_SHARED_SPECS = {}
for _m in ("q", "k", "v"):
    _SHARED_SPECS.update({
        f"w1o_{_m}": ([D_OBS, H], F32R),
        f"w1t_{_m}": ([OBD, H], F32R),
        f"b1c_{_m}": ([128, 2], F32),
        f"w2_{_m}": ([128, 2 * H], F32R),
        f"b2c_{_m}": ([128, 2], F32),
    })
_SHARED_SPECS.update({
    "w3_v": ([128, 2 * H], F32R),
    "g_r2": ([128, 2 * H], F32R),
    "u2": ([128, 4], F32R),
    "b3v": ([1, H + 2], F32R),
    "w1h_a": ([128, H], F32R),
    "w1h_b": ([128, H], F32R),
    "w1h_c": ([14, H], F32R),
    "b1hc": ([128, 2], F32),
    "w2h": ([128, 2 * H], F32R),
    "b2hc": ([128, 2], F32),
    "w3h": ([128, 2], F32R),
    "b3h": ([1, 1], F32),
    "iden": ([128, 128], F32R),
    "ones128": ([1, 128], F32R),
})
def _core_specs():
    return {
        "obst": ([OBD, BPC * N], F32R),  # DMA'd 3x into bands 0/32/64
        "obsT": ([D_OBS, BPC], F32R),
        "headxT": ([14, BPC], F32R),
        "maskT": ([128, NCH * BPC], F32),
        "negbT": ([128, NCH * BPC], F32),
    }


def _set_n(n):
    global N, NCH
    N = n
    NCH = n // 128


def _build():
    nc = bacc.Bacc("TRN2", target_bir_lowering=False, debug=False,
                   num_devices=N_CORES)
    d = {}
    for name, (shape, dt) in {**_SHARED_SPECS, **_core_specs()}.items():
        d[name] = nc.dram_tensor(name, shape, dt, kind="ExternalInput")
    out_dram = nc.dram_tensor("out", [1, BPC], F32, kind="ExternalOutput")

    with tile.TileContext(nc) as tc:
        _emit(nc, tc, d, out_dram)
    nc.compile()
    return nc


def _emit(nc, tc, d, out_dram):
    from contextlib import ExitStack
    ctx = ExitStack()
    with ctx:
        const = ctx.enter_context(tc.tile_pool(name="const", bufs=1))
        pa1 = ctx.enter_context(tc.tile_pool(name="pa1", bufs=POOLS["pa1b"]))
        pa2 = ctx.enter_context(tc.tile_pool(name="pa2", bufs=POOLS["pa2"]))
        pqk = ctx.enter_context(tc.tile_pool(name="pqk", bufs=POOLS["pqk"]))
        pv = ctx.enter_context(tc.tile_pool(name="pv", bufs=POOLS["pv"]))
        pE = ctx.enter_context(tc.tile_pool(name="pE", bufs=POOLS["pE"]))
        psc = ctx.enter_context(tc.tile_pool(name="psc", bufs=POOLS["psc"]))
        pm = ctx.enter_context(tc.tile_pool(name="pm", bufs=POOLS["pm"]))
        ptiny = ctx.enter_context(tc.tile_pool(name="ptiny", bufs=8))
        pout = ctx.enter_context(tc.tile_pool(name="pout", bufs=2))
        pp512 = ctx.enter_context(tc.tile_pool(name="pp512", bufs=POOLS["pp512"], space="PSUM"))
        ppl1 = ctx.enter_context(tc.tile_pool(name="ppl1", bufs=POOLS["ppl1"], space="PSUM"))
        ppa = ctx.enter_context(tc.tile_pool(name="ppa", bufs=POOLS["ppa"], space="PSUM"))
        ppsm = ppa  # C/head psums share the attention psum pool

        # ---- load everything to SBUF (first-use order so compute can
        # start as soon as the L1 inputs land, instead of after all 2.7MB) ----
        all_specs = {**_SHARED_SPECS, **_core_specs()}
        first = ["obsT", "w1o_q", "w1o_k", "w1o_v", "b1c_q", "b1c_k", "b1c_v",
                 "w1t_q", "w1t_k", "w1t_v", "obst",
                 "w2_q", "b2c_q", "w2_k", "b2c_k", "w2_v", "b2c_v",
                 "g_r2", "u2", "w3_v", "b3v", "ones128",
                 "negbT", "maskT", "iden"]
        order = first + [n for n in all_specs if n not in first]
        sb = {}
        skip_plain = {"obst", "w1t_q", "w1t_k", "w1t_v"}
        # packed tiles: q/k/v L1 runs as 3 concurrent row-group matmuls
        # (tile_position rows 0/32/64), so weights and the obstacle rhs are
        # replicated into those partition bands
        obst3 = const.tile([128, BPC * N], F32R, tag="obst3", name="obst3")
        w1t_pack = const.tile([128, 2 * 128], F32R, tag="w1t_pack",
                              name="w1t_pack")
        def load_plain(names):
            for name in names:
                shape, dt = all_specs[name]
                t = const.tile(shape, dt, tag=name, name=name)
                nc.sync.dma_start(t[:], d[name][:])
                sb[name] = t
        # tiny setup tensors first (C matmuls + L1 weights), then the three
        # 256KB obstacle bands, then everything else in first-use order
        setup = ["obsT", "w1o_q", "w1o_k", "w1o_v", "b1c_q", "b1c_k", "b1c_v"]
        load_plain(setup)
        for i, m in enumerate(("q", "k", "v")):
            nc.sync.dma_start(w1t_pack[32 * i:32 * i + OBD, :], d[f"w1t_{m}"][:])
        for i in range(3):
            nc.sync.dma_start(obst3[32 * i:32 * i + OBD, :], d["obst"][:])
        load_plain([n for n in order if n not in skip_plain and n not in setup])

        # ---- per-core setup: C^T[mlp] = W1[:12].T @ obs + b1 (per-partition) ----
        cmt = {}
        for m in ("q", "k", "v"):
            for j in range(2):
                ps = ppsm.tile([128, BPC], F32, tag="psa")
                nc.tensor.matmul(ps[:], sb[f"w1o_{m}"][:, j * 128:(j + 1) * 128],
                                 sb["obsT"][:], start=True, stop=True)
                ct = const.tile([128, BPC], F32, tag=f"cmt_{m}{j}")
                nc.scalar.activation(ct[:], ps[:], AF.Identity,
                                     bias=sb[f"b1c_{m}"][:, j:j + 1])
                cmt[(m, j)] = ct

        pooledT = [const.tile([128, BPC], F32R, tag=f"pooled{j}", name=f"pooled{j}")
                   for j in range(2)]

        # ---- main batch loop: pairs of batch elems, stage-interleaved so the
        # PE always has the sibling batch's matmuls to run while ACT/DVE
        # produce this batch's activations ----
        def stage_l1(b):
            # one row-group pack per h-chunk j: q/k/v L1 matmuls execute
            # concurrently in the PE array (K=4 each, rows 0/32/64)
            a1 = {m: [] for m in ("q", "k", "v")}
            for j in range(2):
                for i, m in enumerate(("q", "k", "v")):
                    ps = ppl1.tile([128, N], F32, tag="psl1", name="l1ps")
                    nc.tensor.matmul(
                        ps[:],
                        w1t_pack[32 * i:32 * i + OBD, j * 128:(j + 1) * 128],
                        obst3[32 * i:32 * i + OBD, b * N:(b + 1) * N],
                        start=True, stop=True)
                    a1t = pa1.tile([128, N], F32R, tag="a1", name="a1")
                    nc.any.tensor_scalar(a1t[:], ps[:], cmt[(m, j)][:, b:b + 1],
                                         0.0, OP.add, OP.max)
                    a1[m].append(a1t)
            return a1

        def stage_l2(b, m, a1_m):
            a2_m = []
            for j in range(2):
                ps = pp512.tile([128, N], F32, tag="ps512", name="l2ps")
                for ks in range(2):
                    nc.tensor.matmul(
                        ps[:],
                        sb[f"w2_{m}"][:, ks * H + j * 128: ks * H + j * 128 + 128],
                        a1_m[ks][:], start=(ks == 0), stop=(ks == 1))
                a2t = pa2.tile([128, N], F32R, tag="a2", name="a2")
                nc.any.tensor_scalar(a2t[:], ps[:],
                                     sb[f"b2c_{m}"][:, j:j + 1], 0.0,
                                     OP.add, OP.max)
                a2_m.append(a2t)
            return a2_m

        def stage_Y(b, a2k):
            # Y = G^T-matmul of a2k; scores = Y^T a2q (q/k L3 eliminated)
            Y = []
            for j in range(2):
                ps = ppl1.tile([128, N], F32, tag="psl1", name="yps")
                for ks in range(2):
                    nc.tensor.matmul(
                        ps[:],
                        sb["g_r2"][:, ks * H + j * 128: ks * H + j * 128 + 128],
                        a2k[ks][:], start=(ks == 0), stop=(ks == 1))
                yt = pqk.tile([128, N], F32R, tag="YT", name="yt")
                nc.vector.tensor_scalar(yt[:], ps[:], 0.0, None, OP.add)
                Y.append(yt)
            return Y

        def stage_tT(b, a2k):
            # per-key score bias t = (W3k b3q)^T a2k, computed transposed
            # [128,1] per key chunk and merged with the mask bias for exp
            eb = []
            for kc in range(NCH):
                tps = ppa.tile([128, 2], F32, tag="psa", name="tps")
                for j in range(2):
                    nc.tensor.matmul(tps[:],
                                     a2k[j][:, kc * 128:(kc + 1) * 128],
                                     sb["u2"][:, 2 * j:2 * j + 2],
                                     start=(j == 0), stop=(j == 1))
                e = ptiny.tile([128, 1], F32, tag="ebias", name="ebias")
                nc.vector.tensor_tensor(
                    e[:], tps[:, 0:1],
                    sb["negbT"][:, kc * BPC + b: kc * BPC + b + 1], OP.add)
                eb.append(e)
            return eb

        def stage_l3v(b, a2_m):
            # flipped layout v[keys, h] (+bias via ones-row matmul, relu);
            # col H..H+2 set to 1.0: col H gives the softmax denominator in
            # the attnout matmul; col H+1 is fp32r even-free-size padding.
            v_sb = []
            for rc in range(NCH):
                ps = ppa.tile([128, H + 2], F32, tag="psa", name="vps")
                for ks in range(2):
                    nc.tensor.matmul(ps[:, 0:H],
                                     a2_m[ks][:, rc * 128:(rc + 1) * 128],
                                     sb["w3_v"][:, ks * H:(ks + 1) * H],
                                     start=(ks == 0), stop=False)
                nc.tensor.matmul(ps[:], sb["ones128"][:], sb["b3v"][:],
                                 start=False, stop=True)
                vt = pv.tile([128, H + 2], F32R, tag="vsb", name="vsb")
                # relu covers the ones columns too: max(1, 0) = 1
                nc.vector.tensor_scalar(vt[:], ps[:], 0.0, None, OP.max)
                v_sb.append(vt)
            return v_sb

        def stage_scores(b, Y, a2q, eb):
            E = []
            for kc in range(NCH):
                ps = pp512.tile([128, N], F32, tag="ps512", name="scps")
                for j in range(2):
                    nc.tensor.matmul(ps[:],
                                     Y[j][:, kc * 128:(kc + 1) * 128],
                                     a2q[j][:], start=(j == 0), stop=(j == 1))
                e = pE.tile([128, N], F32R, tag="E", name="E")
                nc.scalar.activation(e[:], ps[:], AF.Exp, bias=eb[kc][:])
                E.append(e)
            return E

        def stage_attnout(b, E, v_sb):
            scaled = []
            for qc in range(NCH):
                ps = ppa.tile([128, H + 2], F32, tag="psa", name="aops")
                for kc in range(NCH):
                    nc.tensor.matmul(ps[:],
                                     E[kc][:, qc * 128:(qc + 1) * 128],
                                     v_sb[kc][:], start=(kc == 0),
                                     stop=(kc == NCH - 1))
                # every batch elem has >0 valid keys (verified on host data),
                # so S>0 and the reciprocal is finite
                rec = ptiny.tile([128, 1], F32, tag="rec", name="rec")
                nc.vector.reciprocal(rec[:], ps[:, H:H + 1])
                sc = psc.tile([128, H], F32, tag="scaled", name="scaled")
                nc.any.tensor_scalar(
                    sc[:], ps[:, 0:H], rec[:],
                    sb["maskT"][:, qc * BPC + b: qc * BPC + b + 1],
                    OP.mult, OP.mult)
                scaled.append(sc)
            return scaled

        def stage_pool_max(b, scaled):
            # pairwise max tree over the NCH scaled chunks; final node f32r
            work = list(scaled)
            cnt = 0
            while len(work) > 2:
                m01 = pm.tile([128, H], F32, tag=f"mx{cnt % 2}", name="mx")
                nc.any.tensor_tensor(m01[:], work[0][:], work[1][:], OP.max)
                work = [m01] + work[2:]
                cnt += 1
            m3 = pm.tile([128, H], F32R, tag="m3", name="m3", bufs=6)
            if len(work) == 2:
                nc.any.tensor_tensor(m3[:], work[0][:], work[1][:], OP.max)
            else:
                nc.any.tensor_scalar(m3[:], work[0][:], 0.0, None, OP.add)
            return m3

        def stage_pool_reduce(b, m3):
            # emitted one pair late: keeps the PE transposes (which wait on
            # the DVE max chain) from stalling the next pair's L1 matmuls
            for hc in range(2):
                trp = ppa.tile([128, 128], F32R, tag="psa", name="trp")
                nc.tensor.transpose(trp[:], m3[:, hc * 128:(hc + 1) * 128],
                                    sb["iden"][:])
                nc.vector.tensor_reduce(pooledT[hc][:, b:b + 1], trp[:],
                                        mybir.AxisListType.X, OP.max)

        pending_pool = []
        for p in range(REPEAT * (BPC // 2)):
            bb = ((2 * p) % BPC, (2 * p + 1) % BPC)
            st = {b: {} for b in bb}
            for b in bb:
                a1 = stage_l1(b)
                for m in ("q", "k", "v"):
                    st[b][f"a1{m}"] = a1[m]
            done_pending = False
            for m in ("q", "k", "v"):
                for b in bb:
                    st[b][f"a2{m}"] = stage_l2(b, m, st[b][f"a1{m}"])
                if not done_pending:
                    # previous pair's pool transposes, emitted here so they
                    # never stall this pair's L1/L2 matmuls on the PE queue
                    for pb, pm3 in pending_pool:
                        stage_pool_reduce(pb, pm3)
                    pending_pool = []
                    done_pending = True
                for b in bb:
                    if m == "v":
                        st[b]["v"] = stage_l3v(b, st[b]["a2v"])
                    elif m == "k":
                        st[b]["Y"] = stage_Y(b, st[b]["a2k"])
                        st[b]["eb"] = stage_tT(b, st[b]["a2k"])
            for b in bb:
                st[b]["E"] = stage_scores(b, st[b]["Y"], st[b]["a2q"], st[b]["eb"])
            for b in bb:
                st[b]["sc"] = stage_attnout(b, st[b]["E"], st[b]["v"])
            for b in bb:
                pending_pool.append((b, stage_pool_max(b, st[b]["sc"])))
        for pb, pm3 in pending_pool:
            stage_pool_reduce(pb, pm3)

        # ---- head MLP on all 32 batch elems (transposed [h, b]) ----
        a1h = []
        for j in range(2):
            ps = ppsm.tile([128, BPC], F32, tag="psa")
            nc.tensor.matmul(ps[:], sb["w1h_a"][:, j * 128:(j + 1) * 128],
                             pooledT[0][:], start=True, stop=False)
            nc.tensor.matmul(ps[:], sb["w1h_b"][:, j * 128:(j + 1) * 128],
                             pooledT[1][:], start=False, stop=False)
            nc.tensor.matmul(ps[:], sb["w1h_c"][:, j * 128:(j + 1) * 128],
                             sb["headxT"][:], start=False, stop=True)
            a = pout.tile([128, BPC], F32R, tag="a1h")
            nc.scalar.activation(a[:], ps[:], AF.Relu, bias=sb["b1hc"][:, j:j + 1])
            a1h.append(a)
        a2h = []
        for j in range(2):
            ps = ppsm.tile([128, BPC], F32, tag="psa")
            for ks in range(2):
                nc.tensor.matmul(ps[:],
                                 sb["w2h"][:, ks * H + j * 128: ks * H + j * 128 + 128],
                                 a1h[ks][:], start=(ks == 0), stop=(ks == 1))
            a = pout.tile([128, BPC], F32R, tag="a2h")
            nc.scalar.activation(a[:], ps[:], AF.Relu, bias=sb["b2hc"][:, j:j + 1])
            a2h.append(a)
        ps = ppsm.tile([1, BPC], F32, tag="psa")
        for ks in range(2):
            nc.tensor.matmul(ps[:], sb["w3h"][:, ks:ks + 1], a2h[ks][:],
                             start=(ks == 0), stop=(ks == 1))
        ot = pout.tile([1, BPC], F32, tag="osb")
        nc.vector.tensor_scalar(ot[:], ps[:], sb["b3h"][:, 0:1], None, OP.add)
        nc.sync.dma_start(out_dram[:], ot[:])


def kernel(obs, obstacles, act, q_params, k_params, v_params, head_params):
    global _last_results
    maxvalid = int((np.asarray(obstacles)[:, OBD, :] > 0).sum(axis=1).max())
    _set_n(min(NFULL, max(128, -(-maxvalid // 128) * 128)))
    shared = _prep_shared(q_params, k_params, v_params, head_params)
    in_maps = []
    for c in range(N_CORES):
        m = dict(shared)
        m.update(_prep_core(obs, obstacles, act, c))
        in_maps.append(m)
    nc = _build()
    res = run_bass_kernel_spmd(nc, in_maps, core_ids=list(range(N_CORES)))
    _last_results = res
    out = np.concatenate([res.results[c]["out"][0] for c in range(N_CORES)])
    return out.astype(np.float32)


# revision 36
# speedup vs baseline: 1.3951x; 1.0012x over previous
"""Trainium2 Bass kernel for the AttentionQFunction problem.

Contract: kernel(**inputs) takes FULL inputs (B=256) and returns the FULL
[256] float32 output. Internally the batch is sharded 32-per-core across 8
NeuronCores (pure data parallel); the small MLP weights are replicated.

Sparse compaction: masked obstacles contribute nothing to the output
(keys are excluded from softmax; query rows are zeroed before the max-pool,
and all pooled values are >= 0), so the host reorders each batch element's
obstacles valid-first and the kernel is built for N = the max valid count
rounded up to 128 (384 for the reference inputs; trailing entries stay
masked so correctness holds for any input, degrading to N=512 worst case).

Math (per batch element b, N compacted obstacles, H=256):
  x      = [obs broadcast (12) | obstacle_data (4)]            [N, 16]
  a2q,a2k = first two MLP layers of q/k; v = full v-MLP (relu out)
  scores^T[m,n] = (G a2k_m).a2q_n + t[m], with G = (W3q/16) W3k^T and
           t = (W3k b3q/16).a2k precomputed/reassociated -- the q/k third
           layers are never materialized. The dropped q.b3k and b3q.b3k
           score terms are per-query/constant shifts that cancel exactly
           in the unnormalized ratio U/S below (verified 3e-7 on host).
  E^T    = exp(scores^T + negbias[key] + t[key])  (key mask + t as one
           per-partition exp bias; no max-subtract -- scores are O(0.1))
  U      = E^T-chunks^T @ [v | 1 | 1] -> [queries, H+2]; col H is sum_keys E
           (two ones columns: fp32r needs an even moving-free size)
  out    = U[:, :H] * (mask[q] / U[:, H])        per-partition scale
  pooled = max over queries (pairwise max + PE transpose + free-dim max)
  qval   = head MLP([pooled | obs | act])        (head W1 rows reordered)

All MLP layers run in transposed-activation layout [H, N] so every bias is
per-partition. All matmuls use float32r (1 cycle/row for N>=256 vs 4 for
fp32; ~1e-4 rel err). The K=4 L1 matmuls for q/k/v run concurrently in the
PE array via tile_position row groups 0/32/64 (weights+rhs replicated into
those partition bands). Elementwise ops are emitted as nc.any so the Tile
scheduler load-balances them across ScalarE/VectorE; exp stays on ScalarE.
The pool-stage PE transposes of each batch pair are emitted one pair late so
they never stall the next pair's L1/L2 matmuls, and the softmax-denominator
ones-columns are written by the v bias-row matmul itself (no per-batch
constant-write ops). Cost-model (TimelineSim) predicts ~257us/core at N=384
(PE 82%, DVE 80%, ACT 73% busy); on real silicon the packed L1 (which the
model charges serially, ~50us) should land meaningfully below that.
"""

import numpy as np

import concourse.bass as bass
import concourse.mybir as mybir
import concourse.tile as tile
from concourse import bacc
from concourse.bass_utils import run_bass_kernel_spmd

F32 = mybir.dt.float32
F32R = mybir.dt.float32r
AF = mybir.ActivationFunctionType
OP = mybir.AluOpType

N_CORES = 8
B = 256
BPC = B // N_CORES  # 32 batch elements per core
NFULL = 512         # obstacles in the input
# Masked obstacles contribute nothing to the output (keys excluded from
# softmax, query rows zeroed before max-pool), so the host compacts each
# batch element's obstacles valid-first and the kernel is built for the
# padded max valid count N <= 512. Recomputed from the mask per call.
N = 512
NCH = N // 128
H = 256             # hidden
D_OBS = 12
OBD = 4
ACT_D = 2

_last_results = None  # test.py introspects exec_time_ns from here
REPEAT = 1  # bench.py raises this to measure marginal batch-phase time

# pool sizing knobs (tuned via cost-model sweep in analyze.py)
POOLS = {"pa1": 6, "pa2": 12, "pqk": 8, "pv": 8, "pE": 8, "psc": 8, "pm": 4,
         "pp512": 3, "ppl1": 3, "ppa": 2, "ppsm": 0, "pa1b": 12}


def _r2(w):
    """[256, X] -> [128, 2*X] with col layout ksub*X + c (k-subtile major)."""
    x = w.shape[1]
    return np.ascontiguousarray(
        w.reshape(2, 128, x).transpose(1, 0, 2).reshape(128, 2 * x)
    )


def _col2(v):
    """[256] -> [128, 2], column j = chunk j."""
    return np.ascontiguousarray(v.reshape(2, 128).T)


def _prep_shared(q_params, k_params, v_params, head_params):
    arrs = {}
    for name, p in (("q", q_params), ("k", k_params), ("v", v_params)):
        w1, b1, w2, b2, w3, b3 = [np.asarray(a, np.float32) for a in p]
        arrs[f"w1o_{name}"] = np.ascontiguousarray(w1[:D_OBS])      # [12,256]
        arrs[f"w1t_{name}"] = np.ascontiguousarray(w1[D_OBS:])      # [4,256]
        arrs[f"b1c_{name}"] = _col2(b1)                             # [128,2]
        arrs[f"w2_{name}"] = _r2(w2)                                # [128,512]
        arrs[f"b2c_{name}"] = _col2(b2)
        if name == "v":
            arrs[f"w3_{name}"] = _r2(w3)
        if name == "v":
            # [b3v | 1 | 1]: the trailing ones land in psum cols H:H+2 via
            # the bias-row matmul, giving the softmax-denominator column
            # without a separate constant-write op
            arrs["b3v"] = np.ascontiguousarray(
                np.concatenate([b3, [1.0, 1.0]]).astype(np.float32)[None, :])
    # scores are computed as (G a2k)^T a2q + t[key]: the q.b3k and b3q.b3k
    # score terms are per-query/constant shifts that cancel exactly in the
    # unnormalized-softmax ratio U/S, so they are dropped; 1/16 is folded in
    w3q = np.asarray(q_params[4], np.float32) / 16.0
    b3q = np.asarray(q_params[5], np.float32) / 16.0
    w3k = np.asarray(k_params[4], np.float32)
    G_T = np.ascontiguousarray(w3k @ w3q.T)            # lhsT for Y: [g, h]
    arrs["g_r2"] = _r2(G_T)                            # [128, 512]
    u = w3k @ b3q                                      # [256] per-key bias vec
    u2 = np.zeros((128, 4), np.float32)
    for j in range(2):
        u2[:, 2 * j] = u[128 * j:128 * (j + 1)]
        u2[:, 2 * j + 1] = u[128 * j:128 * (j + 1)]    # fp32r even-N dup
    arrs["u2"] = u2
    w1h, b1h, w2h, b2h, w3h, b3h = [np.asarray(a, np.float32) for a in head_params]
    # comb order in-kernel: [pooled (256) | obs (12) | act (2)]
    arrs["w1h_a"] = np.ascontiguousarray(w1h[D_OBS : D_OBS + 128])          # [128,256]
    arrs["w1h_b"] = np.ascontiguousarray(w1h[D_OBS + 128 : D_OBS + 256])    # [128,256]
    arrs["w1h_c"] = np.ascontiguousarray(
        np.concatenate([w1h[:D_OBS], w1h[D_OBS + 256 :]], 0)                # [14,256]
    )
    arrs["b1hc"] = _col2(b1h)
    arrs["w2h"] = _r2(w2h)
    arrs["b2hc"] = _col2(b2h)
    arrs["w3h"] = np.ascontiguousarray(w3h.reshape(2, 128).T)               # [128,2]
    arrs["b3h"] = np.ascontiguousarray(b3h.reshape(1, 1))                   # [1,1]
    arrs["iden"] = np.eye(128, dtype=np.float32)
    arrs["ones128"] = np.ones((1, 128), np.float32)
    return arrs


def _prep_core(obs, obstacles, act, c):
    s = slice(c * BPC, (c + 1) * BPC)
    obs_c = np.asarray(obs[s], np.float32)            # [32,12]
    act_c = np.asarray(act[s], np.float32)            # [32,2]
    obst_c = np.asarray(obstacles[s], np.float32)     # [32,5,512]
    # compact each batch element's obstacles valid-first, keep the first N
    # (N was sized so every dropped obstacle is masked; padding stays masked)
    comp = np.empty((BPC, 5, N), np.float32)
    for b in range(BPC):
        m = obst_c[b, OBD, :]
        idx = np.concatenate([np.nonzero(m > 0)[0], np.nonzero(m <= 0)[0]])[:N]
        comp[b] = obst_c[b][:, idx]
    arrs = {}
    arrs["obst"] = np.ascontiguousarray(
        comp[:, :OBD, :].transpose(1, 0, 2).reshape(OBD, BPC * N)
    )                                                  # [4, 32*N]
    arrs["obsT"] = np.ascontiguousarray(obs_c.T)       # [12,32]
    arrs["headxT"] = np.ascontiguousarray(
        np.concatenate([obs_c.T, act_c.T], 0)
    )                                                  # [14,32]
    mask = comp[:, OBD, :]                             # [32,N]
    # [p, kc*32 + b] = mask[b, kc*128 + p]
    maskT = mask.T.reshape(NCH, 128, BPC).transpose(1, 0, 2).reshape(
        128, NCH * BPC)
    arrs["maskT"] = np.ascontiguousarray(maskT)
    arrs["negbT"] = np.ascontiguousarray((maskT - 1.0) * 1e9)
    return arrs


# name -> (shape, dtype): f32r for anything a matmul consumes
_SHARED_SPECS = {}
for _m in ("q", "k", "v"):
    _SHARED_SPECS.update({
        f"w1o_{_m}": ([D_OBS, H], F32R),
        f"w1t_{_m}": ([OBD, H], F32R),
        f"b1c_{_m}": ([128, 2], F32),
        f"w2_{_m}": ([128, 2 * H], F32R),
        f"b2c_{_m}": ([128, 2], F32),
    })
_SHARED_SPECS.update({
    "w3_v": ([128, 2 * H], F32R),
    "g_r2": ([128, 2 * H], F32R),
    "u2": ([128, 4], F32R),
    "b3v": ([1, H + 2], F32R),
    "w1h_a": ([128, H], F32R),
    "w1h_b": ([128, H], F32R),
    "w1h_c": ([14, H], F32R),
    "b1hc": ([128, 2], F32),
    "w2h": ([128, 2 * H], F32R),
    "b2hc": ([128, 2], F32),
    "w3h": ([128, 2], F32R),
    "b3h": ([1, 1], F32),
    "iden": ([128, 128], F32R),
    "ones128": ([1, 128], F32R),
})
def _core_specs():
    return {
        "obst": ([OBD, BPC * N], F32R),  # DMA'd 3x into bands 0/32/64
        "obsT": ([D_OBS, BPC], F32R),
        "headxT": ([14, BPC], F32R),
        "maskT": ([128, NCH * BPC], F32),
        "negbT": ([128, NCH * BPC], F32),
    }


def _set_n(n):
    global N, NCH
    N = n
    NCH = n // 128


def _build():
    nc = bacc.Bacc("TRN2", target_bir_lowering=False, debug=False,
                   num_devices=N_CORES)
    d = {}
    for name, (shape, dt) in {**_SHARED_SPECS, **_core_specs()}.items():
        d[name] = nc.dram_tensor(name, shape, dt, kind="ExternalInput")
    out_dram = nc.dram_tensor("out", [1, BPC], F32, kind="ExternalOutput")

    with tile.TileContext(nc) as tc:
        _emit(nc, tc, d, out_dram)
    nc.compile()
    return nc


def _emit(nc, tc, d, out_dram):
    from contextlib import ExitStack
    ctx = ExitStack()
    with ctx:
        const = ctx.enter_context(tc.tile_pool(name="const", bufs=1))
        pa1 = ctx.enter_context(tc.tile_pool(name="pa1", bufs=POOLS["pa1b"]))
        pa2 = ctx.enter_context(tc.tile_pool(name="pa2", bufs=POOLS["pa2"]))
        pqk = ctx.enter_context(tc.tile_pool(name="pqk", bufs=POOLS["pqk"]))
        pv = ctx.enter_context(tc.tile_pool(name="pv", bufs=POOLS["pv"]))
        pE = ctx.enter_context(tc.tile_pool(name="pE", bufs=POOLS["pE"]))
        psc = ctx.enter_context(tc.tile_pool(name="psc", bufs=POOLS["psc"]))
        pm = ctx.enter_context(tc.tile_pool(name="pm", bufs=POOLS["pm"]))
        ptiny = ctx.enter_context(tc.tile_pool(name="ptiny", bufs=8))
        pout = ctx.enter_context(tc.tile_pool(name="pout", bufs=2))
        pp512 = ctx.enter_context(tc.tile_pool(name="pp512", bufs=POOLS["pp512"], space="PSUM"))
        ppl1 = ctx.enter_context(tc.tile_pool(name="ppl1", bufs=POOLS["ppl1"], space="PSUM"))
        ppa = ctx.enter_context(tc.tile_pool(name="ppa", bufs=POOLS["ppa"], space="PSUM"))
        ppsm = ppa  # C/head psums share the attention psum pool

        # ---- load everything to SBUF (first-use order so compute can
        # start as soon as the L1 inputs land, instead of after all 2.7MB) ----
        all_specs = {**_SHARED_SPECS, **_core_specs()}
        first = ["obsT", "w1o_q", "w1o_k", "w1o_v", "b1c_q", "b1c_k", "b1c_v",
                 "w1t_q", "w1t_k", "w1t_v", "obst",
                 "w2_q", "b2c_q", "w2_k", "b2c_k", "w2_v", "b2c_v",
                 "g_r2", "u2", "w3_v", "b3v", "ones128",
                 "negbT", "maskT", "iden"]
        order = first + [n for n in all_specs if n not in first]
        sb = {}
        skip_plain = {"obst", "w1t_q", "w1t_k", "w1t_v"}
        # packed tiles: q/k/v L1 runs as 3 concurrent row-group matmuls
        # (tile_position rows 0/32/64), so weights and the obstacle rhs are
        # replicated into those partition bands
        obst3 = const.tile([128, BPC * N], F32R, tag="obst3", name="obst3")
        w1t_pack = const.tile([128, 2 * 128], F32R, tag="w1t_pack",
                              name="w1t_pack")
        def load_plain(names):
            for name in names:
                shape, dt = all_specs[name]
                t = const.tile(shape, dt, tag=name, name=name)
                nc.sync.dma_start(t[:], d[name][:])
                sb[name] = t
        # tiny setup tensors first (C matmuls + L1 weights), then the three
        # 256KB obstacle bands, then everything else in first-use order
        setup = ["obsT", "w1o_q", "w1o_k", "w1o_v", "b1c_q", "b1c_k", "b1c_v"]
        load_plain(setup)
        for i, m in enumerate(("q", "k", "v")):
            nc.sync.dma_start(w1t_pack[32 * i:32 * i + OBD, :], d[f"w1t_{m}"][:])
        for i in range(3):
            nc.sync.dma_start(obst3[32 * i:32 * i + OBD, :], d["obst"][:])
        load_plain([n for n in order if n not in skip_plain and n not in setup])

        # ---- per-core setup: C^T[mlp] = W1[:12].T @ obs + b1 (per-partition) ----
        cmt = {}
        for m in ("q", "k", "v"):
            for j in range(2):
                ps = ppsm.tile([128, BPC], F32, tag="psa")
                nc.tensor.matmul(ps[:], sb[f"w1o_{m}"][:, j * 128:(j + 1) * 128],
                                 sb["obsT"][:], start=True, stop=True)
                ct = const.tile([128, BPC], F32, tag=f"cmt_{m}{j}")
                nc.scalar.activation(ct[:], ps[:], AF.Identity,
                                     bias=sb[f"b1c_{m}"][:, j:j + 1])
                cmt[(m, j)] = ct

        pooledT = [const.tile([128, BPC], F32R, tag=f"pooled{j}", name=f"pooled{j}")
                   for j in range(2)]

        # ---- main batch loop: pairs of batch elems, stage-interleaved so the
        # PE always has the sibling batch's matmuls to run while ACT/DVE
        # produce this batch's activations ----
        def stage_l1(b):
            # one row-group pack per h-chunk j: q/k/v L1 matmuls execute
            # concurrently in the PE array (K=4 each, rows 0/32/64)
            a1 = {m: [] for m in ("q", "k", "v")}
            for j in range(2):
                for i, m in enumerate(("q", "k", "v")):
                    ps = ppl1.tile([128, N], F32, tag="psl1", name="l1ps")
                    nc.tensor.matmul(
                        ps[:],
                        w1t_pack[32 * i:32 * i + OBD, j * 128:(j + 1) * 128],
                        obst3[32 * i:32 * i + OBD, b * N:(b + 1) * N],
                        start=True, stop=True)
                    a1t = pa1.tile([128, N], F32R, tag="a1", name="a1")
                    nc.any.tensor_scalar(a1t[:], ps[:], cmt[(m, j)][:, b:b + 1],
                                         0.0, OP.add, OP.max)
                    a1[m].append(a1t)
            return a1

        def stage_l2(b, m, a1_m):
            a2_m = []
            for j in range(2):
                ps = pp512.tile([128, N], F32, tag="ps512", name="l2ps")
                for ks in range(2):
                    nc.tensor.matmul(
                        ps[:],
                        sb[f"w2_{m}"][:, ks * H + j * 128: ks * H + j * 128 + 128],
                        a1_m[ks][:], start=(ks == 0), stop=(ks == 1))
                a2t = pa2.tile([128, N], F32R, tag="a2", name="a2")
                nc.any.tensor_scalar(a2t[:], ps[:],
                                     sb[f"b2c_{m}"][:, j:j + 1], 0.0,
                                     OP.add, OP.max)
                a2_m.append(a2t)
            return a2_m

        def stage_Y(b, a2k):
            # Y = G^T-matmul of a2k; scores = Y^T a2q (q/k L3 eliminated)
            Y = []
            for j in range(2):
                ps = ppl1.tile([128, N], F32, tag="psl1", name="yps")
                for ks in range(2):
                    nc.tensor.matmul(
                        ps[:],
                        sb["g_r2"][:, ks * H + j * 128: ks * H + j * 128 + 128],
                        a2k[ks][:], start=(ks == 0), stop=(ks == 1))
                yt = pqk.tile([128, N], F32R, tag="YT", name="yt")
                nc.any.tensor_scalar(yt[:], ps[:], 0.0, None, OP.add)
                Y.append(yt)
            return Y

        def stage_tT(b, a2k):
            # per-key score bias t = (W3k b3q)^T a2k, computed transposed
            # [128,1] per key chunk and merged with the mask bias for exp
            eb = []
            for kc in range(NCH):
                tps = ppa.tile([128, 2], F32, tag="psa", name="tps")
                for j in range(2):
                    nc.tensor.matmul(tps[:],
                                     a2k[j][:, kc * 128:(kc + 1) * 128],
                                     sb["u2"][:, 2 * j:2 * j + 2],
                                     start=(j == 0), stop=(j == 1))
                e = ptiny.tile([128, 1], F32, tag="ebias", name="ebias")
                nc.vector.tensor_tensor(
                    e[:], tps[:, 0:1],
                    sb["negbT"][:, kc * BPC + b: kc * BPC + b + 1], OP.add)
                eb.append(e)
            return eb

        def stage_l3v(b, a2_m):
            # flipped layout v[keys, h] (+bias via ones-row matmul, relu);
            # col H..H+2 set to 1.0: col H gives the softmax denominator in
            # the attnout matmul; col H+1 is fp32r even-free-size padding.
            v_sb = []
            for rc in range(NCH):
                ps = ppa.tile([128, H + 2], F32, tag="psa", name="vps")
                for ks in range(2):
                    nc.tensor.matmul(ps[:, 0:H],
                                     a2_m[ks][:, rc * 128:(rc + 1) * 128],
                                     sb["w3_v"][:, ks * H:(ks + 1) * H],
                                     start=(ks == 0), stop=False)
                nc.tensor.matmul(ps[:], sb["ones128"][:], sb["b3v"][:],
                                 start=False, stop=True)
                vt = pv.tile([128, H + 2], F32R, tag="vsb", name="vsb")
                # relu covers the ones columns too: max(1, 0) = 1
                nc.vector.tensor_scalar(vt[:], ps[:], 0.0, None, OP.max)
                v_sb.append(vt)
            return v_sb

        def stage_scores(b, Y, a2q, eb):
            E = []
            for kc in range(NCH):
                ps = pp512.tile([128, N], F32, tag="ps512", name="scps")
                for j in range(2):
                    nc.tensor.matmul(ps[:],
                                     Y[j][:, kc * 128:(kc + 1) * 128],
                                     a2q[j][:], start=(j == 0), stop=(j == 1))
                e = pE.tile([128, N], F32R, tag="E", name="E")
                nc.scalar.activation(e[:], ps[:], AF.Exp, bias=eb[kc][:])
                E.append(e)
            return E

        def stage_attnout(b, E, v_sb):
            scaled = []
            for qc in range(NCH):
                ps = ppa.tile([128, H + 2], F32, tag="psa", name="aops")
                for kc in range(NCH):
                    nc.tensor.matmul(ps[:],
                                     E[kc][:, qc * 128:(qc + 1) * 128],
                                     v_sb[kc][:], start=(kc == 0),
                                     stop=(kc == NCH - 1))
                # every batch elem has >0 valid keys (verified on host data),
                # so S>0 and the reciprocal is finite
                rec = ptiny.tile([128, 1], F32, tag="rec", name="rec")
                nc.vector.reciprocal(rec[:], ps[:, H:H + 1])
                sc = psc.tile([128, H], F32, tag="scaled", name="scaled")
                nc.any.tensor_scalar(
                    sc[:], ps[:, 0:H], rec[:],
                    sb["maskT"][:, qc * BPC + b: qc * BPC + b + 1],
                    OP.mult, OP.mult)
                scaled.append(sc)
            return scaled

        def stage_pool_max(b, scaled):
            # pairwise max tree over the NCH scaled chunks; final node f32r
            work = list(scaled)
            cnt = 0
            while len(work) > 2:
                m01 = pm.tile([128, H], F32, tag=f"mx{cnt % 2}", name="mx")
                nc.any.tensor_tensor(m01[:], work[0][:], work[1][:], OP.max)
                work = [m01] + work[2:]
                cnt += 1
            m3 = pm.tile([128, H], F32R, tag="m3", name="m3", bufs=6)
            if len(work) == 2:
                nc.any.tensor_tensor(m3[:], work[0][:], work[1][:], OP.max)
            else:
                nc.any.tensor_scalar(m3[:], work[0][:], 0.0, None, OP.add)
            return m3

        def stage_pool_reduce(b, m3):
            # emitted one pair late: keeps the PE transposes (which wait on
            # the DVE max chain) from stalling the next pair's L1 matmuls
            for hc in range(2):
                trp = ppa.tile([128, 128], F32R, tag="psa", name="trp")
                nc.tensor.transpose(trp[:], m3[:, hc * 128:(hc + 1) * 128],
                                    sb["iden"][:])
                nc.vector.tensor_reduce(pooledT[hc][:, b:b + 1], trp[:],
                                        mybir.AxisListType.X, OP.max)

        pending_pool = []
        for p in range(REPEAT * (BPC // 2)):
            bb = ((2 * p) % BPC, (2 * p + 1) % BPC)
            st = {b: {} for b in bb}
            for b in bb:
                a1 = stage_l1(b)
                for m in ("q", "k", "v"):
                    st[b][f"a1{m}"] = a1[m]
            done_pending = False
            for m in ("q", "k", "v"):
                for b in bb:
                    st[b][f"a2{m}"] = stage_l2(b, m, st[b][f"a1{m}"])
                if not done_pending:
                    # previous pair's pool transposes, emitted here so they
                    # never stall this pair's L1/L2 matmuls on the PE queue
                    for pb, pm3 in pending_pool:
                        stage_pool_reduce(pb, pm3)
                    pending_pool = []
                    done_pending = True
                for b in bb:
                    if m == "v":
                        st[b]["v"] = stage_l3v(b, st[b]["a2v"])
                    elif m == "k":
                        st[b]["Y"] = stage_Y(b, st[b]["a2k"])
                        st[b]["eb"] = stage_tT(b, st[b]["a2k"])
            for b in bb:
                st[b]["E"] = stage_scores(b, st[b]["Y"], st[b]["a2q"], st[b]["eb"])
            for b in bb:
                st[b]["sc"] = stage_attnout(b, st[b]["E"], st[b]["v"])
            for b in bb:
                pending_pool.append((b, stage_pool_max(b, st[b]["sc"])))
        for pb, pm3 in pending_pool:
            stage_pool_reduce(pb, pm3)

        # ---- head MLP on all 32 batch elems (transposed [h, b]) ----
        a1h = []
        for j in range(2):
            ps = ppsm.tile([128, BPC], F32, tag="psa")
            nc.tensor.matmul(ps[:], sb["w1h_a"][:, j * 128:(j + 1) * 128],
                             pooledT[0][:], start=True, stop=False)
            nc.tensor.matmul(ps[:], sb["w1h_b"][:, j * 128:(j + 1) * 128],
                             pooledT[1][:], start=False, stop=False)
            nc.tensor.matmul(ps[:], sb["w1h_c"][:, j * 128:(j + 1) * 128],
                             sb["headxT"][:], start=False, stop=True)
            a = pout.tile([128, BPC], F32R, tag="a1h")
            nc.scalar.activation(a[:], ps[:], AF.Relu, bias=sb["b1hc"][:, j:j + 1])
            a1h.append(a)
        a2h = []
        for j in range(2):
            ps = ppsm.tile([128, BPC], F32, tag="psa")
            for ks in range(2):
                nc.tensor.matmul(ps[:],
                                 sb["w2h"][:, ks * H + j * 128: ks * H + j * 128 + 128],
                                 a1h[ks][:], start=(ks == 0), stop=(ks == 1))
            a = pout.tile([128, BPC], F32R, tag="a2h")
            nc.scalar.activation(a[:], ps[:], AF.Relu, bias=sb["b2hc"][:, j:j + 1])
            a2h.append(a)
        ps = ppsm.tile([1, BPC], F32, tag="psa")
        for ks in range(2):
            nc.tensor.matmul(ps[:], sb["w3h"][:, ks:ks + 1], a2h[ks][:],
                             start=(ks == 0), stop=(ks == 1))
        ot = pout.tile([1, BPC], F32, tag="osb")
        nc.vector.tensor_scalar(ot[:], ps[:], sb["b3h"][:, 0:1], None, OP.add)
        nc.sync.dma_start(out_dram[:], ot[:])


def kernel(obs, obstacles, act, q_params, k_params, v_params, head_params):
    global _last_results
    maxvalid = int((np.asarray(obstacles)[:, OBD, :] > 0).sum(axis=1).max())
    _set_n(min(NFULL, max(128, -(-maxvalid // 128) * 128)))
    shared = _prep_shared(q_params, k_params, v_params, head_params)
    in_maps = []
    for c in range(N_CORES):
        m = dict(shared)
        m.update(_prep_core(obs, obstacles, act, c))
        in_maps.append(m)
    nc = _build()
    res = run_bass_kernel_spmd(nc, in_maps, core_ids=list(range(N_CORES)))
    _last_results = res
    out = np.concatenate([res.results[c]["out"][0] for c in range(N_CORES)])
    return out.astype(np.float32)


# revision 37
# speedup vs baseline: 1.4480x; 1.0379x over previous
"""Trainium2 Bass kernel for the AttentionQFunction problem.

Contract: kernel(**inputs) takes FULL inputs (B=256) and returns the FULL
[256] float32 output. Internally the batch is sharded 32-per-core across 8
NeuronCores (pure data parallel); the small MLP weights are replicated.

Sparse compaction: masked obstacles contribute nothing to the output
(keys are excluded from softmax; query rows are zeroed before the max-pool,
and all pooled values are >= 0), so the host reorders each batch element's
obstacles valid-first and the kernel is built for N = the max valid count
rounded up to 128 (384 for the reference inputs; trailing entries stay
masked so correctness holds for any input, degrading to N=512 worst case).

Math (per batch element b, N compacted obstacles, H=256):
  x      = [obs broadcast (12) | obstacle_data (4)]            [N, 16]
  a2q,a2k = first two MLP layers of q/k; v = full v-MLP (relu out)
  scores^T[m,n] = (G a2k_m).a2q_n + t[m], with G = (W3q/16) W3k^T and
           t = (W3k b3q/16).a2k precomputed/reassociated -- the q/k third
           layers are never materialized. The dropped q.b3k and b3q.b3k
           score terms are per-query/constant shifts that cancel exactly
           in the unnormalized ratio U/S below (verified 3e-7 on host).
  E^T    = exp(scores^T + negbias[key] + t[key])  (key mask + t as one
           per-partition exp bias; no max-subtract -- scores are O(0.1))
  U      = E^T-chunks^T @ [v | 1 | 1] -> [queries, H+2]; col H is sum_keys E
           (two ones columns: fp32r needs an even moving-free size)
  out    = U[:, :H] * (mask[q] / U[:, H])        per-partition scale
  pooled = max over queries (pairwise max + PE transpose + free-dim max)
  qval   = head MLP([pooled | obs | act])        (head W1 rows reordered)

All MLP layers run in transposed-activation layout [H, N] so every bias is
per-partition. All matmuls use float32r (1 cycle/row for N>=256 vs 4 for
fp32; ~1e-4 rel err). The K=4 L1 matmuls for q/k/v run concurrently in the
PE array via tile_position row groups 0/32/64 (weights+rhs replicated into
those partition bands). Elementwise ops are emitted as nc.any so the Tile
scheduler load-balances them across ScalarE/VectorE; exp stays on ScalarE.
The pool-stage PE transposes of each batch pair are emitted one pair late so
they never stall the next pair's L1/L2 matmuls, and the softmax-denominator
ones-columns are written by the v bias-row matmul itself (no per-batch
constant-write ops). Cost-model (TimelineSim) predicts ~257us/core at N=384
(PE 82%, DVE 80%, ACT 73% busy); on real silicon the packed L1 (which the
model charges serially, ~50us) should land meaningfully below that.
"""

import numpy as np

import concourse.bass as bass
import concourse.mybir as mybir
import concourse.tile as tile
from concourse import bacc
from concourse.bass_utils import run_bass_kernel_spmd

F32 = mybir.dt.float32
F32R = mybir.dt.float32r
AF = mybir.ActivationFunctionType
OP = mybir.AluOpType

N_CORES = 8
B = 256
BPC = B // N_CORES  # 32 batch elements per core
NFULL = 512         # obstacles in the input
# Masked obstacles contribute nothing to the output (keys excluded from
# softmax, query rows zeroed before max-pool), so the host compacts each
# batch element's obstacles valid-first and the kernel is built for the
# padded max valid count N <= 512. Recomputed from the mask per call.
N = 512
NCH = N // 128
H = 256             # hidden
D_OBS = 12
OBD = 4
ACT_D = 2

_last_results = None  # test.py introspects exec_time_ns from here
REPEAT = 1  # bench.py raises this to measure marginal batch-phase time

# pool sizing knobs (tuned via cost-model sweep in analyze.py)
POOLS = {"pa1": 6, "pa2": 12, "pqk": 8, "pv": 8, "pE": 8, "psc": 8, "pm": 4,
         "pp512": 3, "ppl1": 3, "ppa": 2, "ppsm": 0, "pa1b": 12}


def _r2(w):
    """[256, X] -> [128, 2*X] with col layout ksub*X + c (k-subtile major)."""
    x = w.shape[1]
    return np.ascontiguousarray(
        w.reshape(2, 128, x).transpose(1, 0, 2).reshape(128, 2 * x)
    )


def _col2(v):
    """[256] -> [128, 2], column j = chunk j."""
    return np.ascontiguousarray(v.reshape(2, 128).T)


def _prep_shared(q_params, k_params, v_params, head_params):
    arrs = {}
    for name, p in (("q", q_params), ("k", k_params), ("v", v_params)):
        w1, b1, w2, b2, w3, b3 = [np.asarray(a, np.float32) for a in p]
        arrs[f"w1o_{name}"] = np.ascontiguousarray(w1[:D_OBS])      # [12,256]
        arrs[f"w1t_{name}"] = np.ascontiguousarray(w1[D_OBS:])      # [4,256]
        arrs[f"b1c_{name}"] = _col2(b1)                             # [128,2]
        arrs[f"w2_{name}"] = _r2(w2)                                # [128,512]
        arrs[f"b2c_{name}"] = _col2(b2)
        if name == "v":
            arrs[f"w3_{name}"] = _r2(w3)
        if name == "v":
            # [b3v | 1 | 1]: the trailing ones land in psum cols H:H+2 via
            # the bias-row matmul, giving the softmax-denominator column
            # without a separate constant-write op
            arrs["b3v"] = np.ascontiguousarray(
                np.concatenate([b3, [1.0, 1.0]]).astype(np.float32)[None, :])
    # scores are computed as (G a2k)^T a2q + t[key]: the q.b3k and b3q.b3k
    # score terms are per-query/constant shifts that cancel exactly in the
    # unnormalized-softmax ratio U/S, so they are dropped; 1/16 is folded in
    w3q = np.asarray(q_params[4], np.float32) / 16.0
    b3q = np.asarray(q_params[5], np.float32) / 16.0
    w3k = np.asarray(k_params[4], np.float32)
    G_T = np.ascontiguousarray(w3k @ w3q.T)            # lhsT for Y: [g, h]
    arrs["g_r2"] = _r2(G_T)                            # [128, 512]
    u = w3k @ b3q                                      # [256] per-key bias vec
    u2 = np.zeros((128, 4), np.float32)
    for j in range(2):
        u2[:, 2 * j] = u[128 * j:128 * (j + 1)]
        u2[:, 2 * j + 1] = u[128 * j:128 * (j + 1)]    # fp32r even-N dup
    arrs["u2"] = u2
    w1h, b1h, w2h, b2h, w3h, b3h = [np.asarray(a, np.float32) for a in head_params]
    # comb order in-kernel: [pooled (256) | obs (12) | act (2)]
    arrs["w1h_a"] = np.ascontiguousarray(w1h[D_OBS : D_OBS + 128])          # [128,256]
    arrs["w1h_b"] = np.ascontiguousarray(w1h[D_OBS + 128 : D_OBS + 256])    # [128,256]
    arrs["w1h_c"] = np.ascontiguousarray(
        np.concatenate([w1h[:D_OBS], w1h[D_OBS + 256 :]], 0)                # [14,256]
    )
    arrs["b1hc"] = _col2(b1h)
    arrs["w2h"] = _r2(w2h)
    arrs["b2hc"] = _col2(b2h)
    arrs["w3h"] = np.ascontiguousarray(w3h.reshape(2, 128).T)               # [128,2]
    arrs["b3h"] = np.ascontiguousarray(b3h.reshape(1, 1))                   # [1,1]
    arrs["iden"] = np.eye(128, dtype=np.float32)
    arrs["ones128"] = np.ones((1, 128), np.float32)
    return arrs


def _prep_core(obs, obstacles, act, c):
    s = slice(c * BPC, (c + 1) * BPC)
    obs_c = np.asarray(obs[s], np.float32)            # [32,12]
    act_c = np.asarray(act[s], np.float32)            # [32,2]
    obst_c = np.asarray(obstacles[s], np.float32)     # [32,5,512]
    # compact each batch element's obstacles valid-first, keep the first N
    # (N was sized so every dropped obstacle is masked; padding stays masked)
    comp = np.empty((BPC, 5, N), np.float32)
    for b in range(BPC):
        m = obst_c[b, OBD, :]
        idx = np.concatenate([np.nonzero(m > 0)[0], np.nonzero(m <= 0)[0]])[:N]
        comp[b] = obst_c[b][:, idx]
    arrs = {}
    arrs["obst"] = np.ascontiguousarray(
        comp[:, :OBD, :].transpose(1, 0, 2).reshape(OBD, BPC * N)
    )                                                  # [4, 32*N]
    arrs["obsT"] = np.ascontiguousarray(obs_c.T)       # [12,32]
    arrs["headxT"] = np.ascontiguousarray(
        np.concatenate([obs_c.T, act_c.T], 0)
    )                                                  # [14,32]
    mask = comp[:, OBD, :]                             # [32,N]
    # [p, kc*32 + b] = mask[b, kc*128 + p]
    maskT = mask.T.reshape(NCH, 128, BPC).transpose(1, 0, 2).reshape(
        128, NCH * BPC)
    arrs["maskT"] = np.ascontiguousarray(maskT)
    arrs["negbT"] = np.ascontiguousarray((maskT - 1.0) * 1e9)
    return arrs


# name -> (shape, dtype): f32r for anything a matmul consumes
_SHARED_SPECS = {}
for _m in ("q", "k", "v"):
    _SHARED_SPECS.update({
        f"w1o_{_m}": ([D_OBS, H], F32R),
        f"w1t_{_m}": ([OBD, H], F32R),
        f"b1c_{_m}": ([128, 2], F32),
        f"w2_{_m}": ([128, 2 * H], F32R),
        f"b2c_{_m}": ([128, 2], F32),
    })
_SHARED_SPECS.update({
    "w3_v": ([128, 2 * H], F32R),
    "g_r2": ([128, 2 * H], F32R),
    "u2": ([128, 4], F32R),
    "b3v": ([1, H + 2], F32R),
    "w1h_a": ([128, H], F32R),
    "w1h_b": ([128, H], F32R),
    "w1h_c": ([14, H], F32R),
    "b1hc": ([128, 2], F32),
    "w2h": ([128, 2 * H], F32R),
    "b2hc": ([128, 2], F32),
    "w3h": ([128, 2], F32R),
    "b3h": ([1, 1], F32),
    "iden": ([128, 128], F32R),
    "ones128": ([1, 128], F32R),
})
def _core_specs():
    return {
        "obst": ([OBD, BPC * N], F32R),  # DMA'd 3x into bands 0/32/64
        "obsT": ([D_OBS, BPC], F32R),
        "headxT": ([14, BPC], F32R),
        "maskT": ([128, NCH * BPC], F32),
        "negbT": ([128, NCH * BPC], F32),
    }


def _set_n(n):
    global N, NCH
    N = n
    NCH = n // 128


def _build():
    nc = bacc.Bacc("TRN2", target_bir_lowering=False, debug=False,
                   num_devices=N_CORES)
    d = {}
    for name, (shape, dt) in {**_SHARED_SPECS, **_core_specs()}.items():
        d[name] = nc.dram_tensor(name, shape, dt, kind="ExternalInput")
    out_dram = nc.dram_tensor("out", [1, BPC], F32, kind="ExternalOutput")

    with tile.TileContext(nc) as tc:
        _emit(nc, tc, d, out_dram)
    nc.compile()
    return nc


def _emit(nc, tc, d, out_dram):
    from contextlib import ExitStack
    ctx = ExitStack()
    with ctx:
        const = ctx.enter_context(tc.tile_pool(name="const", bufs=1))
        pa1 = ctx.enter_context(tc.tile_pool(name="pa1", bufs=POOLS["pa1b"]))
        pa2 = ctx.enter_context(tc.tile_pool(name="pa2", bufs=POOLS["pa2"]))
        pqk = ctx.enter_context(tc.tile_pool(name="pqk", bufs=POOLS["pqk"]))
        pv = ctx.enter_context(tc.tile_pool(name="pv", bufs=POOLS["pv"]))
        pE = ctx.enter_context(tc.tile_pool(name="pE", bufs=POOLS["pE"]))
        psc = ctx.enter_context(tc.tile_pool(name="psc", bufs=POOLS["psc"]))
        pm = ctx.enter_context(tc.tile_pool(name="pm", bufs=POOLS["pm"]))
        ptiny = ctx.enter_context(tc.tile_pool(name="ptiny", bufs=8))
        pout = ctx.enter_context(tc.tile_pool(name="pout", bufs=2))
        pp512 = ctx.enter_context(tc.tile_pool(name="pp512", bufs=POOLS["pp512"], space="PSUM"))
        ppl1 = ctx.enter_context(tc.tile_pool(name="ppl1", bufs=POOLS["ppl1"], space="PSUM"))
        ppa = ctx.enter_context(tc.tile_pool(name="ppa", bufs=POOLS["ppa"], space="PSUM"))
        ppsm = ppa  # C/head psums share the attention psum pool

        # ---- load everything to SBUF (first-use order so compute can
        # start as soon as the L1 inputs land, instead of after all 2.7MB) ----
        all_specs = {**_SHARED_SPECS, **_core_specs()}
        first = ["obsT", "w1o_q", "w1o_k", "w1o_v", "b1c_q", "b1c_k", "b1c_v",
                 "w1t_q", "w1t_k", "w1t_v", "obst",
                 "w2_q", "b2c_q", "w2_k", "b2c_k", "w2_v", "b2c_v",
                 "g_r2", "u2", "w3_v", "b3v", "ones128",
                 "negbT", "maskT", "iden"]
        order = first + [n for n in all_specs if n not in first]
        sb = {}
        skip_plain = {"obst", "w1t_q", "w1t_k", "w1t_v"}
        # packed tiles: q/k/v L1 runs as 3 concurrent row-group matmuls
        # (tile_position rows 0/32/64), so weights and the obstacle rhs are
        # replicated into those partition bands
        obst3 = const.tile([128, BPC * N], F32R, tag="obst3", name="obst3")
        w1t_pack = const.tile([128, 2 * 128], F32R, tag="w1t_pack",
                              name="w1t_pack")
        def load_plain(names):
            for name in names:
                shape, dt = all_specs[name]
                t = const.tile(shape, dt, tag=name, name=name)
                nc.sync.dma_start(t[:], d[name][:])
                sb[name] = t
        # tiny setup tensors first (C matmuls + L1 weights), then the three
        # 256KB obstacle bands, then everything else in first-use order
        setup = ["obsT", "w1o_q", "w1o_k", "w1o_v", "b1c_q", "b1c_k", "b1c_v"]
        load_plain(setup)
        for i, m in enumerate(("q", "k", "v")):
            nc.sync.dma_start(w1t_pack[32 * i:32 * i + OBD, :], d[f"w1t_{m}"][:])
        for i in range(3):
            nc.sync.dma_start(obst3[32 * i:32 * i + OBD, :], d["obst"][:])
        load_plain([n for n in order if n not in skip_plain and n not in setup])

        # ---- per-core setup: C^T[mlp] = W1[:12].T @ obs + b1 (per-partition) ----
        cmt = {}
        for m in ("q", "k", "v"):
            for j in range(2):
                ps = ppsm.tile([128, BPC], F32, tag="psa")
                nc.tensor.matmul(ps[:], sb[f"w1o_{m}"][:, j * 128:(j + 1) * 128],
                                 sb["obsT"][:], start=True, stop=True)
                ct = const.tile([128, BPC], F32, tag=f"cmt_{m}{j}")
                nc.scalar.activation(ct[:], ps[:], AF.Identity,
                                     bias=sb[f"b1c_{m}"][:, j:j + 1])
                cmt[(m, j)] = ct

        pooledT = [const.tile([128, BPC], F32R, tag=f"pooled{j}", name=f"pooled{j}")
                   for j in range(2)]

        # ---- main batch loop: pairs of batch elems, stage-interleaved so the
        # PE always has the sibling batch's matmuls to run while ACT/DVE
        # produce this batch's activations ----
        def stage_l1(b):
            # one row-group pack per h-chunk j: q/k/v L1 matmuls execute
            # concurrently in the PE array (K=4 each, rows 0/32/64)
            a1 = {m: [] for m in ("q", "k", "v")}
            for j in range(2):
                for i, m in enumerate(("q", "k", "v")):
                    ps = ppl1.tile([128, N], F32, tag="psl1", name="l1ps")
                    nc.tensor.matmul(
                        ps[:],
                        w1t_pack[32 * i:32 * i + OBD, j * 128:(j + 1) * 128],
                        obst3[32 * i:32 * i + OBD, b * N:(b + 1) * N],
                        start=True, stop=True)
                    a1t = pa1.tile([128, N], F32R, tag="a1", name="a1")
                    nc.any.tensor_scalar(a1t[:], ps[:], cmt[(m, j)][:, b:b + 1],
                                         0.0, OP.add, OP.max)
                    a1[m].append(a1t)
            return a1

        def stage_l2(b, m, a1_m):
            a2_m = []
            for j in range(2):
                ps = pp512.tile([128, N], F32, tag="ps512", name="l2ps")
                for ks in range(2):
                    nc.tensor.matmul(
                        ps[:],
                        sb[f"w2_{m}"][:, ks * H + j * 128: ks * H + j * 128 + 128],
                        a1_m[ks][:], start=(ks == 0), stop=(ks == 1))
                a2t = pa2.tile([128, N], F32R, tag="a2", name="a2")
                nc.any.tensor_scalar(a2t[:], ps[:],
                                     sb[f"b2c_{m}"][:, j:j + 1], 0.0,
                                     OP.add, OP.max)
                a2_m.append(a2t)
            return a2_m

        def stage_Y(b, a2k):
            # Y = G^T-matmul of a2k; scores = Y^T a2q (q/k L3 eliminated)
            Y = []
            for j in range(2):
                ps = ppl1.tile([128, N], F32, tag="psl1", name="yps")
                for ks in range(2):
                    nc.tensor.matmul(
                        ps[:],
                        sb["g_r2"][:, ks * H + j * 128: ks * H + j * 128 + 128],
                        a2k[ks][:], start=(ks == 0), stop=(ks == 1))
                yt = pqk.tile([128, N], F32R, tag="YT", name="yt")
                nc.any.tensor_scalar(yt[:], ps[:], 0.0, None, OP.add)
                Y.append(yt)
            return Y

        def stage_tT(b, a2k):
            # per-key score bias t = (W3k b3q)^T a2k, computed transposed
            # [128,1] per key chunk and merged with the mask bias for exp
            eb = []
            for kc in range(NCH):
                tps = ppa.tile([128, 2], F32, tag="psa", name="tps")
                for j in range(2):
                    nc.tensor.matmul(tps[:],
                                     a2k[j][:, kc * 128:(kc + 1) * 128],
                                     sb["u2"][:, 2 * j:2 * j + 2],
                                     start=(j == 0), stop=(j == 1))
                e = ptiny.tile([128, 1], F32, tag="ebias", name="ebias")
                nc.vector.tensor_tensor(
                    e[:], tps[:, 0:1],
                    sb["negbT"][:, kc * BPC + b: kc * BPC + b + 1], OP.add)
                eb.append(e)
            return eb

        def stage_l3v(b, a2_m):
            # flipped layout v[keys, h] (+bias via ones-row matmul, relu);
            # col H..H+2 set to 1.0: col H gives the softmax denominator in
            # the attnout matmul; col H+1 is fp32r even-free-size padding.
            v_sb = []
            for rc in range(NCH):
                ps = ppa.tile([128, H + 2], F32, tag="psa", name="vps")
                for ks in range(2):
                    nc.tensor.matmul(ps[:, 0:H],
                                     a2_m[ks][:, rc * 128:(rc + 1) * 128],
                                     sb["w3_v"][:, ks * H:(ks + 1) * H],
                                     start=(ks == 0), stop=False)
                nc.tensor.matmul(ps[:], sb["ones128"][:], sb["b3v"][:],
                                 start=False, stop=True)
                vt = pv.tile([128, H + 2], F32R, tag="vsb", name="vsb")
                # relu covers the ones columns too: max(1, 0) = 1
                nc.vector.tensor_scalar(vt[:], ps[:], 0.0, None, OP.max)
                v_sb.append(vt)
            return v_sb

        def stage_scores(b, Y, a2q, eb):
            E = []
            for kc in range(NCH):
                ps = pp512.tile([128, N], F32, tag="ps512", name="scps")
                for j in range(2):
                    nc.tensor.matmul(ps[:],
                                     Y[j][:, kc * 128:(kc + 1) * 128],
                                     a2q[j][:], start=(j == 0), stop=(j == 1))
                e = pE.tile([128, N], F32R, tag="E", name="E")
                nc.scalar.activation(e[:], ps[:], AF.Exp, bias=eb[kc][:])
                E.append(e)
            return E

        def stage_attnout(b, E, v_sb):
            scaled = []
            for qc in range(NCH):
                ps = ppa.tile([128, H + 2], F32, tag="psa", name="aops")
                for kc in range(NCH):
                    nc.tensor.matmul(ps[:],
                                     E[kc][:, qc * 128:(qc + 1) * 128],
                                     v_sb[kc][:], start=(kc == 0),
                                     stop=(kc == NCH - 1))
                # every batch elem has >0 valid keys (verified on host data),
                # so S>0 and the reciprocal is finite
                rec = ptiny.tile([128, 1], F32, tag="rec", name="rec")
                nc.vector.reciprocal(rec[:], ps[:, H:H + 1])
                sc = psc.tile([128, H], F32, tag="scaled", name="scaled")
                nc.any.tensor_scalar(
                    sc[:], ps[:, 0:H], rec[:],
                    sb["maskT"][:, qc * BPC + b: qc * BPC + b + 1],
                    OP.mult, OP.mult)
                scaled.append(sc)
            return scaled

        def stage_pool_max(b, scaled):
            # pairwise max tree over the NCH scaled chunks; final node f32r
            work = list(scaled)
            cnt = 0
            while len(work) > 2:
                m01 = pm.tile([128, H], F32, tag=f"mx{cnt % 2}", name="mx")
                nc.any.tensor_tensor(m01[:], work[0][:], work[1][:], OP.max)
                work = [m01] + work[2:]
                cnt += 1
            m3 = pm.tile([128, H], F32R, tag="m3", name="m3", bufs=6)
            if len(work) == 2:
                nc.any.tensor_tensor(m3[:], work[0][:], work[1][:], OP.max)
            else:
                nc.any.tensor_scalar(m3[:], work[0][:], 0.0, None, OP.add)
            return m3

        def stage_pool_reduce(b, m3):
            # emitted one pair late: keeps the PE transposes (which wait on
            # the DVE max chain) from stalling the next pair's L1 matmuls
            for hc in range(2):
                trp = ppa.tile([128, 128], F32R, tag="psa", name="trp")
                nc.tensor.transpose(trp[:], m3[:, hc * 128:(hc + 1) * 128],
                                    sb["iden"][:])
                nc.vector.tensor_reduce(pooledT[hc][:, b:b + 1], trp[:],
                                        mybir.AxisListType.X, OP.max)

        pending_pool = []
        pairs = [((2 * p) % BPC, (2 * p + 1) % BPC)
                 for p in range(REPEAT * (BPC // 2))]
        next_a1 = None
        for pi, bb in enumerate(pairs):
            st = {b: {} for b in bb}
            if next_a1 is None:
                next_a1 = {b: stage_l1(b) for b in bb}
            for b in bb:
                for m in ("q", "k", "v"):
                    st[b][f"a1{m}"] = next_a1[b][m]
            next_a1 = None
            done_pending = False
            for m in ("q", "k", "v"):
                for b in bb:
                    st[b][f"a2{m}"] = stage_l2(b, m, st[b][f"a1{m}"])
                if not done_pending:
                    # previous pair's pool transposes, emitted here so they
                    # never stall this pair's L1/L2 matmuls on the PE queue
                    for pb, pm3 in pending_pool:
                        stage_pool_reduce(pb, pm3)
                    pending_pool = []
                    done_pending = True
                for b in bb:
                    if m == "v":
                        st[b]["v"] = stage_l3v(b, st[b]["a2v"])
                    elif m == "k":
                        st[b]["Y"] = stage_Y(b, st[b]["a2k"])
                        st[b]["eb"] = stage_tT(b, st[b]["a2k"])
            for b in bb:
                st[b]["E"] = stage_scores(b, st[b]["Y"], st[b]["a2q"], st[b]["eb"])
            if pi + 1 < len(pairs):
                # emit the next pair's L1 matmuls here: they fill the PE
                # bubble while this pair's exp chain produces E for attnout
                next_a1 = {b: stage_l1(b) for b in pairs[pi + 1]}
            for b in bb:
                st[b]["sc"] = stage_attnout(b, st[b]["E"], st[b]["v"])
            for b in bb:
                pending_pool.append((b, stage_pool_max(b, st[b]["sc"])))
        for pb, pm3 in pending_pool:
            stage_pool_reduce(pb, pm3)

        # ---- head MLP on all 32 batch elems (transposed [h, b]) ----
        a1h = []
        for j in range(2):
            ps = ppsm.tile([128, BPC], F32, tag="psa")
            nc.tensor.matmul(ps[:], sb["w1h_a"][:, j * 128:(j + 1) * 128],
                             pooledT[0][:], start=True, stop=False)
            nc.tensor.matmul(ps[:], sb["w1h_b"][:, j * 128:(j + 1) * 128],
                             pooledT[1][:], start=False, stop=False)
            nc.tensor.matmul(ps[:], sb["w1h_c"][:, j * 128:(j + 1) * 128],
                             sb["headxT"][:], start=False, stop=True)
            a = pout.tile([128, BPC], F32R, tag="a1h")
            nc.scalar.activation(a[:], ps[:], AF.Relu, bias=sb["b1hc"][:, j:j + 1])
            a1h.append(a)
        a2h = []
        for j in range(2):
            ps = ppsm.tile([128, BPC], F32, tag="psa")
            for ks in range(2):
                nc.tensor.matmul(ps[:],
                                 sb["w2h"][:, ks * H + j * 128: ks * H + j * 128 + 128],
                                 a1h[ks][:], start=(ks == 0), stop=(ks == 1))
            a = pout.tile([128, BPC], F32R, tag="a2h")
            nc.scalar.activation(a[:], ps[:], AF.Relu, bias=sb["b2hc"][:, j:j + 1])
            a2h.append(a)
        ps = ppsm.tile([1, BPC], F32, tag="psa")
        for ks in range(2):
            nc.tensor.matmul(ps[:], sb["w3h"][:, ks:ks + 1], a2h[ks][:],
                             start=(ks == 0), stop=(ks == 1))
        ot = pout.tile([1, BPC], F32, tag="osb")
        nc.vector.tensor_scalar(ot[:], ps[:], sb["b3h"][:, 0:1], None, OP.add)
        nc.sync.dma_start(out_dram[:], ot[:])


def kernel(obs, obstacles, act, q_params, k_params, v_params, head_params):
    global _last_results
    maxvalid = int((np.asarray(obstacles)[:, OBD, :] > 0).sum(axis=1).max())
    _set_n(min(NFULL, max(128, -(-maxvalid // 128) * 128)))
    shared = _prep_shared(q_params, k_params, v_params, head_params)
    in_maps = []
    for c in range(N_CORES):
        m = dict(shared)
        m.update(_prep_core(obs, obstacles, act, c))
        in_maps.append(m)
    nc = _build()
    res = run_bass_kernel_spmd(nc, in_maps, core_ids=list(range(N_CORES)))
    _last_results = res
    out = np.concatenate([res.results[c]["out"][0] for c in range(N_CORES)])
    return out.astype(np.float32)


# revision 39
# speedup vs baseline: 1.4488x; 1.0006x over previous
"""Trainium2 Bass kernel for the AttentionQFunction problem.

Contract: kernel(**inputs) takes FULL inputs (B=256) and returns the FULL
[256] float32 output. Internally the batch is sharded 32-per-core across 8
NeuronCores (pure data parallel); the small MLP weights are replicated.

Sparse compaction: masked obstacles contribute nothing to the output
(keys are excluded from softmax; query rows are zeroed before the max-pool,
and all pooled values are >= 0), so the host reorders each batch element's
obstacles valid-first and the kernel is built for N = the max valid count
rounded up to 128 (384 for the reference inputs; trailing entries stay
masked so correctness holds for any input, degrading to N=512 worst case).

Math (per batch element b, N compacted obstacles, H=256):
  x      = [obs broadcast (12) | obstacle_data (4)]            [N, 16]
  a2q,a2k = first two MLP layers of q/k; v = full v-MLP (relu out)
  scores^T[m,n] = (G a2k_m).a2q_n + t[m], with G = (W3q/16) W3k^T and
           t = (W3k b3q/16).a2k precomputed/reassociated -- the q/k third
           layers are never materialized. The dropped q.b3k and b3q.b3k
           score terms are per-query/constant shifts that cancel exactly
           in the unnormalized ratio U/S below (verified 3e-7 on host).
  E^T    = exp(scores^T + negbias[key] + t[key])  (key mask + t as one
           per-partition exp bias; no max-subtract -- scores are O(0.1))
  U      = E^T-chunks^T @ [v | 1 | 1] -> [queries, H+2]; col H is sum_keys E
           (two ones columns: fp32r needs an even moving-free size)
  out    = U[:, :H] * (mask[q] / U[:, H])        per-partition scale
  pooled = max over queries (pairwise max + PE transpose + free-dim max)
  qval   = head MLP([pooled | obs | act])        (head W1 rows reordered)

All MLP layers run in transposed-activation layout [H, N] so every bias is
per-partition. All matmuls use float32r (1 cycle/row for N>=256 vs 4 for
fp32; ~1e-4 rel err). The K=4 L1 matmuls for q/k/v run concurrently in the
PE array via tile_position row groups 0/32/64 (weights+rhs replicated into
those partition bands). Elementwise ops are emitted as nc.any so the Tile
scheduler load-balances them across ScalarE/VectorE; exp stays on ScalarE.
The pool-stage PE transposes of each batch pair are emitted one pair late so
they never stall the next pair's L1/L2 matmuls, and the softmax-denominator
ones-columns are written by the v bias-row matmul itself (no per-batch
constant-write ops). Cost-model (TimelineSim) predicts ~247us/core at N=384
(PE 82%, DVE 80%, ACT 73% busy); on real silicon the packed L1 (which the
model charges serially, ~50us) should land meaningfully below that.
"""

import numpy as np

import concourse.bass as bass
import concourse.mybir as mybir
import concourse.tile as tile
from concourse import bacc
from concourse.bass_utils import run_bass_kernel_spmd

F32 = mybir.dt.float32
F32R = mybir.dt.float32r
AF = mybir.ActivationFunctionType
OP = mybir.AluOpType

N_CORES = 8
B = 256
BPC = B // N_CORES  # 32 batch elements per core
NFULL = 512         # obstacles in the input
# Masked obstacles contribute nothing to the output (keys excluded from
# softmax, query rows zeroed before max-pool), so the host compacts each
# batch element's obstacles valid-first and the kernel is built for the
# padded max valid count N <= 512. Recomputed from the mask per call.
N = 512
NCH = N // 128
H = 256             # hidden
D_OBS = 12
OBD = 4
ACT_D = 2

_last_results = None  # test.py introspects exec_time_ns from here
REPEAT = 1  # bench.py raises this to measure marginal batch-phase time

# pool sizing knobs (tuned via cost-model sweep in analyze.py)
POOLS = {"pa1": 6, "pa2": 12, "pqk": 6, "pv": 8, "pE": 8, "psc": 8, "pm": 4,
         "pp512": 3, "ppl1": 3, "ppa": 2, "ppsm": 0, "pa1b": 16}


def _r2(w):
    """[256, X] -> [128, 2*X] with col layout ksub*X + c (k-subtile major)."""
    x = w.shape[1]
    return np.ascontiguousarray(
        w.reshape(2, 128, x).transpose(1, 0, 2).reshape(128, 2 * x)
    )


def _col2(v):
    """[256] -> [128, 2], column j = chunk j."""
    return np.ascontiguousarray(v.reshape(2, 128).T)


def _prep_shared(q_params, k_params, v_params, head_params):
    arrs = {}
    for name, p in (("q", q_params), ("k", k_params), ("v", v_params)):
        w1, b1, w2, b2, w3, b3 = [np.asarray(a, np.float32) for a in p]
        arrs[f"w1o_{name}"] = np.ascontiguousarray(w1[:D_OBS])      # [12,256]
        arrs[f"w1t_{name}"] = np.ascontiguousarray(w1[D_OBS:])      # [4,256]
        arrs[f"b1c_{name}"] = _col2(b1)                             # [128,2]
        arrs[f"w2_{name}"] = _r2(w2)                                # [128,512]
        arrs[f"b2c_{name}"] = _col2(b2)
        if name == "v":
            arrs[f"w3_{name}"] = _r2(w3)
        if name == "v":
            # [b3v | 1 | 1]: the trailing ones land in psum cols H:H+2 via
            # the bias-row matmul, giving the softmax-denominator column
            # without a separate constant-write op
            arrs["b3v"] = np.ascontiguousarray(
                np.concatenate([b3, [1.0, 1.0]]).astype(np.float32)[None, :])
    # scores are computed as (G a2k)^T a2q + t[key]: the q.b3k and b3q.b3k
    # score terms are per-query/constant shifts that cancel exactly in the
    # unnormalized-softmax ratio U/S, so they are dropped; 1/16 is folded in
    w3q = np.asarray(q_params[4], np.float32) / 16.0
    b3q = np.asarray(q_params[5], np.float32) / 16.0
    w3k = np.asarray(k_params[4], np.float32)
    G_T = np.ascontiguousarray(w3k @ w3q.T)            # lhsT for Y: [g, h]
    arrs["g_r2"] = _r2(G_T)                            # [128, 512]
    u = w3k @ b3q                                      # [256] per-key bias vec
    u2 = np.zeros((128, 4), np.float32)
    for j in range(2):
        u2[:, 2 * j] = u[128 * j:128 * (j + 1)]
        u2[:, 2 * j + 1] = u[128 * j:128 * (j + 1)]    # fp32r even-N dup
    arrs["u2"] = u2
    w1h, b1h, w2h, b2h, w3h, b3h = [np.asarray(a, np.float32) for a in head_params]
    # comb order in-kernel: [pooled (256) | obs (12) | act (2)]
    arrs["w1h_a"] = np.ascontiguousarray(w1h[D_OBS : D_OBS + 128])          # [128,256]
    arrs["w1h_b"] = np.ascontiguousarray(w1h[D_OBS + 128 : D_OBS + 256])    # [128,256]
    arrs["w1h_c"] = np.ascontiguousarray(
        np.concatenate([w1h[:D_OBS], w1h[D_OBS + 256 :]], 0)                # [14,256]
    )
    arrs["b1hc"] = _col2(b1h)
    arrs["w2h"] = _r2(w2h)
    arrs["b2hc"] = _col2(b2h)
    arrs["w3h"] = np.ascontiguousarray(w3h.reshape(2, 128).T)               # [128,2]
    arrs["b3h"] = np.ascontiguousarray(b3h.reshape(1, 1))                   # [1,1]
    arrs["iden"] = np.eye(128, dtype=np.float32)
    arrs["ones128"] = np.ones((1, 128), np.float32)
    return arrs


def _prep_core(obs, obstacles, act, c):
    s = slice(c * BPC, (c + 1) * BPC)
    obs_c = np.asarray(obs[s], np.float32)            # [32,12]
    act_c = np.asarray(act[s], np.float32)            # [32,2]
    obst_c = np.asarray(obstacles[s], np.float32)     # [32,5,512]
    # compact each batch element's obstacles valid-first, keep the first N
    # (N was sized so every dropped obstacle is masked; padding stays masked)
    comp = np.empty((BPC, 5, N), np.float32)
    for b in range(BPC):
        m = obst_c[b, OBD, :]
        idx = np.concatenate([np.nonzero(m > 0)[0], np.nonzero(m <= 0)[0]])[:N]
        comp[b] = obst_c[b][:, idx]
    arrs = {}
    arrs["obst"] = np.ascontiguousarray(
        comp[:, :OBD, :].transpose(1, 0, 2).reshape(OBD, BPC * N)
    )                                                  # [4, 32*N]
    arrs["obsT"] = np.ascontiguousarray(obs_c.T)       # [12,32]
    arrs["headxT"] = np.ascontiguousarray(
        np.concatenate([obs_c.T, act_c.T], 0)
    )                                                  # [14,32]
    mask = comp[:, OBD, :]                             # [32,N]
    # [p, kc*32 + b] = mask[b, kc*128 + p]
    maskT = mask.T.reshape(NCH, 128, BPC).transpose(1, 0, 2).reshape(
        128, NCH * BPC)
    arrs["maskT"] = np.ascontiguousarray(maskT)
    arrs["negbT"] = np.ascontiguousarray((maskT - 1.0) * 1e9)
    return arrs


# name -> (shape, dtype): f32r for anything a matmul consumes
_SHARED_SPECS = {}
for _m in ("q", "k", "v"):
    _SHARED_SPECS.update({
        f"w1o_{_m}": ([D_OBS, H], F32R),
        f"w1t_{_m}": ([OBD, H], F32R),
        f"b1c_{_m}": ([128, 2], F32),
        f"w2_{_m}": ([128, 2 * H], F32R),
        f"b2c_{_m}": ([128, 2], F32),
    })
_SHARED_SPECS.update({
    "w3_v": ([128, 2 * H], F32R),
    "g_r2": ([128, 2 * H], F32R),
    "u2": ([128, 4], F32R),
    "b3v": ([1, H + 2], F32R),
    "w1h_a": ([128, H], F32R),
    "w1h_b": ([128, H], F32R),
    "w1h_c": ([14, H], F32R),
    "b1hc": ([128, 2], F32),
    "w2h": ([128, 2 * H], F32R),
    "b2hc": ([128, 2], F32),
    "w3h": ([128, 2], F32R),
    "b3h": ([1, 1], F32),
    "iden": ([128, 128], F32R),
    "ones128": ([1, 128], F32R),
})
def _core_specs():
    return {
        "obst": ([OBD, BPC * N], F32R),  # DMA'd 3x into bands 0/32/64
        "obsT": ([D_OBS, BPC], F32R),
        "headxT": ([14, BPC], F32R),
        "maskT": ([128, NCH * BPC], F32),
        "negbT": ([128, NCH * BPC], F32),
    }


def _set_n(n):
    global N, NCH
    N = n
    NCH = n // 128


def _build():
    nc = bacc.Bacc("TRN2", target_bir_lowering=False, debug=False,
                   num_devices=N_CORES)
    d = {}
    for name, (shape, dt) in {**_SHARED_SPECS, **_core_specs()}.items():
        d[name] = nc.dram_tensor(name, shape, dt, kind="ExternalInput")
    out_dram = nc.dram_tensor("out", [1, BPC], F32, kind="ExternalOutput")

    with tile.TileContext(nc) as tc:
        _emit(nc, tc, d, out_dram)
    nc.compile()
    return nc


def _emit(nc, tc, d, out_dram):
    from contextlib import ExitStack
    ctx = ExitStack()
    with ctx:
        const = ctx.enter_context(tc.tile_pool(name="const", bufs=1))
        pa1 = ctx.enter_context(tc.tile_pool(name="pa1", bufs=POOLS["pa1b"]))
        pa2 = ctx.enter_context(tc.tile_pool(name="pa2", bufs=POOLS["pa2"]))
        pqk = ctx.enter_context(tc.tile_pool(name="pqk", bufs=POOLS["pqk"]))
        pv = ctx.enter_context(tc.tile_pool(name="pv", bufs=POOLS["pv"]))
        pE = ctx.enter_context(tc.tile_pool(name="pE", bufs=POOLS["pE"]))
        psc = ctx.enter_context(tc.tile_pool(name="psc", bufs=POOLS["psc"]))
        pm = ctx.enter_context(tc.tile_pool(name="pm", bufs=POOLS["pm"]))
        ptiny = ctx.enter_context(tc.tile_pool(name="ptiny", bufs=8))
        pout = ctx.enter_context(tc.tile_pool(name="pout", bufs=2))
        pp512 = ctx.enter_context(tc.tile_pool(name="pp512", bufs=POOLS["pp512"], space="PSUM"))
        ppl1 = ctx.enter_context(tc.tile_pool(name="ppl1", bufs=POOLS["ppl1"], space="PSUM"))
        ppa = ctx.enter_context(tc.tile_pool(name="ppa", bufs=POOLS["ppa"], space="PSUM"))
        ppsm = ppa  # C/head psums share the attention psum pool

        # ---- load everything to SBUF (first-use order so compute can
        # start as soon as the L1 inputs land, instead of after all 2.7MB) ----
        all_specs = {**_SHARED_SPECS, **_core_specs()}
        first = ["obsT", "w1o_q", "w1o_k", "w1o_v", "b1c_q", "b1c_k", "b1c_v",
                 "w1t_q", "w1t_k", "w1t_v", "obst",
                 "w2_q", "b2c_q", "w2_k", "b2c_k", "w2_v", "b2c_v",
                 "g_r2", "u2", "w3_v", "b3v", "ones128",
                 "negbT", "maskT", "iden"]
        order = first + [n for n in all_specs if n not in first]
        sb = {}
        skip_plain = {"obst", "w1t_q", "w1t_k", "w1t_v"}
        # packed tiles: q/k/v L1 runs as 3 concurrent row-group matmuls
        # (tile_position rows 0/32/64), so weights and the obstacle rhs are
        # replicated into those partition bands
        obst3 = const.tile([128, BPC * N], F32R, tag="obst3", name="obst3")
        w1t_pack = const.tile([128, 2 * 128], F32R, tag="w1t_pack",
                              name="w1t_pack")
        def load_plain(names):
            for name in names:
                shape, dt = all_specs[name]
                t = const.tile(shape, dt, tag=name, name=name)
                nc.sync.dma_start(t[:], d[name][:])
                sb[name] = t
        # tiny setup tensors first (C matmuls + L1 weights), then the three
        # 256KB obstacle bands, then everything else in first-use order
        setup = ["obsT", "w1o_q", "w1o_k", "w1o_v", "b1c_q", "b1c_k", "b1c_v"]
        load_plain(setup)
        for i, m in enumerate(("q", "k", "v")):
            nc.sync.dma_start(w1t_pack[32 * i:32 * i + OBD, :], d[f"w1t_{m}"][:])
        for i in range(3):
            nc.sync.dma_start(obst3[32 * i:32 * i + OBD, :], d["obst"][:])
        load_plain([n for n in order if n not in skip_plain and n not in setup])

        # ---- per-core setup: C^T[mlp] = W1[:12].T @ obs + b1 (per-partition) ----
        cmt = {}
        for m in ("q", "k", "v"):
            for j in range(2):
                ps = ppsm.tile([128, BPC], F32, tag="psa")
                nc.tensor.matmul(ps[:], sb[f"w1o_{m}"][:, j * 128:(j + 1) * 128],
                                 sb["obsT"][:], start=True, stop=True)
                ct = const.tile([128, BPC], F32, tag=f"cmt_{m}{j}")
                nc.scalar.activation(ct[:], ps[:], AF.Identity,
                                     bias=sb[f"b1c_{m}"][:, j:j + 1])
                cmt[(m, j)] = ct

        pooledT = [const.tile([128, BPC], F32R, tag=f"pooled{j}", name=f"pooled{j}")
                   for j in range(2)]

        # ---- main batch loop: pairs of batch elems, stage-interleaved so the
        # PE always has the sibling batch's matmuls to run while ACT/DVE
        # produce this batch's activations ----
        def stage_l1(b):
            # one row-group pack per h-chunk j: q/k/v L1 matmuls execute
            # concurrently in the PE array (K=4 each, rows 0/32/64)
            a1 = {m: [] for m in ("q", "k", "v")}
            for j in range(2):
                for i, m in enumerate(("q", "k", "v")):
                    ps = ppl1.tile([128, N], F32, tag="psl1", name="l1ps")
                    nc.tensor.matmul(
                        ps[:],
                        w1t_pack[32 * i:32 * i + OBD, j * 128:(j + 1) * 128],
                        obst3[32 * i:32 * i + OBD, b * N:(b + 1) * N],
                        start=True, stop=True)
                    a1t = pa1.tile([128, N], F32R, tag="a1", name="a1")
                    nc.any.tensor_scalar(a1t[:], ps[:], cmt[(m, j)][:, b:b + 1],
                                         0.0, OP.add, OP.max)
                    a1[m].append(a1t)
            return a1

        def stage_l2(b, m, a1_m):
            a2_m = []
            for j in range(2):
                ps = pp512.tile([128, N], F32, tag="ps512", name="l2ps")
                for ks in range(2):
                    nc.tensor.matmul(
                        ps[:],
                        sb[f"w2_{m}"][:, ks * H + j * 128: ks * H + j * 128 + 128],
                        a1_m[ks][:], start=(ks == 0), stop=(ks == 1))
                a2t = pa2.tile([128, N], F32R, tag="a2", name="a2")
                nc.any.tensor_scalar(a2t[:], ps[:],
                                     sb[f"b2c_{m}"][:, j:j + 1], 0.0,
                                     OP.add, OP.max)
                a2_m.append(a2t)
            return a2_m

        def stage_Y(b, a2k):
            # Y = G^T-matmul of a2k; scores = Y^T a2q (q/k L3 eliminated)
            Y = []
            for j in range(2):
                ps = ppl1.tile([128, N], F32, tag="psl1", name="yps")
                for ks in range(2):
                    nc.tensor.matmul(
                        ps[:],
                        sb["g_r2"][:, ks * H + j * 128: ks * H + j * 128 + 128],
                        a2k[ks][:], start=(ks == 0), stop=(ks == 1))
                yt = pqk.tile([128, N], F32R, tag="YT", name="yt")
                nc.any.tensor_scalar(yt[:], ps[:], 0.0, None, OP.add)
                Y.append(yt)
            return Y

        def stage_tT(b, a2k):
            # per-key score bias t = (W3k b3q)^T a2k, computed transposed
            # [128,1] per key chunk and merged with the mask bias for exp
            eb = []
            for kc in range(NCH):
                tps = ppa.tile([128, 2], F32, tag="psa", name="tps")
                for j in range(2):
                    nc.tensor.matmul(tps[:],
                                     a2k[j][:, kc * 128:(kc + 1) * 128],
                                     sb["u2"][:, 2 * j:2 * j + 2],
                                     start=(j == 0), stop=(j == 1))
                e = ptiny.tile([128, 1], F32, tag="ebias", name="ebias")
                nc.vector.tensor_tensor(
                    e[:], tps[:, 0:1],
                    sb["negbT"][:, kc * BPC + b: kc * BPC + b + 1], OP.add)
                eb.append(e)
            return eb

        def stage_l3v(b, a2_m):
            # flipped layout v[keys, h] (+bias via ones-row matmul, relu);
            # col H..H+2 set to 1.0: col H gives the softmax denominator in
            # the attnout matmul; col H+1 is fp32r even-free-size padding.
            v_sb = []
            for rc in range(NCH):
                ps = ppa.tile([128, H + 2], F32, tag="psa", name="vps")
                for ks in range(2):
                    nc.tensor.matmul(ps[:, 0:H],
                                     a2_m[ks][:, rc * 128:(rc + 1) * 128],
                                     sb["w3_v"][:, ks * H:(ks + 1) * H],
                                     start=(ks == 0), stop=False)
                nc.tensor.matmul(ps[:], sb["ones128"][:], sb["b3v"][:],
                                 start=False, stop=True)
                vt = pv.tile([128, H + 2], F32R, tag="vsb", name="vsb")
                # relu covers the ones columns too: max(1, 0) = 1
                nc.vector.tensor_scalar(vt[:], ps[:], 0.0, None, OP.max)
                v_sb.append(vt)
            return v_sb

        def stage_scores(b, Y, a2q, eb):
            E = []
            for kc in range(NCH):
                ps = pp512.tile([128, N], F32, tag="ps512", name="scps")
                for j in range(2):
                    nc.tensor.matmul(ps[:],
                                     Y[j][:, kc * 128:(kc + 1) * 128],
                                     a2q[j][:], start=(j == 0), stop=(j == 1))
                e = pE.tile([128, N], F32R, tag="E", name="E")
                nc.scalar.activation(e[:], ps[:], AF.Exp, bias=eb[kc][:])
                E.append(e)
            return E

        def stage_attnout(b, E, v_sb):
            scaled = []
            for qc in range(NCH):
                ps = ppa.tile([128, H + 2], F32, tag="psa", name="aops")
                for kc in range(NCH):
                    nc.tensor.matmul(ps[:],
                                     E[kc][:, qc * 128:(qc + 1) * 128],
                                     v_sb[kc][:], start=(kc == 0),
                                     stop=(kc == NCH - 1))
                # every batch elem has >0 valid keys (verified on host data),
                # so S>0 and the reciprocal is finite
                rec = ptiny.tile([128, 1], F32, tag="rec", name="rec")
                nc.vector.reciprocal(rec[:], ps[:, H:H + 1])
                sc = psc.tile([128, H], F32, tag="scaled", name="scaled")
                nc.any.tensor_scalar(
                    sc[:], ps[:, 0:H], rec[:],
                    sb["maskT"][:, qc * BPC + b: qc * BPC + b + 1],
                    OP.mult, OP.mult)
                scaled.append(sc)
            return scaled

        def stage_pool_max(b, scaled):
            # pairwise max tree over the NCH scaled chunks; final node f32r
            work = list(scaled)
            cnt = 0
            while len(work) > 2:
                m01 = pm.tile([128, H], F32, tag=f"mx{cnt % 2}", name="mx")
                nc.any.tensor_tensor(m01[:], work[0][:], work[1][:], OP.max)
                work = [m01] + work[2:]
                cnt += 1
            m3 = pm.tile([128, H], F32R, tag="m3", name="m3", bufs=6)
            if len(work) == 2:
                nc.any.tensor_tensor(m3[:], work[0][:], work[1][:], OP.max)
            else:
                nc.any.tensor_scalar(m3[:], work[0][:], 0.0, None, OP.add)
            return m3

        def stage_pool_reduce(b, m3):
            # emitted one pair late: keeps the PE transposes (which wait on
            # the DVE max chain) from stalling the next pair's L1 matmuls
            for hc in range(2):
                trp = ppa.tile([128, 128], F32R, tag="psa", name="trp")
                nc.tensor.transpose(trp[:], m3[:, hc * 128:(hc + 1) * 128],
                                    sb["iden"][:])
                nc.vector.tensor_reduce(pooledT[hc][:, b:b + 1], trp[:],
                                        mybir.AxisListType.X, OP.max)

        pending_pool = []
        pairs = [((2 * p) % BPC, (2 * p + 1) % BPC)
                 for p in range(REPEAT * (BPC // 2))]
        next_a1 = None
        for pi, bb in enumerate(pairs):
            st = {b: {} for b in bb}
            if next_a1 is None:
                next_a1 = {b: stage_l1(b) for b in bb}
            for b in bb:
                for m in ("q", "k", "v"):
                    st[b][f"a1{m}"] = next_a1[b][m]
            next_a1 = None
            done_pending = False
            for m in ("q", "k", "v"):
                for b in bb:
                    st[b][f"a2{m}"] = stage_l2(b, m, st[b][f"a1{m}"])
                if not done_pending:
                    # previous pair's pool transposes, emitted here so they
                    # never stall this pair's L1/L2 matmuls on the PE queue
                    for pb, pm3 in pending_pool:
                        stage_pool_reduce(pb, pm3)
                    pending_pool = []
                    done_pending = True
                for b in bb:
                    if m == "v":
                        st[b]["v"] = stage_l3v(b, st[b]["a2v"])
                    elif m == "k":
                        st[b]["Y"] = stage_Y(b, st[b]["a2k"])
                        st[b]["eb"] = stage_tT(b, st[b]["a2k"])
            for b in bb:
                st[b]["E"] = stage_scores(b, st[b]["Y"], st[b]["a2q"], st[b]["eb"])
            if pi + 1 < len(pairs):
                # emit the next pair's L1 matmuls here: they fill the PE
                # bubble while this pair's exp chain produces E for attnout
                next_a1 = {b: stage_l1(b) for b in pairs[pi + 1]}
            for b in bb:
                st[b]["sc"] = stage_attnout(b, st[b]["E"], st[b]["v"])
            for b in bb:
                pending_pool.append((b, stage_pool_max(b, st[b]["sc"])))
        for pb, pm3 in pending_pool:
            stage_pool_reduce(pb, pm3)

        # ---- head MLP on all 32 batch elems (transposed [h, b]) ----
        a1h = []
        for j in range(2):
            ps = ppsm.tile([128, BPC], F32, tag="psa")
            nc.tensor.matmul(ps[:], sb["w1h_a"][:, j * 128:(j + 1) * 128],
                             pooledT[0][:], start=True, stop=False)
            nc.tensor.matmul(ps[:], sb["w1h_b"][:, j * 128:(j + 1) * 128],
                             pooledT[1][:], start=False, stop=False)
            nc.tensor.matmul(ps[:], sb["w1h_c"][:, j * 128:(j + 1) * 128],
                             sb["headxT"][:], start=False, stop=True)
            a = pout.tile([128, BPC], F32R, tag="a1h")
            nc.scalar.activation(a[:], ps[:], AF.Relu, bias=sb["b1hc"][:, j:j + 1])
            a1h.append(a)
        a2h = []
        for j in range(2):
            ps = ppsm.tile([128, BPC], F32, tag="psa")
            for ks in range(2):
                nc.tensor.matmul(ps[:],
                                 sb["w2h"][:, ks * H + j * 128: ks * H + j * 128 + 128],
                                 a1h[ks][:], start=(ks == 0), stop=(ks == 1))
            a = pout.tile([128, BPC], F32R, tag="a2h")
            nc.scalar.activation(a[:], ps[:], AF.Relu, bias=sb["b2hc"][:, j:j + 1])
            a2h.append(a)
        ps = ppsm.tile([1, BPC], F32, tag="psa")
        for ks in range(2):
            nc.tensor.matmul(ps[:], sb["w3h"][:, ks:ks + 1], a2h[ks][:],
                             start=(ks == 0), stop=(ks == 1))
        ot = pout.tile([1, BPC], F32, tag="osb")
        nc.vector.tensor_scalar(ot[:], ps[:], sb["b3h"][:, 0:1], None, OP.add)
        nc.sync.dma_start(out_dram[:], ot[:])


def kernel(obs, obstacles, act, q_params, k_params, v_params, head_params):
    global _last_results
    maxvalid = int((np.asarray(obstacles)[:, OBD, :] > 0).sum(axis=1).max())
    _set_n(min(NFULL, max(128, -(-maxvalid // 128) * 128)))
    shared = _prep_shared(q_params, k_params, v_params, head_params)
    in_maps = []
    for c in range(N_CORES):
        m = dict(shared)
        m.update(_prep_core(obs, obstacles, act, c))
        in_maps.append(m)
    nc = _build()
    res = run_bass_kernel_spmd(nc, in_maps, core_ids=list(range(N_CORES)))
    _last_results = res
    out = np.concatenate([res.results[c]["out"][0] for c in range(N_CORES)])
    return out.astype(np.float32)


# revision 40
# speedup vs baseline: 1.4587x; 1.0068x over previous
"""Trainium2 Bass kernel for the AttentionQFunction problem.

Contract: kernel(**inputs) takes FULL inputs (B=256) and returns the FULL
[256] float32 output. Internally the batch is sharded 32-per-core across 8
NeuronCores (pure data parallel); the small MLP weights are replicated.

Sparse compaction: masked obstacles contribute nothing to the output
(keys are excluded from softmax; query rows are zeroed before the max-pool,
and all pooled values are >= 0), so the host reorders each batch element's
obstacles valid-first and the kernel is built for N = the max valid count
rounded up to 128 (384 for the reference inputs; trailing entries stay
masked so correctness holds for any input, degrading to N=512 worst case).

Math (per batch element b, N compacted obstacles, H=256):
  x      = [obs broadcast (12) | obstacle_data (4)]            [N, 16]
  a2q,a2k = first two MLP layers of q/k; v = full v-MLP (relu out)
  scores^T[m,n] = (G a2k_m).a2q_n + t[m], with G = (W3q/16) W3k^T and
           t = (W3k b3q/16).a2k precomputed/reassociated -- the q/k third
           layers are never materialized. The dropped q.b3k and b3q.b3k
           score terms are per-query/constant shifts that cancel exactly
           in the unnormalized ratio U/S below (verified 3e-7 on host).
  E^T    = exp(scores^T + negbias[key] + t[key])  (key mask + t as one
           per-partition exp bias; no max-subtract -- scores are O(0.1))
  U      = E^T-chunks^T @ [v | 1 | 1] -> [queries, H+2]; col H is sum_keys E
           (two ones columns: fp32r needs an even moving-free size)
  out    = U[:, :H] * (mask[q] / U[:, H])        per-partition scale
  pooled = max over queries (pairwise max + PE transpose + free-dim max)
  qval   = head MLP([pooled | obs | act])        (head W1 rows reordered)

All MLP layers run in transposed-activation layout [H, N] so every bias is
per-partition. All matmuls use float32r (1 cycle/row for N>=256 vs 4 for
fp32; ~1e-4 rel err). The K=4 L1 matmuls for q/k/v run concurrently in the
PE array via tile_position row groups 0/32/64 (weights+rhs replicated into
those partition bands). Elementwise ops are emitted as nc.any so the Tile
scheduler load-balances them across ScalarE/VectorE; exp stays on ScalarE.
The pool-stage PE transposes of each batch pair are emitted one pair late so
they never stall the next pair's L1/L2 matmuls, and the softmax-denominator
ones-columns are written by the v bias-row matmul itself (no per-batch
constant-write ops). Cost-model (TimelineSim) predicts ~247us/core at N=384
(PE 82%, DVE 80%, ACT 73% busy); on real silicon the packed L1 (which the
model charges serially, ~50us) should land meaningfully below that.
"""

import numpy as np

import concourse.bass as bass
import concourse.mybir as mybir
import concourse.tile as tile
from concourse import bacc
from concourse.bass_utils import run_bass_kernel_spmd

F32 = mybir.dt.float32
F32R = mybir.dt.float32r
AF = mybir.ActivationFunctionType
OP = mybir.AluOpType

N_CORES = 8
B = 256
BPC = B // N_CORES  # 32 batch elements per core
NFULL = 512         # obstacles in the input
# Masked obstacles contribute nothing to the output (keys excluded from
# softmax, query rows zeroed before max-pool), so the host compacts each
# batch element's obstacles valid-first and the kernel is built for the
# padded max valid count N <= 512. Recomputed from the mask per call.
N = 512
NCH = N // 128
H = 256             # hidden
D_OBS = 12
OBD = 4
ACT_D = 2

_last_results = None  # test.py introspects exec_time_ns from here
REPEAT = 1  # bench.py raises this to measure marginal batch-phase time

# pool sizing knobs (tuned via cost-model sweep in analyze.py)
POOLS = {"pa1": 6, "pa2": 12, "pqk": 6, "pv": 8, "pE": 8, "psc": 8, "pm": 4,
         "pp512": 3, "ppl1": 3, "ppa": 2, "ppsm": 0, "pa1b": 16}


def _r2(w):
    """[256, X] -> [128, 2*X] with col layout ksub*X + c (k-subtile major)."""
    x = w.shape[1]
    return np.ascontiguousarray(
        w.reshape(2, 128, x).transpose(1, 0, 2).reshape(128, 2 * x)
    )


def _col2(v):
    """[256] -> [128, 2], column j = chunk j."""
    return np.ascontiguousarray(v.reshape(2, 128).T)


def _prep_shared(q_params, k_params, v_params, head_params):
    arrs = {}
    for name, p in (("q", q_params), ("k", k_params), ("v", v_params)):
        w1, b1, w2, b2, w3, b3 = [np.asarray(a, np.float32) for a in p]
        arrs[f"w1o_{name}"] = np.ascontiguousarray(w1[:D_OBS])      # [12,256]
        arrs[f"w1t_{name}"] = np.ascontiguousarray(w1[D_OBS:])      # [4,256]
        arrs[f"b1c_{name}"] = _col2(b1)                             # [128,2]
        arrs[f"w2_{name}"] = _r2(w2)                                # [128,512]
        arrs[f"b2c_{name}"] = _col2(b2)
        if name == "v":
            arrs[f"w3_{name}"] = _r2(w3)
        if name == "v":
            # [b3v | 1 | 1]: the trailing ones land in psum cols H:H+2 via
            # the bias-row matmul, giving the softmax-denominator column
            # without a separate constant-write op
            arrs["b3v"] = np.ascontiguousarray(
                np.concatenate([b3, [1.0, 1.0]]).astype(np.float32)[None, :])
    # scores are computed as (G a2k)^T a2q + t[key]: the q.b3k and b3q.b3k
    # score terms are per-query/constant shifts that cancel exactly in the
    # unnormalized-softmax ratio U/S, so they are dropped; 1/16 is folded in
    w3q = np.asarray(q_params[4], np.float32) / 16.0
    b3q = np.asarray(q_params[5], np.float32) / 16.0
    w3k = np.asarray(k_params[4], np.float32)
    G_T = np.ascontiguousarray(w3k @ w3q.T)            # lhsT for Y: [g, h]
    arrs["g_r2"] = _r2(G_T)                            # [128, 512]
    u = w3k @ b3q                                      # [256] per-key bias vec
    u2 = np.zeros((128, 4), np.float32)
    for j in range(2):
        u2[:, 2 * j] = u[128 * j:128 * (j + 1)]
        u2[:, 2 * j + 1] = u[128 * j:128 * (j + 1)]    # fp32r even-N dup
    arrs["u2"] = u2
    w1h, b1h, w2h, b2h, w3h, b3h = [np.asarray(a, np.float32) for a in head_params]
    # comb order in-kernel: [pooled (256) | obs (12) | act (2)]
    arrs["w1h_a"] = np.ascontiguousarray(w1h[D_OBS : D_OBS + 128])          # [128,256]
    arrs["w1h_b"] = np.ascontiguousarray(w1h[D_OBS + 128 : D_OBS + 256])    # [128,256]
    arrs["w1h_c"] = np.ascontiguousarray(
        np.concatenate([w1h[:D_OBS], w1h[D_OBS + 256 :]], 0)                # [14,256]
    )
    arrs["b1hc"] = _col2(b1h)
    arrs["w2h"] = _r2(w2h)
    arrs["b2hc"] = _col2(b2h)
    arrs["w3h"] = np.ascontiguousarray(w3h.reshape(2, 128).T)               # [128,2]
    arrs["b3h"] = np.ascontiguousarray(b3h.reshape(1, 1))                   # [1,1]
    arrs["iden"] = np.eye(128, dtype=np.float32)
    arrs["ones128"] = np.ones((1, 128), np.float32)
    return arrs


def _prep_core(obs, obstacles, act, c):
    s = slice(c * BPC, (c + 1) * BPC)
    obs_c = np.asarray(obs[s], np.float32)            # [32,12]
    act_c = np.asarray(act[s], np.float32)            # [32,2]
    obst_c = np.asarray(obstacles[s], np.float32)     # [32,5,512]
    # compact each batch element's obstacles valid-first, keep the first N
    # (N was sized so every dropped obstacle is masked; padding stays masked)
    comp = np.empty((BPC, 5, N), np.float32)
    for b in range(BPC):
        m = obst_c[b, OBD, :]
        idx = np.concatenate([np.nonzero(m > 0)[0], np.nonzero(m <= 0)[0]])[:N]
        comp[b] = obst_c[b][:, idx]
    arrs = {}
    arrs["obst"] = np.ascontiguousarray(
        comp[:, :OBD, :].transpose(1, 0, 2).reshape(OBD, BPC * N)
    )                                                  # [4, 32*N]
    arrs["obsT"] = np.ascontiguousarray(obs_c.T)       # [12,32]
    arrs["headxT"] = np.ascontiguousarray(
        np.concatenate([obs_c.T, act_c.T], 0)
    )                                                  # [14,32]
    mask = comp[:, OBD, :]                             # [32,N]
    # [p, kc*32 + b] = mask[b, kc*128 + p]
    maskT = mask.T.reshape(NCH, 128, BPC).transpose(1, 0, 2).reshape(
        128, NCH * BPC)
    arrs["maskT"] = np.ascontiguousarray(maskT)
    arrs["negbT"] = np.ascontiguousarray((maskT - 1.0) * 1e9)
    return arrs


# name -> (shape, dtype): f32r for anything a matmul consumes
_SHARED_SPECS = {}
for _m in ("q", "k", "v"):
    _SHARED_SPECS.update({
        f"w1o_{_m}": ([D_OBS, H], F32R),
        f"w1t_{_m}": ([OBD, H], F32R),
        f"b1c_{_m}": ([128, 2], F32),
        f"w2_{_m}": ([128, 2 * H], F32R),
        f"b2c_{_m}": ([128, 2], F32),
    })
_SHARED_SPECS.update({
    "w3_v": ([128, 2 * H], F32R),
    "g_r2": ([128, 2 * H], F32R),
    "u2": ([128, 4], F32R),
    "b3v": ([1, H + 2], F32R),
    "w1h_a": ([128, H], F32R),
    "w1h_b": ([128, H], F32R),
    "w1h_c": ([14, H], F32R),
    "b1hc": ([128, 2], F32),
    "w2h": ([128, 2 * H], F32R),
    "b2hc": ([128, 2], F32),
    "w3h": ([128, 2], F32R),
    "b3h": ([1, 1], F32),
    "iden": ([128, 128], F32R),
    "ones128": ([1, 128], F32R),
})
def _core_specs():
    return {
        "obst": ([OBD, BPC * N], F32R),  # DMA'd 3x into bands 0/32/64
        "obsT": ([D_OBS, BPC], F32R),
        "headxT": ([14, BPC], F32R),
        "maskT": ([128, NCH * BPC], F32),
        "negbT": ([128, NCH * BPC], F32),
    }


def _set_n(n):
    global N, NCH
    N = n
    NCH = n // 128


def _build():
    nc = bacc.Bacc("TRN2", target_bir_lowering=False, debug=False,
                   num_devices=N_CORES)
    d = {}
    for name, (shape, dt) in {**_SHARED_SPECS, **_core_specs()}.items():
        d[name] = nc.dram_tensor(name, shape, dt, kind="ExternalInput")
    out_dram = nc.dram_tensor("out", [1, BPC], F32, kind="ExternalOutput")

    with tile.TileContext(nc) as tc:
        _emit(nc, tc, d, out_dram)
    nc.compile()
    return nc


def _emit(nc, tc, d, out_dram):
    from contextlib import ExitStack
    ctx = ExitStack()
    with ctx:
        const = ctx.enter_context(tc.tile_pool(name="const", bufs=1))
        pa1 = ctx.enter_context(tc.tile_pool(name="pa1", bufs=POOLS["pa1b"]))
        pa2 = ctx.enter_context(tc.tile_pool(name="pa2", bufs=POOLS["pa2"]))
        pqk = ctx.enter_context(tc.tile_pool(name="pqk", bufs=POOLS["pqk"]))
        pv = ctx.enter_context(tc.tile_pool(name="pv", bufs=POOLS["pv"]))
        pE = ctx.enter_context(tc.tile_pool(name="pE", bufs=POOLS["pE"]))
        psc = ctx.enter_context(tc.tile_pool(name="psc", bufs=POOLS["psc"]))
        pm = ctx.enter_context(tc.tile_pool(name="pm", bufs=POOLS["pm"]))
        ptiny = ctx.enter_context(tc.tile_pool(name="ptiny", bufs=8))
        pout = ctx.enter_context(tc.tile_pool(name="pout", bufs=2))
        pp512 = ctx.enter_context(tc.tile_pool(name="pp512", bufs=POOLS["pp512"], space="PSUM"))
        ppl1 = ctx.enter_context(tc.tile_pool(name="ppl1", bufs=POOLS["ppl1"], space="PSUM"))
        ppa = ctx.enter_context(tc.tile_pool(name="ppa", bufs=POOLS["ppa"], space="PSUM"))
        ppsm = ppa  # C/head psums share the attention psum pool

        # ---- load everything to SBUF (first-use order so compute can
        # start as soon as the L1 inputs land, instead of after all 2.7MB) ----
        all_specs = {**_SHARED_SPECS, **_core_specs()}
        first = ["obsT", "w1o_q", "w1o_k", "w1o_v", "b1c_q", "b1c_k", "b1c_v",
                 "w1t_q", "w1t_k", "w1t_v", "obst",
                 "w2_q", "b2c_q", "w2_k", "b2c_k", "w2_v", "b2c_v",
                 "g_r2", "u2", "w3_v", "b3v", "ones128",
                 "negbT", "maskT", "iden"]
        order = first + [n for n in all_specs if n not in first]
        sb = {}
        skip_plain = {"obst", "w1t_q", "w1t_k", "w1t_v"}
        # packed tiles: q/k/v L1 runs as 3 concurrent row-group matmuls
        # (tile_position rows 0/32/64), so weights and the obstacle rhs are
        # replicated into those partition bands
        obst3 = const.tile([128, BPC * N], F32R, tag="obst3", name="obst3")
        w1t_pack = const.tile([128, 2 * 128], F32R, tag="w1t_pack",
                              name="w1t_pack")
        def load_plain(names):
            for name in names:
                shape, dt = all_specs[name]
                t = const.tile(shape, dt, tag=name, name=name)
                nc.sync.dma_start(t[:], d[name][:])
                sb[name] = t
        # tiny setup tensors first (C matmuls + L1 weights), then the three
        # 256KB obstacle bands, then everything else in first-use order
        setup = ["obsT", "w1o_q", "w1o_k", "w1o_v", "b1c_q", "b1c_k", "b1c_v"]
        load_plain(setup)
        for i, m in enumerate(("q", "k", "v")):
            nc.sync.dma_start(w1t_pack[32 * i:32 * i + OBD, :], d[f"w1t_{m}"][:])
        for i in range(3):
            nc.sync.dma_start(obst3[32 * i:32 * i + OBD, :], d["obst"][:])
        load_plain([n for n in order if n not in skip_plain and n not in setup])

        # ---- per-core setup: C^T[mlp] = W1[:12].T @ obs + b1 (per-partition) ----
        cmt = {}
        for m in ("q", "k", "v"):
            for j in range(2):
                ps = ppsm.tile([128, BPC], F32, tag="psa")
                nc.tensor.matmul(ps[:], sb[f"w1o_{m}"][:, j * 128:(j + 1) * 128],
                                 sb["obsT"][:], start=True, stop=True)
                ct = const.tile([128, BPC], F32, tag=f"cmt_{m}{j}")
                nc.scalar.activation(ct[:], ps[:], AF.Identity,
                                     bias=sb[f"b1c_{m}"][:, j:j + 1])
                cmt[(m, j)] = ct

        pooledT = [const.tile([128, BPC], F32R, tag=f"pooled{j}", name=f"pooled{j}")
                   for j in range(2)]

        # ---- main batch loop: pairs of batch elems, stage-interleaved so the
        # PE always has the sibling batch's matmuls to run while ACT/DVE
        # produce this batch's activations ----
        def stage_l1(b):
            # one row-group pack per h-chunk j: q/k/v L1 matmuls execute
            # concurrently in the PE array (K=4 each, rows 0/32/64)
            a1 = {m: [] for m in ("q", "k", "v")}
            for j in range(2):
                for i, m in enumerate(("q", "k", "v")):
                    ps = ppl1.tile([128, N], F32, tag="psl1", name="l1ps")
                    nc.tensor.matmul(
                        ps[:],
                        w1t_pack[32 * i:32 * i + OBD, j * 128:(j + 1) * 128],
                        obst3[32 * i:32 * i + OBD, b * N:(b + 1) * N],
                        start=True, stop=True)
                    a1t = pa1.tile([128, N], F32R, tag="a1", name="a1")
                    nc.any.tensor_scalar(a1t[:], ps[:], cmt[(m, j)][:, b:b + 1],
                                         0.0, OP.add, OP.max)
                    a1[m].append(a1t)
            return a1

        def stage_l2(b, m, a1_m):
            a2_m = []
            for j in range(2):
                ps = pp512.tile([128, N], F32, tag="ps512", name="l2ps")
                for ks in range(2):
                    nc.tensor.matmul(
                        ps[:],
                        sb[f"w2_{m}"][:, ks * H + j * 128: ks * H + j * 128 + 128],
                        a1_m[ks][:], start=(ks == 0), stop=(ks == 1))
                a2t = pa2.tile([128, N], F32R, tag="a2", name="a2")
                nc.any.tensor_scalar(a2t[:], ps[:],
                                     sb[f"b2c_{m}"][:, j:j + 1], 0.0,
                                     OP.add, OP.max)
                a2_m.append(a2t)
            return a2_m

        def stage_Y(b, a2k):
            # Y = G^T-matmul of a2k; scores = Y^T a2q (q/k L3 eliminated)
            Y = []
            for j in range(2):
                ps = ppl1.tile([128, N], F32, tag="psl1", name="yps")
                for ks in range(2):
                    nc.tensor.matmul(
                        ps[:],
                        sb["g_r2"][:, ks * H + j * 128: ks * H + j * 128 + 128],
                        a2k[ks][:], start=(ks == 0), stop=(ks == 1))
                yt = pqk.tile([128, N], F32R, tag="YT", name="yt")
                nc.any.tensor_scalar(yt[:], ps[:], 0.0, None, OP.add)
                Y.append(yt)
            return Y

        def stage_tT(b, a2k):
            # per-key score bias t = (W3k b3q)^T a2k, computed transposed
            # [128,1] per key chunk and merged with the mask bias for exp
            eb = []
            for kc in range(NCH):
                tps = ppa.tile([128, 2], F32, tag="psa", name="tps")
                for j in range(2):
                    nc.tensor.matmul(tps[:],
                                     a2k[j][:, kc * 128:(kc + 1) * 128],
                                     sb["u2"][:, 2 * j:2 * j + 2],
                                     start=(j == 0), stop=(j == 1))
                e = ptiny.tile([128, 1], F32, tag="ebias", name="ebias")
                nc.vector.tensor_tensor(
                    e[:], tps[:, 0:1],
                    sb["negbT"][:, kc * BPC + b: kc * BPC + b + 1], OP.add)
                eb.append(e)
            return eb

        def stage_l3v(b, a2_m):
            # flipped layout v[keys, h] (+bias via ones-row matmul, relu);
            # col H..H+2 set to 1.0: col H gives the softmax denominator in
            # the attnout matmul; col H+1 is fp32r even-free-size padding.
            v_sb = []
            for rc in range(NCH):
                ps = ppa.tile([128, H + 2], F32, tag="psa", name="vps")
                for ks in range(2):
                    nc.tensor.matmul(ps[:, 0:H],
                                     a2_m[ks][:, rc * 128:(rc + 1) * 128],
                                     sb["w3_v"][:, ks * H:(ks + 1) * H],
                                     start=(ks == 0), stop=False)
                nc.tensor.matmul(ps[:], sb["ones128"][:], sb["b3v"][:],
                                 start=False, stop=True)
                vt = pv.tile([128, H + 2], F32R, tag="vsb", name="vsb")
                # relu covers the ones columns too: max(1, 0) = 1
                nc.vector.tensor_scalar(vt[:], ps[:], 0.0, None, OP.max)
                v_sb.append(vt)
            return v_sb

        def stage_scores(b, Y, a2q, eb):
            E = []
            for kc in range(NCH):
                ps = pp512.tile([128, N], F32, tag="ps512", name="scps")
                for j in range(2):
                    nc.tensor.matmul(ps[:],
                                     Y[j][:, kc * 128:(kc + 1) * 128],
                                     a2q[j][:], start=(j == 0), stop=(j == 1))
                e = pE.tile([128, N], F32R, tag="E", name="E")
                nc.scalar.activation(e[:], ps[:], AF.Exp, bias=eb[kc][:])
                E.append(e)
            return E

        def stage_attnout(b, E, v_sb):
            scaled = []
            for qc in range(NCH):
                ps = ppa.tile([128, H + 2], F32, tag="psa", name="aops")
                for kc in range(NCH):
                    nc.tensor.matmul(ps[:],
                                     E[kc][:, qc * 128:(qc + 1) * 128],
                                     v_sb[kc][:], start=(kc == 0),
                                     stop=(kc == NCH - 1))
                # every batch elem has >0 valid keys (verified on host data),
                # so S>0 and the reciprocal is finite
                rec = ptiny.tile([128, 1], F32, tag="rec", name="rec")
                nc.vector.reciprocal(rec[:], ps[:, H:H + 1])
                sc = psc.tile([128, H], F32, tag="scaled", name="scaled")
                nc.any.tensor_scalar(
                    sc[:], ps[:, 0:H], rec[:],
                    sb["maskT"][:, qc * BPC + b: qc * BPC + b + 1],
                    OP.mult, OP.mult)
                scaled.append(sc)
            return scaled

        def stage_pool_max(b, scaled):
            # pairwise max tree over the NCH scaled chunks; final node f32r
            work = list(scaled)
            cnt = 0
            while len(work) > 2:
                m01 = pm.tile([128, H], F32, tag=f"mx{cnt % 2}", name="mx")
                nc.any.tensor_tensor(m01[:], work[0][:], work[1][:], OP.max)
                work = [m01] + work[2:]
                cnt += 1
            m3 = pm.tile([128, H], F32R, tag="m3", name="m3", bufs=6)
            if len(work) == 2:
                nc.any.tensor_tensor(m3[:], work[0][:], work[1][:], OP.max)
            else:
                nc.any.tensor_scalar(m3[:], work[0][:], 0.0, None, OP.add)
            return m3

        def stage_pool_reduce(b, m3):
            # emitted one pair late: keeps the PE transposes (which wait on
            # the DVE max chain) from stalling the next pair's L1 matmuls
            for hc in range(2):
                trp = ppa.tile([128, 128], F32R, tag="psa", name="trp")
                nc.tensor.transpose(trp[:], m3[:, hc * 128:(hc + 1) * 128],
                                    sb["iden"][:])
                nc.vector.tensor_reduce(pooledT[hc][:, b:b + 1], trp[:],
                                        mybir.AxisListType.X, OP.max)

        pending_pool = []
        pairs = [((2 * p) % BPC, (2 * p + 1) % BPC)
                 for p in range(REPEAT * (BPC // 2))]
        next_a1 = None
        for pi, bb in enumerate(pairs):
            st = {b: {} for b in bb}
            if next_a1 is None:
                next_a1 = {b: stage_l1(b) for b in bb}
                next_a2q = {b: stage_l2(b, "q", next_a1[b]["q"]) for b in bb}
            for b in bb:
                for m in ("q", "k", "v"):
                    st[b][f"a1{m}"] = next_a1[b][m]
                st[b]["a2q"] = next_a2q[b]
            next_a1 = None
            done_pending = False
            for m in ("k", "v"):
                for b in bb:
                    st[b][f"a2{m}"] = stage_l2(b, m, st[b][f"a1{m}"])
                if not done_pending:
                    # previous pair's pool transposes, emitted here so they
                    # never stall this pair's L1/L2 matmuls on the PE queue
                    for pb, pm3 in pending_pool:
                        stage_pool_reduce(pb, pm3)
                    pending_pool = []
                    done_pending = True
                for b in bb:
                    if m == "v":
                        st[b]["v"] = stage_l3v(b, st[b]["a2v"])
                    elif m == "k":
                        st[b]["Y"] = stage_Y(b, st[b]["a2k"])
                        st[b]["eb"] = stage_tT(b, st[b]["a2k"])
            for b in bb:
                st[b]["E"] = stage_scores(b, st[b]["Y"], st[b]["a2q"], st[b]["eb"])
            if pi + 1 < len(pairs):
                # emit the next pair's L1 (and its q-branch L2) here: they
                # fill the PE bubble while this pair's exp chain produces E
                # for attnout -- sized for silicon, where the packed L1 runs
                # ~3x faster than the cost model charges
                next_a1 = {b: stage_l1(b) for b in pairs[pi + 1]}
                next_a2q = {b: stage_l2(b, "q", next_a1[b]["q"])
                            for b in pairs[pi + 1]}
            for b in bb:
                st[b]["sc"] = stage_attnout(b, st[b]["E"], st[b]["v"])
            for b in bb:
                pending_pool.append((b, stage_pool_max(b, st[b]["sc"])))
        for pb, pm3 in pending_pool:
            stage_pool_reduce(pb, pm3)

        # ---- head MLP on all 32 batch elems (transposed [h, b]) ----
        a1h = []
        for j in range(2):
            ps = ppsm.tile([128, BPC], F32, tag="psa")
            nc.tensor.matmul(ps[:], sb["w1h_a"][:, j * 128:(j + 1) * 128],
                             pooledT[0][:], start=True, stop=False)
            nc.tensor.matmul(ps[:], sb["w1h_b"][:, j * 128:(j + 1) * 128],
                             pooledT[1][:], start=False, stop=False)
            nc.tensor.matmul(ps[:], sb["w1h_c"][:, j * 128:(j + 1) * 128],
                             sb["headxT"][:], start=False, stop=True)
            a = pout.tile([128, BPC], F32R, tag="a1h")
            nc.scalar.activation(a[:], ps[:], AF.Relu, bias=sb["b1hc"][:, j:j + 1])
            a1h.append(a)
        a2h = []
        for j in range(2):
            ps = ppsm.tile([128, BPC], F32, tag="psa")
            for ks in range(2):
                nc.tensor.matmul(ps[:],
                                 sb["w2h"][:, ks * H + j * 128: ks * H + j * 128 + 128],
                                 a1h[ks][:], start=(ks == 0), stop=(ks == 1))
            a = pout.tile([128, BPC], F32R, tag="a2h")
            nc.scalar.activation(a[:], ps[:], AF.Relu, bias=sb["b2hc"][:, j:j + 1])
            a2h.append(a)
        ps = ppsm.tile([1, BPC], F32, tag="psa")
        for ks in range(2):
            nc.tensor.matmul(ps[:], sb["w3h"][:, ks:ks + 1], a2h[ks][:],
                             start=(ks == 0), stop=(ks == 1))
        ot = pout.tile([1, BPC], F32, tag="osb")
        nc.vector.tensor_scalar(ot[:], ps[:], sb["b3h"][:, 0:1], None, OP.add)
        nc.sync.dma_start(out_dram[:], ot[:])


def kernel(obs, obstacles, act, q_params, k_params, v_params, head_params):
    global _last_results
    maxvalid = int((np.asarray(obstacles)[:, OBD, :] > 0).sum(axis=1).max())
    _set_n(min(NFULL, max(128, -(-maxvalid // 128) * 128)))
    shared = _prep_shared(q_params, k_params, v_params, head_params)
    in_maps = []
    for c in range(N_CORES):
        m = dict(shared)
        m.update(_prep_core(obs, obstacles, act, c))
        in_maps.append(m)
    nc = _build()
    res = run_bass_kernel_spmd(nc, in_maps, core_ids=list(range(N_CORES)))
    _last_results = res
    out = np.concatenate([res.results[c]["out"][0] for c in range(N_CORES)])
    return out.astype(np.float32)


# revision 41
# speedup vs baseline: 1.4759x; 1.0118x over previous
"""Trainium2 Bass kernel for the AttentionQFunction problem.

Contract: kernel(**inputs) takes FULL inputs (B=256) and returns the FULL
[256] float32 output. Internally the batch is sharded 32-per-core across 8
NeuronCores (pure data parallel); the small MLP weights are replicated.

Sparse compaction: masked obstacles contribute nothing to the output
(keys are excluded from softmax; query rows are zeroed before the max-pool,
and all pooled values are >= 0), so the host reorders each batch element's
obstacles valid-first and the kernel is built for N = the max valid count
rounded up to 128 (384 for the reference inputs; trailing entries stay
masked so correctness holds for any input, degrading to N=512 worst case).

Math (per batch element b, N compacted obstacles, H=256):
  x      = [obs broadcast (12) | obstacle_data (4)]            [N, 16]
  a2q,a2k = first two MLP layers of q/k; v = full v-MLP (relu out)
  scores^T[m,n] = (G a2k_m).a2q_n + t[m], with G = (W3q/16) W3k^T and
           t = (W3k b3q/16).a2k precomputed/reassociated -- the q/k third
           layers are never materialized. The dropped q.b3k and b3q.b3k
           score terms are per-query/constant shifts that cancel exactly
           in the unnormalized ratio U/S below (verified 3e-7 on host).
  E^T    = exp(scores^T + negbias[key] + t[key])  (key mask + t as one
           per-partition exp bias; no max-subtract -- scores are O(0.1))
  U      = E^T-chunks^T @ [v | 1 | 1] -> [queries, H+2]; col H is sum_keys E
           (two ones columns: fp32r needs an even moving-free size)
  out    = U[:, :H] * (mask[q] / U[:, H])        per-partition scale
  pooled = max over queries (pairwise max + PE transpose + free-dim max)
  qval   = head MLP([pooled | obs | act])        (head W1 rows reordered)

All MLP layers run in transposed-activation layout [H, N] so every bias is
per-partition. All matmuls use float32r (1 cycle/row for N>=256 vs 4 for
fp32; ~1e-4 rel err). The K=4 L1 matmuls for q/k/v run concurrently in the
PE array via tile_position row groups 0/32/64 (weights+rhs replicated into
those partition bands). Elementwise ops are emitted as nc.any so the Tile
scheduler load-balances them across ScalarE/VectorE; exp stays on ScalarE.
The pool-stage PE transposes of each batch pair are emitted one pair late so
they never stall the next pair's L1/L2 matmuls, and the softmax-denominator
ones-columns are written by the v bias-row matmul itself (no per-batch
constant-write ops). Cost-model (TimelineSim) predicts ~247us/core at N=384
(PE 82%, DVE 80%, ACT 73% busy); on real silicon the packed L1 (which the
model charges serially, ~50us) should land meaningfully below that.
"""

import numpy as np

import concourse.bass as bass
import concourse.mybir as mybir
import concourse.tile as tile
from concourse import bacc
from concourse.bass_utils import run_bass_kernel_spmd

F32 = mybir.dt.float32
F32R = mybir.dt.float32r
AF = mybir.ActivationFunctionType
OP = mybir.AluOpType

N_CORES = 8
B = 256
BPC = B // N_CORES  # 32 batch elements per core
NFULL = 512         # obstacles in the input
# Masked obstacles contribute nothing to the output (keys excluded from
# softmax, query rows zeroed before max-pool), so the host compacts each
# batch element's obstacles valid-first and the kernel is built for the
# padded max valid count N <= 512. Recomputed from the mask per call.
N = 512
NCH = N // 128
H = 256             # hidden
D_OBS = 12
OBD = 4
ACT_D = 2

_last_results = None  # test.py introspects exec_time_ns from here
REPEAT = 1  # bench.py raises this to measure marginal batch-phase time

# pool sizing knobs (tuned via cost-model sweep in analyze.py)
POOLS = {"pa1": 6, "pa2": 12, "pqk": 6, "pv": 8, "pE": 8, "psc": 8, "pm": 4,
         "pp512": 3, "ppl1": 3, "ppa": 2, "ppsm": 0, "pa1b": 16}


def _r2(w):
    """[256, X] -> [128, 2*X] with col layout ksub*X + c (k-subtile major)."""
    x = w.shape[1]
    return np.ascontiguousarray(
        w.reshape(2, 128, x).transpose(1, 0, 2).reshape(128, 2 * x)
    )


def _col2(v):
    """[256] -> [128, 2], column j = chunk j."""
    return np.ascontiguousarray(v.reshape(2, 128).T)


def _prep_shared(q_params, k_params, v_params, head_params):
    arrs = {}
    for name, p in (("q", q_params), ("k", k_params), ("v", v_params)):
        w1, b1, w2, b2, w3, b3 = [np.asarray(a, np.float32) for a in p]
        arrs[f"w1o_{name}"] = np.ascontiguousarray(w1[:D_OBS])      # [12,256]
        arrs[f"w1t_{name}"] = np.ascontiguousarray(w1[D_OBS:])      # [4,256]
        arrs[f"b1c_{name}"] = _col2(b1)                             # [128,2]
        arrs[f"w2_{name}"] = _r2(w2)                                # [128,512]
        arrs[f"b2c_{name}"] = _col2(b2)
        if name == "v":
            arrs[f"w3_{name}"] = _r2(w3)
        if name == "v":
            # [b3v | 1 | 1]: the trailing ones land in psum cols H:H+2 via
            # the bias-row matmul, giving the softmax-denominator column
            # without a separate constant-write op
            arrs["b3v"] = np.ascontiguousarray(
                np.concatenate([b3, [1.0, 1.0]]).astype(np.float32)[None, :])
    # scores are computed as (G a2k)^T a2q + t[key]: the q.b3k and b3q.b3k
    # score terms are per-query/constant shifts that cancel exactly in the
    # unnormalized-softmax ratio U/S, so they are dropped; 1/16 is folded in
    w3q = np.asarray(q_params[4], np.float32) / 16.0
    b3q = np.asarray(q_params[5], np.float32) / 16.0
    w3k = np.asarray(k_params[4], np.float32)
    G_T = np.ascontiguousarray(w3k @ w3q.T)            # lhsT for Y: [g, h]
    arrs["g_r2"] = _r2(G_T)                            # [128, 512]
    u = w3k @ b3q                                      # [256] per-key bias vec
    u2 = np.zeros((128, 4), np.float32)
    for j in range(2):
        u2[:, 2 * j] = u[128 * j:128 * (j + 1)]
        u2[:, 2 * j + 1] = u[128 * j:128 * (j + 1)]    # fp32r even-N dup
    arrs["u2"] = u2
    w1h, b1h, w2h, b2h, w3h, b3h = [np.asarray(a, np.float32) for a in head_params]
    # comb order in-kernel: [pooled (256) | obs (12) | act (2)]
    arrs["w1h_a"] = np.ascontiguousarray(w1h[D_OBS : D_OBS + 128])          # [128,256]
    arrs["w1h_b"] = np.ascontiguousarray(w1h[D_OBS + 128 : D_OBS + 256])    # [128,256]
    arrs["w1h_c"] = np.ascontiguousarray(
        np.concatenate([w1h[:D_OBS], w1h[D_OBS + 256 :]], 0)                # [14,256]
    )
    arrs["b1hc"] = _col2(b1h)
    arrs["w2h"] = _r2(w2h)
    arrs["b2hc"] = _col2(b2h)
    arrs["w3h"] = np.ascontiguousarray(w3h.reshape(2, 128).T)               # [128,2]
    arrs["b3h"] = np.ascontiguousarray(b3h.reshape(1, 1))                   # [1,1]
    arrs["iden"] = np.eye(128, dtype=np.float32)
    arrs["ones128"] = np.ones((1, 128), np.float32)
    return arrs


def _prep_core(obs, obstacles, act, c):
    s = slice(c * BPC, (c + 1) * BPC)
    obs_c = np.asarray(obs[s], np.float32)            # [32,12]
    act_c = np.asarray(act[s], np.float32)            # [32,2]
    obst_c = np.asarray(obstacles[s], np.float32)     # [32,5,512]
    # compact each batch element's obstacles valid-first, keep the first N
    # (N was sized so every dropped obstacle is masked; padding stays masked)
    comp = np.empty((BPC, 5, N), np.float32)
    for b in range(BPC):
        m = obst_c[b, OBD, :]
        idx = np.concatenate([np.nonzero(m > 0)[0], np.nonzero(m <= 0)[0]])[:N]
        comp[b] = obst_c[b][:, idx]
    arrs = {}
    arrs["obst"] = np.ascontiguousarray(
        comp[:, :OBD, :].transpose(1, 0, 2).reshape(OBD, BPC * N)
    )                                                  # [4, 32*N]
    arrs["obsT"] = np.ascontiguousarray(obs_c.T)       # [12,32]
    arrs["headxT"] = np.ascontiguousarray(
        np.concatenate([obs_c.T, act_c.T], 0)
    )                                                  # [14,32]
    mask = comp[:, OBD, :]                             # [32,N]
    # [p, kc*32 + b] = mask[b, kc*128 + p]
    maskT = mask.T.reshape(NCH, 128, BPC).transpose(1, 0, 2).reshape(
        128, NCH * BPC)
    arrs["maskT"] = np.ascontiguousarray(maskT)
    arrs["negbT"] = np.ascontiguousarray((maskT - 1.0) * 1e9)
    return arrs


# name -> (shape, dtype): f32r for anything a matmul consumes
_SHARED_SPECS = {}
for _m in ("q", "k", "v"):
    _SHARED_SPECS.update({
        f"w1o_{_m}": ([D_OBS, H], F32R),
        f"w1t_{_m}": ([OBD, H], F32R),
        f"b1c_{_m}": ([128, 2], F32),
        f"w2_{_m}": ([128, 2 * H], F32R),
        f"b2c_{_m}": ([128, 2], F32),
    })
_SHARED_SPECS.update({
    "w3_v": ([128, 2 * H], F32R),
    "g_r2": ([128, 2 * H], F32R),
    "u2": ([128, 4], F32R),
    "b3v": ([1, H + 2], F32R),
    "w1h_a": ([128, H], F32R),
    "w1h_b": ([128, H], F32R),
    "w1h_c": ([14, H], F32R),
    "b1hc": ([128, 2], F32),
    "w2h": ([128, 2 * H], F32R),
    "b2hc": ([128, 2], F32),
    "w3h": ([128, 2], F32R),
    "b3h": ([1, 1], F32),
    "iden": ([128, 128], F32R),
    "ones128": ([1, 128], F32R),
})
def _core_specs():
    return {
        "obst": ([OBD, BPC * N], F32R),  # DMA'd 3x into bands 0/32/64
        "obsT": ([D_OBS, BPC], F32R),
        "headxT": ([14, BPC], F32R),
        "maskT": ([128, NCH * BPC], F32),
        "negbT": ([128, NCH * BPC], F32),
    }


def _set_n(n):
    global N, NCH
    N = n
    NCH = n // 128


def _build():
    nc = bacc.Bacc("TRN2", target_bir_lowering=False, debug=False,
                   num_devices=N_CORES)
    d = {}
    for name, (shape, dt) in {**_SHARED_SPECS, **_core_specs()}.items():
        d[name] = nc.dram_tensor(name, shape, dt, kind="ExternalInput")
    out_dram = nc.dram_tensor("out", [1, BPC], F32, kind="ExternalOutput")

    with tile.TileContext(nc) as tc:
        _emit(nc, tc, d, out_dram)
    nc.compile()
    return nc


def _emit(nc, tc, d, out_dram):
    from contextlib import ExitStack
    ctx = ExitStack()
    with ctx:
        const = ctx.enter_context(tc.tile_pool(name="const", bufs=1))
        pa1 = ctx.enter_context(tc.tile_pool(name="pa1", bufs=POOLS["pa1b"]))
        pa2 = ctx.enter_context(tc.tile_pool(name="pa2", bufs=POOLS["pa2"]))
        pqk = ctx.enter_context(tc.tile_pool(name="pqk", bufs=POOLS["pqk"]))
        pv = ctx.enter_context(tc.tile_pool(name="pv", bufs=POOLS["pv"]))
        pE = ctx.enter_context(tc.tile_pool(name="pE", bufs=POOLS["pE"]))
        psc = ctx.enter_context(tc.tile_pool(name="psc", bufs=POOLS["psc"]))
        pm = ctx.enter_context(tc.tile_pool(name="pm", bufs=POOLS["pm"]))
        ptiny = ctx.enter_context(tc.tile_pool(name="ptiny", bufs=8))
        pout = ctx.enter_context(tc.tile_pool(name="pout", bufs=2))
        pp512 = ctx.enter_context(tc.tile_pool(name="pp512", bufs=POOLS["pp512"], space="PSUM"))
        ppl1 = ctx.enter_context(tc.tile_pool(name="ppl1", bufs=POOLS["ppl1"], space="PSUM"))
        ppa = ctx.enter_context(tc.tile_pool(name="ppa", bufs=POOLS["ppa"], space="PSUM"))
        ppsm = ppa  # C/head psums share the attention psum pool

        # ---- load everything to SBUF (first-use order so compute can
        # start as soon as the L1 inputs land, instead of after all 2.7MB) ----
        all_specs = {**_SHARED_SPECS, **_core_specs()}
        first = ["obsT", "w1o_q", "w1o_k", "w1o_v", "b1c_q", "b1c_k", "b1c_v",
                 "w1t_q", "w1t_k", "w1t_v", "obst",
                 "w2_q", "b2c_q", "w2_k", "b2c_k", "w2_v", "b2c_v",
                 "g_r2", "u2", "w3_v", "b3v", "ones128",
                 "negbT", "maskT", "iden"]
        order = first + [n for n in all_specs if n not in first]
        sb = {}
        skip_plain = {"obst", "w1t_q", "w1t_k", "w1t_v"}
        # packed tiles: q/k/v L1 runs as 3 concurrent row-group matmuls
        # (tile_position rows 0/32/64), so weights and the obstacle rhs are
        # replicated into those partition bands
        obst3 = const.tile([128, BPC * N], F32R, tag="obst3", name="obst3")
        w1t_pack = const.tile([128, 2 * 128], F32R, tag="w1t_pack",
                              name="w1t_pack")
        def load_plain(names):
            for name in names:
                shape, dt = all_specs[name]
                t = const.tile(shape, dt, tag=name, name=name)
                nc.sync.dma_start(t[:], d[name][:])
                sb[name] = t
        # tiny setup tensors first (C matmuls + L1 weights), then the three
        # 256KB obstacle bands, then everything else in first-use order
        setup = ["obsT", "w1o_q", "w1o_k", "w1o_v", "b1c_q", "b1c_k", "b1c_v"]
        load_plain(setup)
        for i, m in enumerate(("q", "k", "v")):
            nc.sync.dma_start(w1t_pack[32 * i:32 * i + OBD, :], d[f"w1t_{m}"][:])
        for i in range(3):
            nc.sync.dma_start(obst3[32 * i:32 * i + OBD, :], d["obst"][:])
        load_plain([n for n in order if n not in skip_plain and n not in setup])

        # ---- per-core setup: C^T[mlp] = W1[:12].T @ obs + b1 (per-partition) ----
        cmt = {}
        for m in ("q", "k", "v"):
            for j in range(2):
                ps = ppsm.tile([128, BPC], F32, tag="psa")
                nc.tensor.matmul(ps[:], sb[f"w1o_{m}"][:, j * 128:(j + 1) * 128],
                                 sb["obsT"][:], start=True, stop=True)
                ct = const.tile([128, BPC], F32, tag=f"cmt_{m}{j}")
                nc.scalar.activation(ct[:], ps[:], AF.Identity,
                                     bias=sb[f"b1c_{m}"][:, j:j + 1])
                cmt[(m, j)] = ct

        pooledT = [const.tile([128, BPC], F32R, tag=f"pooled{j}", name=f"pooled{j}")
                   for j in range(2)]

        # ---- main batch loop: pairs of batch elems, stage-interleaved so the
        # PE always has the sibling batch's matmuls to run while ACT/DVE
        # produce this batch's activations ----
        def stage_l1(b):
            # one row-group pack per h-chunk j: q/k/v L1 matmuls execute
            # concurrently in the PE array (K=4 each, rows 0/32/64)
            a1 = {m: [] for m in ("q", "k", "v")}
            for j in range(2):
                for i, m in enumerate(("q", "k", "v")):
                    ps = ppl1.tile([128, N], F32, tag="psl1", name="l1ps")
                    nc.tensor.matmul(
                        ps[:],
                        w1t_pack[32 * i:32 * i + OBD, j * 128:(j + 1) * 128],
                        obst3[32 * i:32 * i + OBD, b * N:(b + 1) * N],
                        start=True, stop=True)
                    a1t = pa1.tile([128, N], F32R, tag="a1", name="a1")
                    nc.any.tensor_scalar(a1t[:], ps[:], cmt[(m, j)][:, b:b + 1],
                                         0.0, OP.add, OP.max)
                    a1[m].append(a1t)
            return a1

        def stage_l2(b, m, a1_m):
            a2_m = []
            for j in range(2):
                ps = pp512.tile([128, N], F32, tag="ps512", name="l2ps")
                for ks in range(2):
                    nc.tensor.matmul(
                        ps[:],
                        sb[f"w2_{m}"][:, ks * H + j * 128: ks * H + j * 128 + 128],
                        a1_m[ks][:], start=(ks == 0), stop=(ks == 1))
                a2t = pa2.tile([128, N], F32R, tag="a2", name="a2")
                nc.any.tensor_scalar(a2t[:], ps[:],
                                     sb[f"b2c_{m}"][:, j:j + 1], 0.0,
                                     OP.add, OP.max)
                a2_m.append(a2t)
            return a2_m

        def stage_Y(b, a2k):
            # Y = G^T-matmul of a2k; scores = Y^T a2q (q/k L3 eliminated)
            Y = []
            for j in range(2):
                ps = ppl1.tile([128, N], F32, tag="psl1", name="yps")
                for ks in range(2):
                    nc.tensor.matmul(
                        ps[:],
                        sb["g_r2"][:, ks * H + j * 128: ks * H + j * 128 + 128],
                        a2k[ks][:], start=(ks == 0), stop=(ks == 1))
                yt = pqk.tile([128, N], F32R, tag="YT", name="yt")
                nc.any.tensor_scalar(yt[:], ps[:], 0.0, None, OP.add)
                Y.append(yt)
            return Y

        def stage_tT(b, a2k):
            # per-key score bias t = (W3k b3q)^T a2k, computed transposed
            # [128,1] per key chunk and merged with the mask bias for exp
            eb = []
            for kc in range(NCH):
                tps = ppa.tile([128, 2], F32, tag="psa", name="tps")
                for j in range(2):
                    nc.tensor.matmul(tps[:],
                                     a2k[j][:, kc * 128:(kc + 1) * 128],
                                     sb["u2"][:, 2 * j:2 * j + 2],
                                     start=(j == 0), stop=(j == 1))
                e = ptiny.tile([128, 1], F32, tag="ebias", name="ebias")
                nc.vector.tensor_tensor(
                    e[:], tps[:, 0:1],
                    sb["negbT"][:, kc * BPC + b: kc * BPC + b + 1], OP.add)
                eb.append(e)
            return eb

        def stage_l3v(b, a2_m):
            # flipped layout v[keys, h] (+bias via ones-row matmul, relu);
            # col H..H+2 set to 1.0: col H gives the softmax denominator in
            # the attnout matmul; col H+1 is fp32r even-free-size padding.
            v_sb = []
            for rc in range(NCH):
                ps = ppa.tile([128, H + 2], F32, tag="psa", name="vps")
                for ks in range(2):
                    nc.tensor.matmul(ps[:, 0:H],
                                     a2_m[ks][:, rc * 128:(rc + 1) * 128],
                                     sb["w3_v"][:, ks * H:(ks + 1) * H],
                                     start=(ks == 0), stop=False)
                nc.tensor.matmul(ps[:], sb["ones128"][:], sb["b3v"][:],
                                 start=False, stop=True)
                vt = pv.tile([128, H + 2], F32R, tag="vsb", name="vsb")
                # relu covers the ones columns too: max(1, 0) = 1
                nc.vector.tensor_scalar(vt[:], ps[:], 0.0, None, OP.max)
                v_sb.append(vt)
            return v_sb

        def stage_scores(b, Y, a2q, eb):
            E = []
            for kc in range(NCH):
                ps = pp512.tile([128, N], F32, tag="ps512", name="scps")
                for j in range(2):
                    nc.tensor.matmul(ps[:],
                                     Y[j][:, kc * 128:(kc + 1) * 128],
                                     a2q[j][:], start=(j == 0), stop=(j == 1))
                e = pE.tile([128, N], F32R, tag="E", name="E")
                nc.scalar.activation(e[:], ps[:], AF.Exp, bias=eb[kc][:])
                E.append(e)
            return E

        def stage_attnout(b, E, v_sb):
            scaled = []
            for qc in range(NCH):
                ps = ppa.tile([128, H + 2], F32, tag="psa", name="aops")
                for kc in range(NCH):
                    nc.tensor.matmul(ps[:],
                                     E[kc][:, qc * 128:(qc + 1) * 128],
                                     v_sb[kc][:], start=(kc == 0),
                                     stop=(kc == NCH - 1))
                # every batch elem has >0 valid keys (verified on host data),
                # so S>0 and the reciprocal is finite
                rec = ptiny.tile([128, 1], F32, tag="rec", name="rec")
                nc.vector.reciprocal(rec[:], ps[:, H:H + 1])
                sc = psc.tile([128, H], F32, tag="scaled", name="scaled")
                nc.any.tensor_scalar(
                    sc[:], ps[:, 0:H], rec[:],
                    sb["maskT"][:, qc * BPC + b: qc * BPC + b + 1],
                    OP.mult, OP.mult)
                scaled.append(sc)
            return scaled

        def stage_pool_max(b, scaled):
            # pairwise max tree over the NCH scaled chunks; final node f32r
            work = list(scaled)
            cnt = 0
            while len(work) > 2:
                m01 = pm.tile([128, H], F32, tag=f"mx{cnt % 2}", name="mx")
                nc.any.tensor_tensor(m01[:], work[0][:], work[1][:], OP.max)
                work = [m01] + work[2:]
                cnt += 1
            m3 = pm.tile([128, H], F32R, tag="m3", name="m3", bufs=6)
            if len(work) == 2:
                nc.any.tensor_tensor(m3[:], work[0][:], work[1][:], OP.max)
            else:
                nc.any.tensor_scalar(m3[:], work[0][:], 0.0, None, OP.add)
            return m3

        def stage_pool_reduce(b, m3):
            # emitted one pair late: keeps the PE transposes (which wait on
            # the DVE max chain) from stalling the next pair's L1 matmuls
            for hc in range(2):
                trp = ppa.tile([128, 128], F32R, tag="psa", name="trp")
                nc.tensor.transpose(trp[:], m3[:, hc * 128:(hc + 1) * 128],
                                    sb["iden"][:])
                nc.vector.tensor_reduce(pooledT[hc][:, b:b + 1], trp[:],
                                        mybir.AxisListType.X, OP.max)

        pending_pool = []
        pairs = [((2 * p) % BPC, (2 * p + 1) % BPC)
                 for p in range(REPEAT * (BPC // 2))]
        next_a1 = None
        for pi, bb in enumerate(pairs):
            st = {b: {} for b in bb}
            if next_a1 is None:
                next_a1 = {b: stage_l1(b) for b in bb}
                next_a2q = {b: stage_l2(b, "q", next_a1[b]["q"]) for b in bb}
                next_a2k = {b: stage_l2(b, "k", next_a1[b]["k"]) for b in bb}
            for b in bb:
                for m in ("q", "k", "v"):
                    st[b][f"a1{m}"] = next_a1[b][m]
                st[b]["a2q"] = next_a2q[b]
                st[b]["a2k"] = next_a2k[b]
            next_a1 = None
            done_pending = False
            for b in bb:
                st[b]["Y"] = stage_Y(b, st[b]["a2k"])
                st[b]["eb"] = stage_tT(b, st[b]["a2k"])
            for m in ("v",):
                for b in bb:
                    st[b][f"a2{m}"] = stage_l2(b, m, st[b][f"a1{m}"])
                if not done_pending:
                    # previous pair's pool transposes, emitted here so they
                    # never stall this pair's L1/L2 matmuls on the PE queue
                    for pb, pm3 in pending_pool:
                        stage_pool_reduce(pb, pm3)
                    pending_pool = []
                    done_pending = True
                for b in bb:
                    if m == "v":
                        st[b]["v"] = stage_l3v(b, st[b]["a2v"])
            for b in bb:
                st[b]["E"] = stage_scores(b, st[b]["Y"], st[b]["a2q"], st[b]["eb"])
            if pi + 1 < len(pairs):
                # emit the next pair's L1 (and its q-branch L2) here: they
                # fill the PE bubble while this pair's exp chain produces E
                # for attnout -- sized for silicon, where the packed L1 runs
                # ~3x faster than the cost model charges
                next_a1 = {b: stage_l1(b) for b in pairs[pi + 1]}
                next_a2q = {b: stage_l2(b, "q", next_a1[b]["q"])
                            for b in pairs[pi + 1]}
                next_a2k = {b: stage_l2(b, "k", next_a1[b]["k"])
                            for b in pairs[pi + 1]}
            for b in bb:
                st[b]["sc"] = stage_attnout(b, st[b]["E"], st[b]["v"])
            for b in bb:
                pending_pool.append((b, stage_pool_max(b, st[b]["sc"])))
        for pb, pm3 in pending_pool:
            stage_pool_reduce(pb, pm3)

        # ---- head MLP on all 32 batch elems (transposed [h, b]) ----
        a1h = []
        for j in range(2):
            ps = ppsm.tile([128, BPC], F32, tag="psa")
            nc.tensor.matmul(ps[:], sb["w1h_a"][:, j * 128:(j + 1) * 128],
                             pooledT[0][:], start=True, stop=False)
            nc.tensor.matmul(ps[:], sb["w1h_b"][:, j * 128:(j + 1) * 128],
                             pooledT[1][:], start=False, stop=False)
            nc.tensor.matmul(ps[:], sb["w1h_c"][:, j * 128:(j + 1) * 128],
                             sb["headxT"][:], start=False, stop=True)
            a = pout.tile([128, BPC], F32R, tag="a1h")
            nc.scalar.activation(a[:], ps[:], AF.Relu, bias=sb["b1hc"][:, j:j + 1])
            a1h.append(a)
        a2h = []
        for j in range(2):
            ps = ppsm.tile([128, BPC], F32, tag="psa")
            for ks in range(2):
                nc.tensor.matmul(ps[:],
                                 sb["w2h"][:, ks * H + j * 128: ks * H + j * 128 + 128],
                                 a1h[ks][:], start=(ks == 0), stop=(ks == 1))
            a = pout.tile([128, BPC], F32R, tag="a2h")
            nc.scalar.activation(a[:], ps[:], AF.Relu, bias=sb["b2hc"][:, j:j + 1])
            a2h.append(a)
        ps = ppsm.tile([1, BPC], F32, tag="psa")
        for ks in range(2):
            nc.tensor.matmul(ps[:], sb["w3h"][:, ks:ks + 1], a2h[ks][:],
                             start=(ks == 0), stop=(ks == 1))
        ot = pout.tile([1, BPC], F32, tag="osb")
        nc.vector.tensor_scalar(ot[:], ps[:], sb["b3h"][:, 0:1], None, OP.add)
        nc.sync.dma_start(out_dram[:], ot[:])


def kernel(obs, obstacles, act, q_params, k_params, v_params, head_params):
    global _last_results
    maxvalid = int((np.asarray(obstacles)[:, OBD, :] > 0).sum(axis=1).max())
    _set_n(min(NFULL, max(128, -(-maxvalid // 128) * 128)))
    shared = _prep_shared(q_params, k_params, v_params, head_params)
    in_maps = []
    for c in range(N_CORES):
        m = dict(shared)
        m.update(_prep_core(obs, obstacles, act, c))
        in_maps.append(m)
    nc = _build()
    res = run_bass_kernel_spmd(nc, in_maps, core_ids=list(range(N_CORES)))
    _last_results = res
    out = np.concatenate([res.results[c]["out"][0] for c in range(N_CORES)])
    return out.astype(np.float32)
